# revision 1
# baseline (speedup 1.0000x reference)
"""Trainium2 Bass kernel for a 3-block GPT (B=2,T=2048,E=1024,H=16,V=32000).

Sharding: sequence-parallel over 8 cores (512 tokens each, weights replicated).
Per layer: QKV local, group-local AllGather of K^T and V (per-batch groups
[[0-3],[4-7]] so the SPMD program is identical on every core), attention in
scores-transposed layout (softmax denominator via ones-matmul), proj/FFN with
weights stationary, LayerNorm in transposed layout via ones-matmul partition
reductions. lm_head produces logits^T [V, 512] per core; host reassembles.
Per-core differences (token ids, causal masks) enter via input data only.
"""

import numpy as np
import ml_dtypes
from contextlib import ExitStack

import concourse.bass as bass
import concourse.mybir as mybir
import concourse.tile as tile
from concourse import bacc
from concourse.masks import make_identity
from concourse import bass_utils

# model dims (hardcoded; harness contract)
B, T, E, H, V = 2, 2048, 1024, 16, 32000
HD, L = 64, 3
NC = 8
S = (B * T) // NC        # 512 tokens per core
CH = NC // B             # 4 chunks (cores) per batch
FF = 4 * E               # 4096
EPS = 1e-5
P = 128
NE = E // P              # 8 e-tiles
NHP = H // 2             # 8 head pairs
NSB = T // P             # 16 key blocks per batch
NVT = V // P             # 250 vocab tiles
NFT = FF // P            # 32 f-tiles
F32 = mybir.dt.float32
BF16 = mybir.dt.bfloat16
I32 = mybir.dt.int32
AF = mybir.ActivationFunctionType
OP = mybir.AluOpType


def _ln_tiles(nc, tc, pools, src, out, g_t, b_t, ones_f, eps_t):
    """LayerNorm over E (partition axis across the 8 [128,S] tiles of src).

    src/out: lists of 8 SBUF tiles [128, S] (xT layout). g_t/b_t: [128, NE]
    param tiles; gcol/bcol: column index. Stats via ones-matmul partition
    reduction broadcast to all 128 partitions."""
    pacc, tp = pools["pacc"], pools["tp"]
    psm = pacc.tile([P, S], F32, tag="ps_a", name="ln_ps", bufs=6)
    pss = pacc.tile([P, S], F32, tag="ps_a", name="ln_ps", bufs=6)
    for e in range(NE):
        nc.tensor.matmul(psm[:], lhsT=ones_f[:], rhs=src[e][:],
                         start=(e == 0), stop=(e == NE - 1), skip_group_check=True)
        sq = tp.tile([P, S], F32, tag="ln_sq", name="ln_sq", bufs=2)
        nc.scalar.square(sq[:], src[e][:])
        nc.tensor.matmul(pss[:], lhsT=ones_f[:], rhs=sq[:],
                         start=(e == 0), stop=(e == NE - 1), skip_group_check=True)
    mean = tp.tile([P, S], F32, tag="ln_mean", name="ln_mean")
    nc.scalar.mul(mean[:], psm[:], 1.0 / E)
    msq = tp.tile([P, S], F32, tag="ln_msq", name="ln_msq")
    nc.scalar.square(msq[:], mean[:])
    var = tp.tile([P, S], F32, tag="ln_var", name="ln_var")
    nc.vector.tensor_scalar(var[:], pss[:], 1.0 / E, None, OP.mult)
    nc.vector.tensor_tensor(out=var[:], in0=var[:], in1=msq[:], op=OP.subtract)
    std = tp.tile([P, S], F32, tag="ln_std", name="ln_std")
    nc.scalar.activation(std[:], var[:], AF.Sqrt, bias=eps_t[:])
    rstd = tp.tile([P, S], F32, tag="ln_rstd", name="ln_rstd")
    nc.vector.reciprocal(rstd[:], std[:])
    for e in range(NE):
        t = tp.tile([P, S], F32, tag="ln_t", name="ln_t", bufs=2)
        nc.vector.tensor_tensor(out=t[:], in0=src[e][:], in1=mean[:], op=OP.subtract)
        nc.vector.tensor_tensor(out=t[:], in0=t[:], in1=rstd[:], op=OP.mult)
        nc.vector.tensor_scalar(out[e][:], t[:], g_t[:, e:e + 1],
                                b_t[:, e:e + 1], OP.mult, OP.add)


def build_program():
    nc = bacc.Bacc("TRN2", target_bir_lowering=False, debug=False, num_devices=NC)

    # ---- DRAM I/O ----
    d_wqr = nc.dram_tensor("wqr", [L, NE, E, P], BF16, kind="ExternalInput")
    d_wkr = nc.dram_tensor("wkr", [L, NE, E, P], BF16, kind="ExternalInput")
    d_wvf = nc.dram_tensor("wvf", [L, E, H * HD], BF16, kind="ExternalInput")
    d_wor = nc.dram_tensor("wor", [L, NE, E, P], BF16, kind="ExternalInput")
    d_w1r = nc.dram_tensor("w1r", [L, NFT, E, P], BF16, kind="ExternalInput")
    d_w2r = nc.dram_tensor("w2r", [L, NE, FF, P], BF16, kind="ExternalInput")
    d_wlm = nc.dram_tensor("wlmr", [NVT, E, P], BF16, kind="ExternalInput")
    d_emb = nc.dram_tensor("emb", [V, E], F32, kind="ExternalInput")
    d_idx = nc.dram_tensor("idx", [S], I32, kind="ExternalInput")
    d_msk = nc.dram_tensor("maskp", [NSB, P, S], BF16, kind="ExternalInput")
    d_ln1g = nc.dram_tensor("ln1g", [L, E], F32, kind="ExternalInput")
    d_ln1b = nc.dram_tensor("ln1b", [L, E], F32, kind="ExternalInput")
    d_ln2g = nc.dram_tensor("ln2g", [L, E], F32, kind="ExternalInput")
    d_ln2b = nc.dram_tensor("ln2b", [L, E], F32, kind="ExternalInput")
    d_bo = nc.dram_tensor("bo", [L, E], F32, kind="ExternalInput")
    d_b1 = nc.dram_tensor("b1", [L, FF], F32, kind="ExternalInput")
    d_b2 = nc.dram_tensor("b2", [L, E], F32, kind="ExternalInput")
    d_lnfg = nc.dram_tensor("lnfg", [E], F32, kind="ExternalInput")
    d_lnfb = nc.dram_tensor("lnfb", [E], F32, kind="ExternalInput")
    d_blm = nc.dram_tensor("blm", [V], F32, kind="ExternalInput")
    d_out = nc.dram_tensor("logt", [V, S], F32, kind="ExternalOutput")

    groups = [[0, 1, 2, 3], [4, 5, 6, 7]]

    with ExitStack() as ctx:
        tc = ctx.enter_context(tile.TileContext(nc, num_cores=NC))
        const = ctx.enter_context(tc.tile_pool(name="const", bufs=1))
        pp_x = ctx.enter_context(tc.tile_pool(name="xres", bufs=1))
        pp_sum = ctx.enter_context(tc.tile_pool(name="xsum", bufs=1))
        pp_msk = ctx.enter_context(tc.tile_pool(name="masks", bufs=1))
        tp = ctx.enter_context(tc.tile_pool(name="tp", bufs=1))
        wp = ctx.enter_context(tc.tile_pool(name="wstream", bufs=1))
        pacc = ctx.enter_context(tc.tile_pool(name="pacc", bufs=1, space="PSUM"))
        dram = ctx.enter_context(tc.tile_pool(name="ccdram", bufs=2, space="DRAM"))
        pools = {"pacc": pacc, "tp": tp}

        ident = const.tile([P, P], F32, name="ident")
        make_identity(nc, ident[:])
        ones_f = const.tile([P, P], F32, name="ones_f")
        nc.vector.memset(ones_f[:], 1.0)
        ones_b = const.tile([P, HD], BF16, name="ones_b")
        nc.vector.memset(ones_b[:], 1.0)
        eps_t = const.tile([P, 1], F32, name="eps_t")
        nc.vector.memset(eps_t[:], EPS)

        # params -> [128, n] tiles
        def ldvec(dt_ap, n, name):
            t = const.tile([P, n], F32, tag=name, name=name)
            nc.sync.dma_start(out=t[:], in_=dt_ap.rearrange("(a p) -> p a", p=P))
            return t

        t_ln1g = [ldvec(d_ln1g.ap()[l], NE, f"ln1g{l}") for l in range(L)]
        t_ln1b = [ldvec(d_ln1b.ap()[l], NE, f"ln1b{l}") for l in range(L)]
        t_ln2g = [ldvec(d_ln2g.ap()[l], NE, f"ln2g{l}") for l in range(L)]
        t_ln2b = [ldvec(d_ln2b.ap()[l], NE, f"ln2b{l}") for l in range(L)]
        t_bo = [ldvec(d_bo.ap()[l], NE, f"bo{l}") for l in range(L)]
        t_b1 = [ldvec(d_b1.ap()[l], NFT, f"b1{l}") for l in range(L)]
        t_b2 = [ldvec(d_b2.ap()[l], NE, f"b2{l}") for l in range(L)]
        t_lnfg = ldvec(d_lnfg.ap(), NE, "lnfg")
        t_lnfb = ldvec(d_lnfb.ap(), NE, "lnfb")
        t_blm = ldvec(d_blm.ap(), NVT, "blm")

        # causal masks (bf16 multiplicative, per-core data)
        mask_t = []
        for sb in range(NSB):
            m = pp_msk.tile([P, S], BF16, tag=f"msk{sb}", name=f"msk{sb}")
            nc.sync.dma_start(out=m[:], in_=d_msk.ap()[sb])
            mask_t.append(m)

        # residual stream xT: 8 tiles [128, S]
        xT = [pp_x.tile([P, S], F32, tag=f"x{e}", name=f"x{e}") for e in range(NE)]
        sum_t = [pp_sum.tile([P, S], F32, tag=f"s{e}", name=f"s{e}") for e in range(NE)]

        # ---- embedding gather + transpose into xT ----
        idx_t = const.tile([P, S // P], I32, name="idx_t")
        nc.sync.dma_start(out=idx_t[:], in_=d_idx.ap().rearrange("(g p) -> p g", p=P))
        for g in range(S // P):
            xg = tp.tile([P, E], F32, tag="embg", name="embg", bufs=2)
            nc.gpsimd.indirect_dma_start(
                out=xg[:], out_offset=None, in_=d_emb.ap(),
                in_offset=bass.IndirectOffsetOnAxis(ap=idx_t[:, g:g + 1], axis=0))
            for e in range(NE):
                pst = pacc.tile([P, S], F32, tag="ps_a", name="tpose", bufs=6)
                nc.tensor.transpose(pst[:, 0:P], xg[:, e * P:(e + 1) * P], ident[:])
                nc.vector.tensor_copy(xT[e][:, g * P:(g + 1) * P], pst[:, 0:P])

        # ---- transformer blocks ----
        for l in range(L):
            # q projection (W-stationary): qT tiles [128, S]
            with tc.tile_pool(name="qT", bufs=1) as pp_q, \
                 tc.tile_pool(name="oT", bufs=1) as pp_o, \
                 tc.tile_pool(name="attn", bufs=1) as ap_t, \
                 tc.tile_pool(name="vslab", bufs=1) as vp:

                # k projection -> kT_local -> DRAM
                xA = []
                for e in range(NE):
                    xa = ap_t.tile([P, S], BF16, tag=f"xa{e}", name=f"xa{e}")
                    nc.vector.tensor_copy(xa[:], xT[e][:])
                    xA.append(xa)
                kvink = dram.tile([E, S], BF16, tag="kvink", name="kvink")
                for m in range(NE):
                    w = wp.tile([P, NE, P], BF16, tag="wb", name="wb", bufs=3)
                    nc.sync.dma_start(out=w[:], in_=d_wkr.ap()[l, m].rearrange(
                        "(ko p) f -> p ko f", p=P))
                    ps = pacc.tile([P, S], F32, tag="ps_a", name="acc", bufs=6)
                    for k in range(NE):
                        nc.tensor.matmul(ps[:], lhsT=w[:, k, :], rhs=xA[k][:],
                                         start=(k == 0), stop=(k == NE - 1))
                    kl = tp.tile([P, S], BF16, tag="klocal", name="klocal", bufs=2)
                    nc.vector.tensor_copy(kl[:], ps[:])
                    nc.sync.dma_start(out=kvink[m * P:(m + 1) * P, :], in_=kl[:])

                # v projection (x-stationary) -> v_local bf16 -> DRAM
                kvinv = dram.tile([S, E], BF16, tag="kvinv", name="kvinv")
                for n in range(2):
                    psv = [pacc.tile([P, 512], F32, tag="ps_a", name="acc", bufs=6) for _ in range(4)]
                    for k in range(NE):
                        wv = wp.tile([P, 512], BF16, tag="wv", name="wv", bufs=3)
                        nc.sync.dma_start(
                            out=wv[:],
                            in_=d_wvf.ap()[l, k * P:(k + 1) * P, n * 512:(n + 1) * 512])
                        for m in range(4):
                            nc.tensor.matmul(psv[m][:], lhsT=xA[k][:, m * P:(m + 1) * P],
                                             rhs=wv[:], start=(k == 0), stop=(k == NE - 1))
                    for m in range(4):
                        vv = tp.tile([P, 512], BF16, tag="vlocal", name="vlocal", bufs=2)
                        nc.vector.tensor_copy(vv[:], psv[m][:])
                        nc.sync.dma_start(
                            out=kvinv[m * P:(m + 1) * P, n * 512:(n + 1) * 512], in_=vv[:])

                # group-local AllGather of kT and v (per-batch groups)
                kvoutk = dram.tile([CH * E, S], BF16, tag="kvoutk", name="kvoutk")
                kvoutv = dram.tile([T, E], BF16, tag="kvoutv", name="kvoutv")
                nc.gpsimd.collective_compute(
                    "AllGather", OP.bypass, replica_groups=groups,
                    ins=[kvink[:]], outs=[kvoutk[:]])
                nc.gpsimd.collective_compute(
                    "AllGather", OP.bypass, replica_groups=groups,
                    ins=[kvinv[:]], outs=[kvoutv[:]])

                qT = []
                for m in range(NE):
                    w = wp.tile([P, NE, P], BF16, tag="wb", name="wb", bufs=3)
                    nc.sync.dma_start(out=w[:], in_=d_wqr.ap()[l, m].rearrange(
                        "(ko p) f -> p ko f", p=P))
                    ps = pacc.tile([P, S], F32, tag="ps_a", name="acc", bufs=6)
                    for k in range(NE):
                        nc.tensor.matmul(ps[:], lhsT=w[:, k, :], rhs=xA[k][:],
                                         start=(k == 0), stop=(k == NE - 1))
                    q = pp_q.tile([P, S], BF16, tag=f"q{m}", name=f"q{m}")
                    nc.vector.tensor_copy(q[:], ps[:])
                    qT.append(q)


                # attention, pair-major; all 16 key blocks on every core
                oT = []
                for p in range(NHP):
                    pav = pacc.tile([P, S], F32, tag="ps_b", name="pav", bufs=2)
                    prs = pacc.tile([P, S], F32, tag="ps_b", name="prs", bufs=2)
                    vsl = []
                    for sb in range(NSB):
                        v = vp.tile([P, P], BF16, tag="vsl", name="vsl", bufs=20)
                        nc.sync.dma_start(
                            out=v[:],
                            in_=kvoutv[sb * P:(sb + 1) * P, p * P:(p + 1) * P])
                        vsl.append(v)
                    for cc in range(CH):
                        ksl = ap_t.tile([P, S], BF16, tag="kslab", name="kslab", bufs=3)
                        nc.sync.dma_start(
                            out=ksl[:],
                            in_=kvoutk[cc * E + p * P:cc * E + (p + 1) * P, :])
                        for j2 in range(4):
                            sb = 4 * cc + j2
                            first, last = (sb == 0), (sb == NSB - 1)
                            sA = pacc.tile([P, S], F32, tag="ps_a", name="sc", bufs=6)
                            sB = pacc.tile([P, S], F32, tag="ps_a", name="sc", bufs=6)
                            nc.tensor.matmul(
                                sA[:], lhsT=ksl[0:HD, j2 * P:(j2 + 1) * P],
                                rhs=qT[p][0:HD, :], tile_position=(0, 0))
                            nc.tensor.matmul(
                                sB[:], lhsT=ksl[HD:P, j2 * P:(j2 + 1) * P],
                                rhs=qT[p][HD:P, :], tile_position=(64, 0))
                            pa = ap_t.tile([P, S], BF16, tag="pt", name="pt", bufs=4)
                            pb = ap_t.tile([P, S], BF16, tag="pt", name="pt", bufs=4)
                            nc.scalar.activation(pa[:], sA[:], AF.Exp, scale=HD ** -0.5)
                            nc.scalar.activation(pb[:], sB[:], AF.Exp, scale=HD ** -0.5)
                            nc.vector.tensor_tensor(out=pa[:], in0=pa[:],
                                                    in1=mask_t[sb][:], op=OP.mult)
                            nc.vector.tensor_tensor(out=pb[:], in0=pb[:],
                                                    in1=mask_t[sb][:], op=OP.mult)
                            nc.tensor.matmul(
                                pav[0:HD, :], lhsT=vsl[sb][:, 0:HD],
                                rhs=pa[:], start=first, stop=last,
                                tile_position=(0, 0), skip_group_check=True)
                            nc.tensor.matmul(
                                pav[HD:P, :], lhsT=vsl[sb][:, HD:P],
                                rhs=pb[:], start=first, stop=last,
                                tile_position=(0, 64), skip_group_check=True)
                            nc.tensor.matmul(
                                prs[0:HD, :], lhsT=ones_b[:], rhs=pa[:],
                                start=first, stop=last,
                                tile_position=(0, 0), skip_group_check=True)
                            nc.tensor.matmul(
                                prs[HD:P, :], lhsT=ones_b[:], rhs=pb[:],
                                start=first, stop=last,
                                tile_position=(0, 64), skip_group_check=True)
                    rec = tp.tile([P, S], F32, tag="rec", name="rec", bufs=2)
                    nc.vector.reciprocal(rec[:], prs[:])
                    o = pp_o.tile([P, S], BF16, tag=f"o{p}", name=f"o{p}")
                    nc.vector.tensor_tensor(out=o[:], in0=pav[:], in1=rec[:], op=OP.mult)
                    oT.append(o)

                # output projection + bias + residual
                for m in range(NE):
                    w = wp.tile([P, NE, P], BF16, tag="wb", name="wb", bufs=3)
                    nc.sync.dma_start(out=w[:], in_=d_wor.ap()[l, m].rearrange(
                        "(ko p) f -> p ko f", p=P))
                    ps = pacc.tile([P, S], F32, tag="ps_a", name="acc", bufs=6)
                    for k in range(NE):
                        nc.tensor.matmul(ps[:], lhsT=w[:, k, :], rhs=oT[k][:],
                                         start=(k == 0), stop=(k == NE - 1))
                    yv = tp.tile([P, S], F32, tag="yv", name="yv", bufs=2)
                    nc.scalar.add(yv[:], ps[:], t_bo[l][:, m:m + 1])
                    nc.vector.tensor_tensor(out=sum_t[m][:], in0=yv[:], in1=xT[m][:],
                                            op=OP.add)

            _ln_tiles(nc, tc, pools, sum_t, xT, t_ln1g[l], t_ln1b[l], ones_f, eps_t)

            # FFN (bf16 matmuls, fp32 psum + residual)
            with tc.tile_pool(name="ht", bufs=1) as pp_h, \
                 tc.tile_pool(name="xbf", bufs=1) as pp_xbf:
                xF = []
                for e in range(NE):
                    xf = pp_xbf.tile([P, S], BF16, tag=f"xf{e}", name=f"xf{e}")
                    nc.vector.tensor_copy(xf[:], xT[e][:])
                    xF.append(xf)
                hT = []
                for f in range(NFT):
                    w = wp.tile([P, NE, P], BF16, tag="wb", name="wb", bufs=3)
                    nc.sync.dma_start(out=w[:], in_=d_w1r.ap()[l, f].rearrange(
                        "(ko p) f2 -> p ko f2", p=P))
                    ps = pacc.tile([P, S], F32, tag="ps_a", name="acc", bufs=6)
                    for k in range(NE):
                        nc.tensor.matmul(ps[:], lhsT=w[:, k, :], rhs=xF[k][:],
                                         start=(k == 0), stop=(k == NE - 1))
                    h = pp_h.tile([P, S], BF16, tag=f"h{f}", name=f"h{f}")
                    nc.scalar.activation(h[:], ps[:], AF.Relu, bias=t_b1[l][:, f:f + 1])
                    hT.append(h)
                for m in range(NE):
                    ps = pacc.tile([P, S], F32, tag="ps_a", name="acc", bufs=6)
                    for fg in range(4):
                        w = wp.tile([P, NE, P], BF16, tag="wb", name="wb", bufs=3)
                        nc.sync.dma_start(
                            out=w[:],
                            in_=d_w2r.ap()[l, m, fg * E:(fg + 1) * E, :].rearrange(
                                "(ko p) f2 -> p ko f2", p=P))
                        for k8 in range(NE):
                            f = fg * NE + k8
                            nc.tensor.matmul(ps[:], lhsT=w[:, k8, :], rhs=hT[f][:],
                                             start=(f == 0), stop=(f == NFT - 1))
                    yv = tp.tile([P, S], F32, tag="yv", name="yv", bufs=2)
                    nc.scalar.add(yv[:], ps[:], t_b2[l][:, m:m + 1])
                    nc.vector.tensor_tensor(out=sum_t[m][:], in0=yv[:], in1=xT[m][:],
                                            op=OP.add)
            _ln_tiles(nc, tc, pools, sum_t, xT, t_ln2g[l], t_ln2b[l], ones_f, eps_t)

        # ---- final LN + lm_head ----
        _ln_tiles(nc, tc, pools, xT, sum_t, t_lnfg, t_lnfb, ones_f, eps_t)
        with tc.tile_pool(name="lg", bufs=1) as pp_lg, \
             tc.tile_pool(name="xb", bufs=1) as pp_xb:
            xB = []
            for e in range(NE):
                xb = pp_xb.tile([P, S], BF16, tag=f"xb{e}", name=f"xb{e}")
                nc.vector.tensor_copy(xb[:], sum_t[e][:])
                xB.append(xb)
            for vt in range(NVT):
                w = wp.tile([P, NE, P], BF16, tag="wlmt", name="wlmt", bufs=3)
                nc.sync.dma_start(out=w[:], in_=d_wlm.ap()[vt].rearrange(
                    "(ko p) f -> p ko f", p=P))
                ps = pacc.tile([P, S], F32, tag="ps_a", name="acc", bufs=6)
                for k in range(NE):
                    nc.tensor.matmul(ps[:], lhsT=w[:, k, :], rhs=xB[k][:],
                                     start=(k == 0), stop=(k == NE - 1))
                lg = pp_lg.tile([P, S], F32, tag="lg", name="lg", bufs=4)
                nc.scalar.add(lg[:], ps[:], t_blm[:, vt:vt + 1])
                nc.sync.dma_start(out=d_out.ap()[vt * P:(vt + 1) * P, :], in_=lg[:])

    nc.compile()
    return nc


_CACHED = {}


def _prep_weights(inputs):
    f32 = np.float32
    Wq, Wk, Wv = inputs["Wq"], inputs["Wk"], inputs["Wv"]
    wq_flat = np.ascontiguousarray(Wq.transpose(0, 2, 1, 3).reshape(L, E, H * HD))
    wk_flat = np.ascontiguousarray(Wk.transpose(0, 2, 1, 3).reshape(L, E, H * HD))
    wv_flat = np.ascontiguousarray(Wv.transpose(0, 2, 1, 3).reshape(L, E, H * HD))
    wqr = np.ascontiguousarray(
        wq_flat.reshape(L, E, NE, P).transpose(0, 2, 1, 3)).astype(f32)
    wkr = np.ascontiguousarray(
        wk_flat.reshape(L, E, NE, P).transpose(0, 2, 1, 3)).astype(f32)
    wor = np.ascontiguousarray(
        inputs["Wo"].reshape(L, E, NE, P).transpose(0, 2, 1, 3)).astype(f32)
    w1r = np.ascontiguousarray(
        inputs["W1"].reshape(L, E, NFT, P).transpose(0, 2, 1, 3)).astype(f32)
    w2r = np.ascontiguousarray(
        inputs["W2"].reshape(L, FF, NE, P).transpose(0, 2, 1, 3)).astype(f32)
    wlmr = np.ascontiguousarray(
        inputs["Wlm"].reshape(E, NVT, P).transpose(1, 0, 2)).astype(f32)
    import ml_dtypes as _md
    bf = _md.bfloat16
    return {
        "wqr": wqr.astype(bf), "wkr": wkr.astype(bf), "wvf": wv_flat.astype(bf), "wor": wor.astype(bf),
        "w1r": w1r.astype(ml_dtypes.bfloat16), "w2r": w2r.astype(ml_dtypes.bfloat16), "wlmr": wlmr.astype(ml_dtypes.bfloat16),
        "emb": np.ascontiguousarray(inputs["emb"]).astype(f32),
        "ln1g": np.ascontiguousarray(inputs["ln1_g"]).astype(f32),
        "ln1b": np.ascontiguousarray(inputs["ln1_b"]).astype(f32),
        "ln2g": np.ascontiguousarray(inputs["ln2_g"]).astype(f32),
        "ln2b": np.ascontiguousarray(inputs["ln2_b"]).astype(f32),
        "bo": np.ascontiguousarray(inputs["bo"]).astype(f32),
        "b1": np.ascontiguousarray(inputs["b1"]).astype(f32),
        "b2": np.ascontiguousarray(inputs["b2"]).astype(f32),
        "lnfg": np.ascontiguousarray(inputs["lnf_g"]).astype(f32),
        "lnfb": np.ascontiguousarray(inputs["lnf_b"]).astype(f32),
        "blm": np.ascontiguousarray(inputs["blm"]).astype(f32),
    }


def kernel(**inputs):
    if "nc" not in _CACHED:
        _CACHED["nc"] = build_program()
    nc = _CACHED["nc"]

    shared = _prep_weights(inputs)
    index = np.asarray(inputs["index"])

    # per-core causal masks and token ids
    jpos = np.arange(S)
    ipos = np.arange(P)
    in_maps = []
    for c in range(NC):
        b, j = c // CH, c % CH
        q0 = j * S
        m = np.zeros((NSB, P, S), np.float32)
        for sb in range(NSB):
            m[sb] = ((sb * P + ipos)[:, None] <= (q0 + jpos)[None, :])
        im = dict(shared)
        im["maskp"] = m.astype(ml_dtypes.bfloat16)
        im["idx"] = np.ascontiguousarray(index[b, q0:q0 + S]).astype(np.int32)
        in_maps.append(im)

    res = bass_utils.run_bass_kernel_spmd(nc, in_maps, core_ids=list(range(NC)))
    out = np.zeros((B, T, V), np.float32)
    for c in range(NC):
        b, j = c // CH, c % CH
        out[b, j * S:(j + 1) * S, :] = res.results[c]["logt"].T
    return out



# revision 13
# speedup vs baseline: 1.1974x; 1.1974x over previous
"""Trainium2 Bass kernel for a 3-block GPT (B=2,T=2048,E=1024,H=16,V=32000).

Sharding: sequence-parallel over 8 cores (512 tokens each, weights replicated).
Per layer: QKV local, group-local AllGather of concat(K^T, V) (per-batch groups
[[0-3],[4-7]] so the SPMD program is identical on every core), attention in
scores-transposed layout (softmax denominator via ones-matmul), proj/FFN with
weights stationary, LayerNorm in transposed layout via ones-matmul partition
reductions. lm_head produces logits^T [V, 512] per core in bf16; host
reassembles. Weights are host-swizzled so every streamed tile is contiguous
per partition (>=2KB DMA elements), and DMAs are batched to relieve the
HWDGE/sequencer issue path.
"""

import numpy as np
import ml_dtypes
from contextlib import ExitStack

import concourse.bass as bass
import concourse.mybir as mybir
import concourse.tile as tile
from concourse import bacc
from concourse.masks import make_identity
from concourse import bass_utils

# model dims (hardcoded; harness contract)
B, T, E, H, V = 2, 2048, 1024, 16, 32000
HD, L = 64, 3
NC = 8
S = (B * T) // NC        # 512 tokens per core
CH = NC // B             # 4 chunks (cores) per batch
FF = 4 * E               # 4096
EPS = 1e-5
P = 128
NE = E // P              # 8 e-tiles
NHP = H // 2             # 8 head pairs
NSB = T // P             # 16 key blocks per batch
NVT = V // P             # 250 vocab tiles
NVG = 50                 # vocab groups (5 tiles each)
VGT = NVT // NVG         # 5
NFT = FF // P            # 32 f-tiles
CS = 2 * E * S           # flat kv elems per core (k: E*S, v: S*E)
VOFF = E * S
F32 = mybir.dt.float32
BF16 = mybir.dt.bfloat16
FP8 = mybir.dt.float8e4
I32 = mybir.dt.int32
KVS = 64.0               # k/v fp8 transport scale (values ~0.013 at layer 1)
AF = mybir.ActivationFunctionType
OP = mybir.AluOpType


def _ln_start(pools):
    """Allocate psum accumulators for LN stats (sum, sum-of-squares)."""
    pacc = pools["pacc"]
    psm = pacc.tile([P, S], F32, tag="ps_b", name="ln_ps", bufs=2)
    pss = pacc.tile([P, S], F32, tag="ps_b", name="ln_ps", bufs=2)
    return psm, pss


def _ln_feed(nc, pools, psm, pss, src_e, e, ones_f):
    """Accumulate LN stats for e-tile `e` (call as each tile is produced)."""
    tp = pools["tp"]
    nc.tensor.matmul(psm[:], lhsT=ones_f[:], rhs=src_e[:],
                     start=(e == 0), stop=(e == NE - 1), skip_group_check=True)
    sq = tp.tile([P, S], F32, tag="ln_sq", name="ln_sq", bufs=2)
    nc.scalar.square(sq[:], src_e[:])
    nc.tensor.matmul(pss[:], lhsT=ones_f[:], rhs=sq[:],
                     start=(e == 0), stop=(e == NE - 1), skip_group_check=True)


def _ln_finish(nc, pools, psm, pss, src, out, g_t, b_t, eps_t):
    """Finalize LN: mean/var/rstd then per-tile normalize."""
    tp = pools["tp"]
    mean = tp.tile([P, S], F32, tag="ln_mean", name="ln_mean")
    nc.scalar.mul(mean[:], psm[:], 1.0 / E)
    msq = tp.tile([P, S], F32, tag="ln_msq", name="ln_msq")
    nc.scalar.square(msq[:], mean[:])
    var = tp.tile([P, S], F32, tag="ln_var", name="ln_var")
    nc.vector.tensor_scalar(var[:], pss[:], 1.0 / E, None, OP.mult)
    nc.vector.tensor_tensor(out=var[:], in0=var[:], in1=msq[:], op=OP.subtract)
    std = tp.tile([P, S], F32, tag="ln_std", name="ln_std")
    nc.scalar.activation(std[:], var[:], AF.Sqrt, bias=eps_t[:])
    rstd = tp.tile([P, S], F32, tag="ln_rstd", name="ln_rstd")
    nc.vector.reciprocal(rstd[:], std[:])
    for e in range(NE):
        t = tp.tile([P, S], F32, tag="ln_t", name="ln_t", bufs=2)
        nc.vector.tensor_tensor(out=t[:], in0=src[e][:], in1=mean[:], op=OP.subtract)
        nc.vector.tensor_tensor(out=t[:], in0=t[:], in1=rstd[:], op=OP.mult)
        nc.scalar.activation(out[e][:], t[:], AF.Identity,
                             bias=b_t[:, e:e + 1], scale=g_t[:, e:e + 1])


def _ln_tiles(nc, tc, pools, src, out, g_t, b_t, ones_f, eps_t):
    """LayerNorm over E (partition axis across the 8 [128,S] tiles of src)."""
    psm, pss = _ln_start(pools)
    for e in range(NE):
        _ln_feed(nc, pools, psm, pss, src[e], e, ones_f)
    _ln_finish(nc, pools, psm, pss, src, out, g_t, b_t, eps_t)


def build_program():
    nc = bacc.Bacc("TRN2", target_bir_lowering=False, debug=False, num_devices=NC)

    # ---- DRAM I/O (all streamed weights pre-swizzled on host so each tile is
    # contiguous per partition) ----
    d_wqr = nc.dram_tensor("wqr", [L, NE, P, NE, P], BF16, kind="ExternalInput")
    d_wkr = nc.dram_tensor("wkr", [L, NE, P, NE, P], BF16, kind="ExternalInput")
    d_wvf = nc.dram_tensor("wvf", [L, NE, P, H * HD], BF16, kind="ExternalInput")
    d_wor = nc.dram_tensor("wor", [L, NE, P, NE, P], BF16, kind="ExternalInput")
    d_w1r = nc.dram_tensor("w1r", [L, NE, P, 4, NE, P], BF16, kind="ExternalInput")
    d_w2r = nc.dram_tensor("w2r", [L, NE, P, NFT, P], BF16, kind="ExternalInput")
    d_wlm = nc.dram_tensor("wlmr", [NVG, P, VGT, NE, P], BF16, kind="ExternalInput")
    d_emb = nc.dram_tensor("emb", [V, E], BF16, kind="ExternalInput")
    d_idx = nc.dram_tensor("idx", [S], I32, kind="ExternalInput")
    d_msk = nc.dram_tensor("maskp", [NSB, P, S], BF16, kind="ExternalInput")
    d_ln1g = nc.dram_tensor("ln1g", [L, E], F32, kind="ExternalInput")
    d_ln1b = nc.dram_tensor("ln1b", [L, E], F32, kind="ExternalInput")
    d_ln2g = nc.dram_tensor("ln2g", [L, E], F32, kind="ExternalInput")
    d_ln2b = nc.dram_tensor("ln2b", [L, E], F32, kind="ExternalInput")
    d_bo = nc.dram_tensor("bo", [L, E], F32, kind="ExternalInput")
    d_b1 = nc.dram_tensor("b1", [L, FF], F32, kind="ExternalInput")
    d_b2 = nc.dram_tensor("b2", [L, E], F32, kind="ExternalInput")
    d_lnfg = nc.dram_tensor("lnfg", [E], F32, kind="ExternalInput")
    d_lnfb = nc.dram_tensor("lnfb", [E], F32, kind="ExternalInput")
    d_blm = nc.dram_tensor("blm", [V], F32, kind="ExternalInput")
    d_out = nc.dram_tensor("logt", [V, S], BF16, kind="ExternalOutput")

    groups = [[0, 1, 2, 3], [4, 5, 6, 7]]

    with ExitStack() as ctx:
        tc = ctx.enter_context(tile.TileContext(nc, num_cores=NC))
        const = ctx.enter_context(tc.tile_pool(name="const", bufs=1))
        pp_x = ctx.enter_context(tc.tile_pool(name="xres", bufs=1))
        pp_sum = ctx.enter_context(tc.tile_pool(name="xsum", bufs=1))
        pp_msk = ctx.enter_context(tc.tile_pool(name="masks", bufs=1))
        tp = ctx.enter_context(tc.tile_pool(name="tp", bufs=1))
        wp = ctx.enter_context(tc.tile_pool(name="wstream", bufs=1))
        pacc = ctx.enter_context(tc.tile_pool(name="pacc", bufs=1, space="PSUM"))
        dram = ctx.enter_context(tc.tile_pool(name="ccdram", bufs=2, space="DRAM"))
        pools = {"pacc": pacc, "tp": tp}

        # ---- embedding gather first: critical path at startup ----
        idx_t = const.tile([P, S // P], I32, name="idx_t")
        nc.sync.dma_start(out=idx_t[:], in_=d_idx.ap().rearrange("(g p) -> p g", p=P))

        ident = const.tile([P, P], F32, name="ident")
        make_identity(nc, ident[:])
        identb = const.tile([P, P], BF16, name="identb")
        nc.vector.tensor_copy(identb[:], ident[:])
        ones_f = const.tile([P, P], F32, name="ones_f")
        nc.vector.memset(ones_f[:], 1.0)
        ones_b = const.tile([P, HD], BF16, name="ones_b")
        nc.vector.memset(ones_b[:], 1.0)
        eps_t = const.tile([P, 1], F32, name="eps_t")
        nc.vector.memset(eps_t[:], EPS)

        # residual stream xT: 8 tiles [128, S]
        xT = [pp_x.tile([P, S], F32, tag=f"x{e}", name=f"x{e}") for e in range(NE)]
        sum_t = [pp_sum.tile([P, S], F32, tag=f"s{e}", name=f"s{e}") for e in range(NE)]

        for g in range(S // P):
            xg = tp.tile([P, E], BF16, tag="embg", name="embg", bufs=2)
            nc.gpsimd.indirect_dma_start(
                out=xg[:], out_offset=None, in_=d_emb.ap(),
                in_offset=bass.IndirectOffsetOnAxis(ap=idx_t[:, g:g + 1], axis=0))
            for e in range(NE):
                pst = pacc.tile([P, S], BF16, tag="ps_a", name="tpose", bufs=6)
                nc.tensor.transpose(pst[:, 0:P], xg[:, e * P:(e + 1) * P], identb[:])
                nc.vector.tensor_copy(xT[e][:, g * P:(g + 1) * P], pst[:, 0:P])

        # ---- params -> [128, n] tiles (after the gather chain is queued) ----
        def ldvec(dt_ap, n, name):
            t = const.tile([P, n], F32, tag=name, name=name)
            nc.sync.dma_start(out=t[:], in_=dt_ap.rearrange("(a p) -> p a", p=P))
            return t

        t_ln1g = [ldvec(d_ln1g.ap()[l], NE, f"ln1g{l}") for l in range(L)]
        t_ln1b = [ldvec(d_ln1b.ap()[l], NE, f"ln1b{l}") for l in range(L)]
        t_ln2g = [ldvec(d_ln2g.ap()[l], NE, f"ln2g{l}") for l in range(L)]
        t_ln2b = [ldvec(d_ln2b.ap()[l], NE, f"ln2b{l}") for l in range(L)]
        t_bo = [ldvec(d_bo.ap()[l], NE, f"bo{l}") for l in range(L)]
        t_b1 = [ldvec(d_b1.ap()[l], NFT, f"b1{l}") for l in range(L)]
        t_b2 = [ldvec(d_b2.ap()[l], NE, f"b2{l}") for l in range(L)]
        t_lnfg = ldvec(d_lnfg.ap(), NE, "lnfg")
        t_lnfb = ldvec(d_lnfb.ap(), NE, "lnfb")
        t_blm = ldvec(d_blm.ap(), NVT, "blm")

        # causal masks (bf16 multiplicative, per-core data): one DMA
        mask3 = pp_msk.tile([P, NSB, S], BF16, tag="msk", name="msk")
        nc.scalar.dma_start(out=mask3[:], in_=d_msk.ap().rearrange("sb p s -> p sb s"))

        # ---- transformer blocks ----
        for l in range(L):
            with tc.tile_pool(name="qT", bufs=1) as pp_q, \
                 tc.tile_pool(name="oT", bufs=1) as pp_o, \
                 tc.tile_pool(name="attn", bufs=1) as ap_t, \
                 tc.tile_pool(name="vslab", bufs=1) as vp:

                # bf16 activations
                xA = []
                for e in range(NE):
                    xa = ap_t.tile([P, S], BF16, tag=f"xa{e}", name=f"xa{e}")
                    nc.vector.tensor_copy(xa[:], xT[e][:])
                    xA.append(xa)

                # k projection -> kT rows of kvcat (flat [2*E*S], fp8 transport)
                kvcat = dram.tile([CS], FP8, tag="kvcat", name="kvcat")
                for m in range(NE):
                    w = wp.tile([P, NE, P], BF16, tag="wb", name="wb", bufs=3)
                    nc.scalar.dma_start(out=w[:], in_=d_wkr.ap()[l, m])
                    ps = pacc.tile([P, S], F32, tag="ps_a", name="acc", bufs=6)
                    for k in range(NE):
                        nc.tensor.matmul(ps[:], lhsT=w[:, k, :], rhs=xA[k][:],
                                         start=(k == 0), stop=(k == NE - 1))
                    kl = tp.tile([P, S], FP8, tag="klocal", name="klocal", bufs=2)
                    nc.vector.tensor_scalar(kl[:], ps[:], KVS, None, OP.mult)
                    nc.sync.dma_start(
                        out=kvcat[m * P * S:(m + 1) * P * S].rearrange(
                            "(p s) -> p s", p=P),
                        in_=kl[:])

                # v projection (x-stationary) -> v rows of kvcat
                for n in range(2):
                    psv = [pacc.tile([P, 512], F32, tag="ps_a", name="acc", bufs=6)
                           for _ in range(4)]
                    for k in range(NE):
                        wv = wp.tile([P, 512], BF16, tag="wv", name="wv", bufs=3)
                        nc.scalar.dma_start(
                            out=wv[:],
                            in_=d_wvf.ap()[l, k, :, n * 512:(n + 1) * 512])
                        for m in range(4):
                            nc.tensor.matmul(psv[m][:], lhsT=xA[k][:, m * P:(m + 1) * P],
                                             rhs=wv[:], start=(k == 0), stop=(k == NE - 1))
                    for m in range(4):
                        vv = tp.tile([P, 512], FP8, tag="vlocal", name="vlocal", bufs=2)
                        nc.vector.tensor_scalar(vv[:], psv[m][:], KVS, None, OP.mult)
                        nc.sync.dma_start(
                            out=kvcat[VOFF + m * P * E:VOFF + (m + 1) * P * E]
                            .rearrange("(p f) -> p f", p=P)[:, n * 512:(n + 1) * 512],
                            in_=vv[:])

                # group-local AllGather of concat(kT, v)
                kvout = dram.tile([CH * CS], FP8, tag="kvout", name="kvout")
                nc.gpsimd.collective_compute(
                    "AllGather", OP.bypass, replica_groups=groups,
                    ins=[kvcat[:]], outs=[kvout[:]])

                qT = []
                for m in range(NE):
                    w = wp.tile([P, NE, P], BF16, tag="wb", name="wb", bufs=3)
                    nc.scalar.dma_start(out=w[:], in_=d_wqr.ap()[l, m])
                    ps = pacc.tile([P, S], F32, tag="ps_a", name="acc", bufs=6)
                    for k in range(NE):
                        nc.tensor.matmul(ps[:], lhsT=w[:, k, :], rhs=xA[k][:],
                                         start=(k == 0), stop=(k == NE - 1))
                    q = pp_q.tile([P, S], BF16, tag=f"q{m}", name=f"q{m}")
                    nc.vector.tensor_copy(q[:], ps[:])
                    qT.append(q)

                # view of kvout v-region: [q, cc, tb, f]  (token = cc*512+tb*128+q)
                vview = kvout[:].rearrange(
                    "(cc half tb q f) -> half q cc tb f",
                    cc=CH, half=2, tb=4, q=P, f=E)[1]

                # attention, pair-major; all 16 key blocks on every core
                oT = []
                for p in range(NHP):
                    pav = pacc.tile([P, S], F32, tag="ps_b", name="pav", bufs=2)
                    prs = pacc.tile([P, S], F32, tag="ps_b", name="prs", bufs=2)
                    vsl8 = vp.tile([P, CH, 4, P], FP8, tag="vsl8", name="vsl8", bufs=2)
                    for cc in range(CH):
                        nc.sync.dma_start(
                            out=vsl8[:, cc, :, :],
                            in_=vview[:, cc, :, p * P:(p + 1) * P])
                    vslab = vp.tile([P, CH, 4, P], BF16, tag="vsl", name="vsl", bufs=2)
                    nc.vector.tensor_scalar(vslab[:], vsl8[:], 1.0 / KVS, None, OP.mult)
                    for cc in range(CH):
                        ksl8 = ap_t.tile([P, S], FP8, tag="ksl8", name="ksl8", bufs=3)
                        nc.sync.dma_start(
                            out=ksl8[:],
                            in_=kvout[cc * CS + p * P * S:cc * CS + (p + 1) * P * S]
                            .rearrange("(p s) -> p s", p=P))
                        ksl = ap_t.tile([P, S], BF16, tag="kslab", name="kslab", bufs=3)
                        nc.vector.tensor_scalar(ksl[:], ksl8[:], 1.0 / KVS, None, OP.mult)
                        for j2 in range(4):
                            sb = 4 * cc + j2
                            first, last = (sb == 0), (sb == NSB - 1)
                            sA = pacc.tile([P, S], F32, tag="ps_a", name="sc", bufs=6)
                            sB = pacc.tile([P, S], F32, tag="ps_a", name="sc", bufs=6)
                            nc.tensor.matmul(
                                sA[:], lhsT=ksl[0:HD, j2 * P:(j2 + 1) * P],
                                rhs=qT[p][0:HD, :], tile_position=(0, 0))
                            nc.tensor.matmul(
                                sB[:], lhsT=ksl[HD:P, j2 * P:(j2 + 1) * P],
                                rhs=qT[p][HD:P, :], tile_position=(64, 0))
                            pa = ap_t.tile([P, S], BF16, tag="pt", name="pt", bufs=4)
                            pb = ap_t.tile([P, S], BF16, tag="pt", name="pt", bufs=4)
                            nc.scalar.activation(pa[:], sA[:], AF.Exp, scale=HD ** -0.5)
                            nc.scalar.activation(pb[:], sB[:], AF.Exp, scale=HD ** -0.5)
                            nc.vector.tensor_tensor(out=pa[:], in0=pa[:],
                                                    in1=mask3[:, sb, :], op=OP.mult)
                            nc.vector.tensor_tensor(out=pb[:], in0=pb[:],
                                                    in1=mask3[:, sb, :], op=OP.mult)
                            nc.tensor.matmul(
                                pav[0:HD, :], lhsT=vslab[:, cc, j2, 0:HD],
                                rhs=pa[:], start=first, stop=last,
                                tile_position=(0, 0), skip_group_check=True)
                            nc.tensor.matmul(
                                pav[HD:P, :], lhsT=vslab[:, cc, j2, HD:P],
                                rhs=pb[:], start=first, stop=last,
                                tile_position=(0, 64), skip_group_check=True)
                            nc.tensor.matmul(
                                prs[0:HD, :], lhsT=ones_b[:], rhs=pa[:],
                                start=first, stop=last,
                                tile_position=(0, 0), skip_group_check=True)
                            nc.tensor.matmul(
                                prs[HD:P, :], lhsT=ones_b[:], rhs=pb[:],
                                start=first, stop=last,
                                tile_position=(0, 64), skip_group_check=True)
                    rec = tp.tile([P, S], F32, tag="rec", name="rec", bufs=2)
                    nc.vector.reciprocal(rec[:], prs[:])
                    o = pp_o.tile([P, S], BF16, tag=f"o{p}", name=f"o{p}")
                    nc.vector.tensor_tensor(out=o[:], in0=pav[:], in1=rec[:], op=OP.mult)
                    oT.append(o)

                # output projection + bias + residual (LN stats fed per tile)
                psm1, pss1 = _ln_start(pools)
                for m in range(NE):
                    w = wp.tile([P, NE, P], BF16, tag="wb", name="wb", bufs=3)
                    nc.scalar.dma_start(out=w[:], in_=d_wor.ap()[l, m])
                    ps = pacc.tile([P, S], F32, tag="ps_a", name="acc", bufs=6)
                    for k in range(NE):
                        nc.tensor.matmul(ps[:], lhsT=w[:, k, :], rhs=oT[k][:],
                                         start=(k == 0), stop=(k == NE - 1))
                    yv = tp.tile([P, S], F32, tag="yv", name="yv", bufs=2)
                    nc.scalar.add(yv[:], ps[:], t_bo[l][:, m:m + 1])
                    nc.vector.tensor_tensor(out=sum_t[m][:], in0=yv[:], in1=xT[m][:],
                                            op=OP.add)
                    _ln_feed(nc, pools, psm1, pss1, sum_t[m], m, ones_f)

            _ln_finish(nc, pools, psm1, pss1, sum_t, xT,
                       t_ln1g[l], t_ln1b[l], eps_t)

            # FFN (bf16 matmuls, fp32 psum + residual)
            with tc.tile_pool(name="ht", bufs=1) as pp_h, \
                 tc.tile_pool(name="xbf", bufs=1) as pp_xbf:
                xF = []
                for e in range(NE):
                    xf = pp_xbf.tile([P, S], BF16, tag=f"xf{e}", name=f"xf{e}")
                    nc.vector.tensor_copy(xf[:], xT[e][:])
                    xF.append(xf)
                hT = []
                for fg in range(NE):
                    w = wp.tile([P, 4, NE, P], BF16, tag="w4", name="w4", bufs=3)
                    nc.scalar.dma_start(out=w[:], in_=d_w1r.ap()[l, fg])
                    for fi in range(4):
                        f = fg * 4 + fi
                        ps = pacc.tile([P, S], F32, tag="ps_a", name="acc", bufs=6)
                        for k in range(NE):
                            nc.tensor.matmul(ps[:], lhsT=w[:, fi, k, :], rhs=xF[k][:],
                                             start=(k == 0), stop=(k == NE - 1))
                        h = pp_h.tile([P, S], BF16, tag=f"h{f}", name=f"h{f}")
                        nc.scalar.activation(h[:], ps[:], AF.Relu,
                                             bias=t_b1[l][:, f:f + 1])
                        hT.append(h)
                psm2, pss2 = _ln_start(pools)
                for m in range(NE):
                    w = wp.tile([P, NFT, P], BF16, tag="w32", name="w32", bufs=2)
                    nc.scalar.dma_start(out=w[:], in_=d_w2r.ap()[l, m])
                    ps = pacc.tile([P, S], F32, tag="ps_a", name="acc", bufs=6)
                    for f in range(NFT):
                        nc.tensor.matmul(ps[:], lhsT=w[:, f, :], rhs=hT[f][:],
                                         start=(f == 0), stop=(f == NFT - 1))
                    yv = tp.tile([P, S], F32, tag="yv", name="yv", bufs=2)
                    nc.scalar.add(yv[:], ps[:], t_b2[l][:, m:m + 1])
                    nc.vector.tensor_tensor(out=sum_t[m][:], in0=yv[:], in1=xT[m][:],
                                            op=OP.add)
                    _ln_feed(nc, pools, psm2, pss2, sum_t[m], m, ones_f)
            _ln_finish(nc, pools, psm2, pss2, sum_t, xT,
                       t_ln2g[l], t_ln2b[l], eps_t)

        # ---- final LN + lm_head (5 vocab tiles per weight DMA / output DMA) ----
        _ln_tiles(nc, tc, pools, xT, sum_t, t_lnfg, t_lnfb, ones_f, eps_t)
        with tc.tile_pool(name="lg", bufs=1) as pp_lg, \
             tc.tile_pool(name="xb", bufs=1) as pp_xb:
            xB = []
            for e in range(NE):
                xb = pp_xb.tile([P, S], BF16, tag=f"xb{e}", name=f"xb{e}")
                nc.vector.tensor_copy(xb[:], sum_t[e][:])
                xB.append(xb)
            for vg in range(NVG):
                w = wp.tile([P, VGT, NE, P], BF16, tag="wlmt", name="wlmt", bufs=3)
                nc.scalar.dma_start(out=w[:], in_=d_wlm.ap()[vg])
                lgc = pp_lg.tile([P, VGT, S], BF16, tag="lg", name="lg", bufs=3)
                for u in range(VGT):
                    vt = vg * VGT + u
                    ps = pacc.tile([P, S], F32, tag="ps_a", name="acc", bufs=6)
                    for k in range(NE):
                        nc.tensor.matmul(ps[:], lhsT=w[:, u, k, :], rhs=xB[k][:],
                                         start=(k == 0), stop=(k == NE - 1))
                    nc.scalar.add(lgc[:, u, :], ps[:], t_blm[:, vt:vt + 1])
                nc.sync.dma_start(
                    out=d_out.ap()[vg * VGT * P:(vg + 1) * VGT * P, :]
                    .rearrange("(u p) s -> p u s", p=P),
                    in_=lgc[:])

    nc.compile()
    return nc


_CACHED = {}


def _prep_weights(inputs):
    f32 = np.float32
    bf = ml_dtypes.bfloat16
    Wq, Wk, Wv = inputs["Wq"], inputs["Wk"], inputs["Wv"]
    # [L,H,E,HD] -> [L,E,H*HD]
    wq_flat = np.ascontiguousarray(Wq.transpose(0, 2, 1, 3).reshape(L, E, H * HD))
    wk_flat = np.ascontiguousarray(Wk.transpose(0, 2, 1, 3).reshape(L, E, H * HD))
    wv_flat = np.ascontiguousarray(Wv.transpose(0, 2, 1, 3).reshape(L, E, H * HD))

    def sw_ee(w):  # [L, E_in, E_out] -> [L, m, p, k, f]
        return np.ascontiguousarray(
            w.reshape(L, NE, P, NE, P).transpose(0, 3, 2, 1, 4)).astype(bf)

    wqr = sw_ee(wq_flat)
    wkr = sw_ee(wk_flat)
    wor = sw_ee(inputs["Wo"])
    wvf = np.ascontiguousarray(
        wv_flat.reshape(L, NE, P, H * HD)).astype(bf)
    w1r = np.ascontiguousarray(
        inputs["W1"].reshape(L, NE, P, NE, 4, P).transpose(0, 3, 2, 4, 1, 5)).astype(bf)
    w2r = np.ascontiguousarray(
        inputs["W2"].reshape(L, NFT, P, NE, P).transpose(0, 3, 2, 1, 4)).astype(bf)
    wlmr = np.ascontiguousarray(
        inputs["Wlm"].reshape(NE, P, NVG, VGT, P).transpose(2, 1, 3, 0, 4)).astype(bf)
    return {
        "wqr": wqr, "wkr": wkr, "wvf": wvf, "wor": wor,
        "w1r": w1r, "w2r": w2r, "wlmr": wlmr,
        "emb": np.ascontiguousarray(inputs["emb"]).astype(bf),
        "ln1g": np.ascontiguousarray(inputs["ln1_g"]).astype(f32),
        "ln1b": np.ascontiguousarray(inputs["ln1_b"]).astype(f32),
        "ln2g": np.ascontiguousarray(inputs["ln2_g"]).astype(f32),
        "ln2b": np.ascontiguousarray(inputs["ln2_b"]).astype(f32),
        "bo": np.ascontiguousarray(inputs["bo"]).astype(f32),
        "b1": np.ascontiguousarray(inputs["b1"]).astype(f32),
        "b2": np.ascontiguousarray(inputs["b2"]).astype(f32),
        "lnfg": np.ascontiguousarray(inputs["lnf_g"]).astype(f32),
        "lnfb": np.ascontiguousarray(inputs["lnf_b"]).astype(f32),
        "blm": np.ascontiguousarray(inputs["blm"]).astype(f32),
    }


def kernel(**inputs):
    if "nc" not in _CACHED:
        _CACHED["nc"] = build_program()
    nc = _CACHED["nc"]

    shared = _prep_weights(inputs)
    index = np.asarray(inputs["index"])

    # per-core causal masks and token ids
    jpos = np.arange(S)
    ipos = np.arange(P)
    in_maps = []
    for c in range(NC):
        b, j = c // CH, c % CH
        q0 = j * S
        m = np.zeros((NSB, P, S), np.float32)
        for sb in range(NSB):
            m[sb] = ((sb * P + ipos)[:, None] <= (q0 + jpos)[None, :])
        im = dict(shared)
        im["maskp"] = m.astype(ml_dtypes.bfloat16)
        im["idx"] = np.ascontiguousarray(index[b, q0:q0 + S]).astype(np.int32)
        in_maps.append(im)

    res = bass_utils.run_bass_kernel_spmd(nc, in_maps, core_ids=list(range(NC)))
    out = np.zeros((B, T, V), np.float32)
    for c in range(NC):
        b, j = c // CH, c % CH
        out[b, j * S:(j + 1) * S, :] = res.results[c]["logt"].astype(np.float32).T
    return out


# revision 38
# speedup vs baseline: 1.3671x; 1.1418x over previous
"""Trainium2 Bass kernel for a 3-block GPT (B=2,T=2048,E=1024,H=16,V=32000).

Sharding: sequence-parallel over 8 cores (512 tokens each, weights replicated).
Per layer: QKV local, group-local AllGather of concat(K^T, V) (per-batch groups
[[0-3],[4-7]] so the SPMD program is identical on every core), attention in
scores-transposed layout (softmax denominator via ones-matmul), proj/FFN with
weights stationary, LayerNorm in transposed layout via ones-matmul partition
reductions. lm_head produces logits^T [V, 512] per core in bf16; host
reassembles. Weights are host-swizzled so every streamed tile is contiguous
per partition (>=2KB DMA elements), and DMAs are batched to relieve the
HWDGE/sequencer issue path.
"""

import numpy as np
import ml_dtypes
from contextlib import ExitStack

import concourse.bass as bass
import concourse.mybir as mybir
import concourse.tile as tile
from concourse import bacc
from concourse.masks import make_identity
from concourse import bass_utils

# model dims (hardcoded; harness contract)
B, T, E, H, V = 2, 2048, 1024, 16, 32000
HD, L = 64, 3
NC = 8
S = (B * T) // NC        # 512 tokens per core
CH = NC // B             # 4 chunks (cores) per batch
FF = 4 * E               # 4096
EPS = 1e-5
P = 128
NE = E // P              # 8 e-tiles
NHP = H // 2             # 8 head pairs
NSB = T // P             # 16 key blocks per batch
NVT = V // P             # 250 vocab tiles
NVG = 50                 # vocab groups (5 tiles each)
VGT = NVT // NVG         # 5
NFT = FF // P            # 32 f-tiles
CS = 2 * E * S           # flat kv elems per core (k: E*S, v: S*E)
VOFF = E * S
# pair-grouped kv transport: 3 collectives so attention can start early.
# group g covers head-pairs PGRP[g]; k rows + v cols for those pairs.
PGRP = [(0, 4), (4, 7), (7, 8)]          # [start_pair, end_pair)
GW = [(e - s) * P for s, e in PGRP]      # widths: 512, 384, 128
GKSZ = [w * S for w in GW]               # k elems per group
GSZ = [2 * w * S for w in GW]            # total elems per group (k + v)
F32 = mybir.dt.float32
BF16 = mybir.dt.bfloat16
FP8 = mybir.dt.float8e4
I32 = mybir.dt.int32
KVS = 64.0               # k/v fp8 transport scale (values ~0.013 at layer 1)
AF = mybir.ActivationFunctionType
OP = mybir.AluOpType


def _ln_start(pools):
    """Allocate psum accumulators for LN stats (sum, sum-of-squares)."""
    pacc = pools["pacc"]
    psm = pacc.tile([P, S], F32, tag="ps_b", name="ln_ps", bufs=2)
    pss = pacc.tile([P, S], F32, tag="ps_b", name="ln_ps", bufs=2)
    return psm, pss


def _ln_feed(nc, pools, psm, pss, src_e, e, ones_f):
    """Accumulate LN stats for e-tile `e` (call as each tile is produced)."""
    tp = pools["tp"]
    nc.tensor.matmul(psm[:], lhsT=ones_f[:], rhs=src_e[:],
                     start=(e == 0), stop=(e == NE - 1), skip_group_check=True)
    sq = tp.tile([P, S], F32, tag="ln_sq", name="ln_sq", bufs=2)
    nc.scalar.square(sq[:], src_e[:])
    nc.tensor.matmul(pss[:], lhsT=ones_f[:], rhs=sq[:],
                     start=(e == 0), stop=(e == NE - 1), skip_group_check=True)


def _ln_finish(nc, pools, psm, pss, src, out, g_t, b_t, eps_t):
    """Finalize LN: mean/var/rstd then per-tile normalize."""
    tp = pools["tp"]
    mean = tp.tile([P, S], F32, tag="ln_mean", name="ln_mean")
    nc.scalar.mul(mean[:], psm[:], 1.0 / E)
    msq = tp.tile([P, S], F32, tag="ln_msq", name="ln_msq")
    nc.scalar.square(msq[:], mean[:])
    var = tp.tile([P, S], F32, tag="ln_var", name="ln_var")
    nc.vector.tensor_scalar(var[:], pss[:], 1.0 / E, None, OP.mult)
    nc.vector.tensor_tensor(out=var[:], in0=var[:], in1=msq[:], op=OP.subtract)
    std = tp.tile([P, S], F32, tag="ln_std", name="ln_std")
    nc.scalar.activation(std[:], var[:], AF.Sqrt, bias=eps_t[:])
    rstd = tp.tile([P, S], F32, tag="ln_rstd", name="ln_rstd")
    nc.vector.reciprocal(rstd[:], std[:])
    for e in range(NE):
        t = tp.tile([P, S], F32, tag="ln_t", name="ln_t", bufs=2)
        nc.vector.tensor_tensor(out=t[:], in0=src[e][:], in1=mean[:], op=OP.subtract)
        nc.vector.tensor_tensor(out=t[:], in0=t[:], in1=rstd[:], op=OP.mult)
        nc.scalar.activation(out[e][:], t[:], AF.Identity,
                             bias=b_t[:, e:e + 1], scale=g_t[:, e:e + 1])


def _ln_tiles(nc, tc, pools, src, out, g_t, b_t, ones_f, eps_t):
    """LayerNorm over E (partition axis across the 8 [128,S] tiles of src)."""
    psm, pss = _ln_start(pools)
    for e in range(NE):
        _ln_feed(nc, pools, psm, pss, src[e], e, ones_f)
    _ln_finish(nc, pools, psm, pss, src, out, g_t, b_t, eps_t)


def build_program():
    nc = bacc.Bacc("TRN2", target_bir_lowering=False, debug=False, num_devices=NC)

    # ---- DRAM I/O (all streamed weights pre-swizzled on host so each tile is
    # contiguous per partition) ----
    d_wqr = nc.dram_tensor("wqr", [L, NE, P, NE, P], BF16, kind="ExternalInput")
    d_wkr = nc.dram_tensor("wkr", [L, NE, P, NE, P], BF16, kind="ExternalInput")
    d_wvf = nc.dram_tensor("wvf", [L, NE, P, H * HD], BF16, kind="ExternalInput")
    d_wor = nc.dram_tensor("wor", [L, NE, HD, H, P], BF16, kind="ExternalInput")
    d_w1r = nc.dram_tensor("w1r", [L, NE, P, 4, NE, P], BF16, kind="ExternalInput")
    d_w2r = nc.dram_tensor("w2r", [L, NE, P, NFT, P], BF16, kind="ExternalInput")
    d_wlm = nc.dram_tensor("wlmr", [NVG, P, VGT, NE, P], BF16, kind="ExternalInput")
    d_emb = nc.dram_tensor("emb", [V, E], BF16, kind="ExternalInput")
    d_idx = nc.dram_tensor("idx", [S], I32, kind="ExternalInput")
    d_msk = nc.dram_tensor("maskp", [NSB, P, S], BF16, kind="ExternalInput")
    d_ln1g = nc.dram_tensor("ln1g", [L, P, NE], F32, kind="ExternalInput")
    d_ln1b = nc.dram_tensor("ln1b", [L, P, NE], F32, kind="ExternalInput")
    d_ln2g = nc.dram_tensor("ln2g", [L, P, NE], F32, kind="ExternalInput")
    d_ln2b = nc.dram_tensor("ln2b", [L, P, NE], F32, kind="ExternalInput")
    d_bo = nc.dram_tensor("bo", [L, P, NE], F32, kind="ExternalInput")
    d_b1 = nc.dram_tensor("b1", [L, P, NFT], F32, kind="ExternalInput")
    d_b2 = nc.dram_tensor("b2", [L, P, NE], F32, kind="ExternalInput")
    d_lnfg = nc.dram_tensor("lnfg", [P, NE], F32, kind="ExternalInput")
    d_lnfb = nc.dram_tensor("lnfb", [P, NE], F32, kind="ExternalInput")
    d_blm = nc.dram_tensor("blm", [P, NVT], F32, kind="ExternalInput")
    d_out = nc.dram_tensor("logt", [V, S], BF16, kind="ExternalOutput")

    groups = [[0, 1, 2, 3], [4, 5, 6, 7]]

    with ExitStack() as ctx:
        tc = ctx.enter_context(tile.TileContext(nc, num_cores=NC))
        const = ctx.enter_context(tc.tile_pool(name="const", bufs=1))
        pp_x = ctx.enter_context(tc.tile_pool(name="xres", bufs=1))
        pp_sum = ctx.enter_context(tc.tile_pool(name="xsum", bufs=1))
        pp_msk = ctx.enter_context(tc.tile_pool(name="masks", bufs=1))
        tp = ctx.enter_context(tc.tile_pool(name="tp", bufs=1))
        pacc = ctx.enter_context(tc.tile_pool(name="pacc", bufs=1, space="PSUM"))
        dram = ctx.enter_context(tc.tile_pool(name="ccdram", bufs=2, space="DRAM"))
        pools = {"pacc": pacc, "tp": tp}

        # ---- embedding gather first: critical path at startup ----
        idx_t = const.tile([P, S // P], I32, name="idx_t")
        nc.sync.dma_start(out=idx_t[:], in_=d_idx.ap().rearrange("(g p) -> p g", p=P))

        ident = const.tile([P, P], F32, name="ident")
        make_identity(nc, ident[:])
        identb = const.tile([P, P], BF16, name="identb")
        nc.vector.tensor_copy(identb[:], ident[:])
        ones_f = const.tile([P, P], F32, name="ones_f")
        nc.vector.memset(ones_f[:], 1.0)
        ones_bc = const.tile([P, HD], F32, name="ones_bc")
        nc.vector.memset(ones_bc[:], 1.0)
        eps_t = const.tile([P, 1], F32, name="eps_t")
        nc.vector.memset(eps_t[:], EPS)

        # residual stream xT: 8 tiles [128, S]
        xT = [pp_x.tile([P, S], F32, tag=f"x{e}", name=f"x{e}") for e in range(NE)]
        sum_t = [pp_sum.tile([P, S], F32, tag=f"s{e}", name=f"s{e}") for e in range(NE)]

        for g in range(S // P):
            xg = tp.tile([P, E], BF16, tag="embg", name="embg", bufs=2)
            nc.gpsimd.indirect_dma_start(
                out=xg[:], out_offset=None, in_=d_emb.ap(),
                in_offset=bass.IndirectOffsetOnAxis(ap=idx_t[:, g:g + 1], axis=0))
            for e in range(NE):
                pst = pacc.tile([P, S], BF16, tag="ps_a", name="tpose", bufs=3)
                nc.tensor.transpose(pst[:, 0:P], xg[:, e * P:(e + 1) * P], identb[:])
                nc.vector.tensor_copy(xT[e][:, g * P:(g + 1) * P], pst[:, 0:P])

        # ---- params -> [128, n] tiles (after the gather chain is queued) ----
        def ldvec(dt_ap, n, name):
            t = const.tile([P, n], F32, tag=name, name=name)
            nc.gpsimd.dma_start(out=t[:], in_=dt_ap)
            return t

        t_ln1g = [ldvec(d_ln1g.ap()[l], NE, f"ln1g{l}") for l in range(L)]
        t_ln1b = [ldvec(d_ln1b.ap()[l], NE, f"ln1b{l}") for l in range(L)]
        t_ln2g = [ldvec(d_ln2g.ap()[l], NE, f"ln2g{l}") for l in range(L)]
        t_ln2b = [ldvec(d_ln2b.ap()[l], NE, f"ln2b{l}") for l in range(L)]
        t_bo = [ldvec(d_bo.ap()[l], NE, f"bo{l}") for l in range(L)]
        t_b1 = [ldvec(d_b1.ap()[l], NFT, f"b1{l}") for l in range(L)]
        t_b2 = [ldvec(d_b2.ap()[l], NE, f"b2{l}") for l in range(L)]
        t_lnfg = ldvec(d_lnfg.ap(), NE, "lnfg")
        t_lnfb = ldvec(d_lnfb.ap(), NE, "lnfb")
        t_blm = ldvec(d_blm.ap(), NVT, "blm")

        # causal masks (bf16 multiplicative, per-core data): one DMA
        mask3 = pp_msk.tile([P, NSB * S], BF16, tag="msk", name="msk")
        nc.gpsimd.dma_start(out=mask3[:].rearrange("p (sb s) -> p sb s", sb=NSB),
                    in_=d_msk.ap().rearrange("sb p s -> p sb s"))

        # ---- transformer blocks ----
        for l in range(L):
            with tc.tile_pool(name="qT", bufs=1) as pp_q, \
                 tc.tile_pool(name="oT", bufs=1) as pp_o, \
                 tc.tile_pool(name="attn", bufs=1) as ap_t, \
                 tc.tile_pool(name="vslab", bufs=1) as vp:

                # bf16 activations
                xA = []
                for e in range(NE):
                    xa = ap_t.tile([P, S], BF16, tag=f"xa{e}", name=f"xa{e}")
                    nc.vector.tensor_copy(xa[:], xT[e][:])
                    xA.append(xa)

                # k/v projections feed pair-grouped transport buffers so the
                # AllGathers launch incrementally and overlap attention.
                # Group 0 ships k and v separately: scores for pairs 0-3 can
                # start as soon as their k rows arrive.
                kvcat = [dram.tile([GKSZ[0]], FP8, tag="kvcat0", name="kvcat0"),
                         dram.tile([GSZ[1]], FP8, tag="kvcat1", name="kvcat1"),
                         dram.tile([GSZ[2]], FP8, tag="kvcat2", name="kvcat2")]
                kvout = [dram.tile([CH * GKSZ[0]], FP8, tag="kvout0", name="kvout0"),
                         dram.tile([CH * GSZ[1]], FP8, tag="kvout1", name="kvout1"),
                         dram.tile([CH * GSZ[2]], FP8, tag="kvout2", name="kvout2")]
                c0v = dram.tile([S * GW[0]], FP8, tag="c0v", name="c0v")
                o0v = dram.tile([CH * S * GW[0]], FP8, tag="o0v", name="o0v")

                def kproj(m, g, lm):
                    w = ap_t.tile([P, NE, P], BF16, tag="wb", name="wb", bufs=3)
                    nc.scalar.dma_start(out=w[:], in_=d_wkr.ap()[l, m])
                    ps = pacc.tile([P, S], F32, tag="ps_a", name="acc", bufs=3)
                    for k in range(NE):
                        nc.tensor.matmul(ps[:], lhsT=w[:, k, :], rhs=xA[k][:],
                                         start=(k == 0), stop=(k == NE - 1))
                    kl = tp.tile([P, S], FP8, tag="klocal", name="klocal", bufs=2)
                    nc.vector.tensor_scalar(kl[:], ps[:], KVS, None, OP.mult)
                    nc.sync.dma_start(
                        out=kvcat[g][lm * P * S:(lm + 1) * P * S].rearrange(
                            "(p s) -> p s", p=P),
                        in_=kl[:])

                def vproj(n):
                    # psv halves cover tokens tb*128.., head cols n*512..n*512+512
                    ps2 = [pacc.tile([P, 2, 512], F32, tag="ps_a", name="acc", bufs=3)
                           for _ in range(2)]
                    psv = [ps2[0][:, 0, :], ps2[0][:, 1, :],
                           ps2[1][:, 0, :], ps2[1][:, 1, :]]
                    for k in range(NE):
                        wv = ap_t.tile([P, 512], BF16, tag="wv", name="wv", bufs=3)
                        nc.scalar.dma_start(
                            out=wv[:],
                            in_=d_wvf.ap()[l, k, :, n * 512:(n + 1) * 512])
                        for m in range(4):
                            nc.tensor.matmul(psv[m], lhsT=xA[k][:, m * P:(m + 1) * P],
                                             rhs=wv[:], start=(k == 0), stop=(k == NE - 1))
                    for tb in range(4):
                        vv = tp.tile([P, 512], FP8, tag="vlocal", name="vlocal", bufs=2)
                        nc.vector.tensor_scalar(vv[:], psv[tb], KVS, None, OP.mult)
                        if n == 0:  # all 512 cols belong to group 0
                            nc.sync.dma_start(
                                out=c0v[tb * P * GW[0]:(tb + 1) * P * GW[0]]
                                .rearrange("(p f) -> p f", p=P),
                                in_=vv[:])
                        else:  # cols 0-383 -> group 1, cols 384-511 -> group 2
                            nc.sync.dma_start(
                                out=kvcat[1][GKSZ[1] + tb * P * GW[1]:
                                             GKSZ[1] + (tb + 1) * P * GW[1]]
                                .rearrange("(p f) -> p f", p=P),
                                in_=vv[:, 0:GW[1]])
                            nc.sync.dma_start(
                                out=kvcat[2][GKSZ[2] + tb * P * GW[2]:
                                             GKSZ[2] + (tb + 1) * P * GW[2]]
                                .rearrange("(p f) -> p f", p=P),
                                in_=vv[:, GW[1]:512])

                for m in range(4):
                    kproj(m, 0, m)
                nc.gpsimd.collective_compute(
                    "AllGather", OP.bypass, replica_groups=groups,
                    ins=[kvcat[0][:]], outs=[kvout[0][:]])
                vproj(0)
                nc.gpsimd.collective_compute(
                    "AllGather", OP.bypass, replica_groups=groups,
                    ins=[c0v[:]], outs=[o0v[:]])
                for m in range(4, 7):
                    kproj(m, 1, m - 4)
                kproj(7, 2, 0)
                vproj(1)
                nc.gpsimd.collective_compute(
                    "AllGather", OP.bypass, replica_groups=groups,
                    ins=[kvcat[1][:]], outs=[kvout[1][:]])
                nc.gpsimd.collective_compute(
                    "AllGather", OP.bypass, replica_groups=groups,
                    ins=[kvcat[2][:]], outs=[kvout[2][:]])

                qT = []
                for m in range(NE):
                    w = ap_t.tile([P, NE, P], BF16, tag="wb", name="wb", bufs=3)
                    nc.scalar.dma_start(out=w[:], in_=d_wqr.ap()[l, m])
                    ps = pacc.tile([P, S], F32, tag="ps_a", name="acc", bufs=3)
                    for k in range(NE):
                        nc.tensor.matmul(ps[:], lhsT=w[:, k, :], rhs=xA[k][:],
                                         start=(k == 0), stop=(k == NE - 1))
                    q = pp_q.tile([P, S], BF16, tag=f"q{m}", name=f"q{m}")
                    nc.vector.tensor_copy(q[:], ps[:])
                    qT.append(q)

                # attention: scores pair-major, AV per head with the softmax
                # denominator fused as a ones column in the v slab
                oT = []
                for p in range(NHP):
                    g = 0 if p < 4 else (1 if p < 7 else 2)
                    lm = p - PGRP[g][0]
                    pav_e = pacc.tile([65, S], F32, tag="ps_b", name="pave", bufs=2)
                    pav_o = pacc.tile([65, S], F32, tag="ps_b", name="pavo", bufs=2)
                    vsl8 = vp.tile([P, CH, 4, P], FP8, tag="vsl8", name="vsl8", bufs=2)
                    for cc in range(CH):
                        if g == 0:
                            vsrc, vbase = o0v, cc * S * GW[0]
                        else:
                            vsrc, vbase = kvout[g], cc * GSZ[g] + GKSZ[g]
                        nc.sync.dma_start(
                            out=vsl8[:, cc, :, :],
                            in_=vsrc[vbase:vbase + S * GW[g]].rearrange(
                                "(tb q f) -> q tb f", tb=4, q=P)[
                                :, :, lm * P:(lm + 1) * P])
                    # [v_even(64) | 1 | v_odd(64) | 1] per key block
                    vslab = vp.tile([P, CH, 4, 130], BF16, tag="vsl", name="vsl",
                                    bufs=1)
                    nc.vector.tensor_scalar(vslab[:, :, :, 0:HD],
                                            vsl8[:, :, :, 0:HD],
                                            1.0 / KVS, None, OP.mult)
                    nc.vector.tensor_scalar(vslab[:, :, :, 65:65 + HD],
                                            vsl8[:, :, :, HD:P],
                                            1.0 / KVS, None, OP.mult)
                    nc.vector.memset(vslab[:, :, :, HD:HD + 1], 1.0)
                    nc.vector.memset(vslab[:, :, :, 129:130], 1.0)
                    for cc in range(CH):
                        kbase = (cc * GKSZ[0] if g == 0 else cc * GSZ[g]) + lm * P * S
                        ksl8 = ap_t.tile([P, S], FP8, tag="ksl8", name="ksl8", bufs=3)
                        nc.sync.dma_start(
                            out=ksl8[:],
                            in_=kvout[g][kbase:kbase + P * S]
                            .rearrange("(p s) -> p s", p=P))
                        ksl = ap_t.tile([P, S], BF16, tag="kslab", name="kslab", bufs=3)
                        nc.vector.tensor_scalar(ksl[:], ksl8[:], 1.0 / KVS, None, OP.mult)
                        for jh in range(2):
                            sb0 = 4 * cc + 2 * jh
                            sA = pacc.tile([P, 2, S], F32, tag="ps_a", name="sc",
                                           bufs=3)
                            sB = pacc.tile([P, 2, S], F32, tag="ps_a", name="sc",
                                           bufs=3)
                            for d in range(2):
                                j2 = 2 * jh + d
                                nc.tensor.matmul(
                                    sA[:, d, :], lhsT=ksl[0:HD, j2 * P:(j2 + 1) * P],
                                    rhs=qT[p][0:HD, :], tile_position=(0, 0))
                                nc.tensor.matmul(
                                    sB[:, d, :], lhsT=ksl[HD:P, j2 * P:(j2 + 1) * P],
                                    rhs=qT[p][HD:P, :], tile_position=(64, 0))
                            pa = ap_t.tile([P, 2 * S], BF16, tag="pt", name="pt", bufs=4)
                            pb = ap_t.tile([P, 2 * S], BF16, tag="pt", name="pt", bufs=4)
                            nc.scalar.activation(pa[:], sA[:], AF.Exp, scale=HD ** -0.5)
                            nc.scalar.activation(pb[:], sB[:], AF.Exp, scale=HD ** -0.5)
                            nc.vector.tensor_tensor(out=pa[:], in0=pa[:],
                                                    in1=mask3[:, sb0 * S:(sb0 + 2) * S],
                                                    op=OP.mult)
                            nc.vector.tensor_tensor(out=pb[:], in0=pb[:],
                                                    in1=mask3[:, sb0 * S:(sb0 + 2) * S],
                                                    op=OP.mult)
                            for d in range(2):
                                sb = sb0 + d
                                first, last = (sb == 0), (sb == NSB - 1)
                                j2 = 2 * jh + d
                                nc.tensor.matmul(
                                    pav_e[:], lhsT=vslab[:, cc, j2, 0:65],
                                    rhs=pa[:, d * S:(d + 1) * S], start=first,
                                    stop=last, skip_group_check=True)
                                nc.tensor.matmul(
                                    pav_o[:], lhsT=vslab[:, cc, j2, 65:130],
                                    rhs=pb[:, d * S:(d + 1) * S], start=first,
                                    stop=last, skip_group_check=True)
                    for hh, pav in ((0, pav_e), (1, pav_o)):
                        dd = tp.tile([65, S], F32, tag="dd", name="dd", bufs=2)
                        nc.vector.reciprocal(dd[64:65, :], pav[64:65, :])
                        rec_ps = pacc.tile([HD, S], F32, tag="ps_a", name="recp",
                                           bufs=3)
                        nc.tensor.matmul(rec_ps[:], lhsT=ones_bc[64:65, :],
                                         rhs=dd[64:65, :], tile_position=(64, 0))
                        rec_s = tp.tile([HD, S], F32, tag="rec_s", name="rec_s",
                                        bufs=2)
                        nc.scalar.copy(rec_s[:], rec_ps[:])
                        o = pp_o.tile([HD, S], BF16, tag=f"o{2 * p + hh}",
                                      name=f"o{2 * p + hh}")
                        nc.vector.tensor_tensor(out=o[:], in0=pav[0:HD, :],
                                                in1=rec_s[:], op=OP.mult)
                        oT.append(o)

                # output projection + bias + residual (LN stats fed per tile)
                psm1, pss1 = _ln_start(pools)
                for m in range(NE):
                    w = ap_t.tile([HD, H, P], BF16, tag="wo", name="wo", bufs=3)
                    nc.scalar.dma_start(out=w[:], in_=d_wor.ap()[l, m])
                    ps = pacc.tile([P, S], F32, tag="ps_a", name="acc", bufs=3)
                    for h in range(H):
                        nc.tensor.matmul(ps[:], lhsT=w[:, h, :], rhs=oT[h][:],
                                         start=(h == 0), stop=(h == H - 1))
                    yv = tp.tile([P, S], F32, tag="yv", name="yv", bufs=2)
                    nc.scalar.add(yv[:], ps[:], t_bo[l][:, m:m + 1])
                    nc.vector.tensor_tensor(out=sum_t[m][:], in0=yv[:], in1=xT[m][:],
                                            op=OP.add)
                    _ln_feed(nc, pools, psm1, pss1, sum_t[m], m, ones_f)

            _ln_finish(nc, pools, psm1, pss1, sum_t, xT,
                       t_ln1g[l], t_ln1b[l], eps_t)

            # FFN (bf16 matmuls, fp32 psum + residual)
            with tc.tile_pool(name="ht", bufs=1) as pp_h, \
                 tc.tile_pool(name="xbf", bufs=1) as pp_xbf:
                xF = []
                for e in range(NE):
                    xf = pp_xbf.tile([P, S], BF16, tag=f"xf{e}", name=f"xf{e}")
                    nc.vector.tensor_copy(xf[:], xT[e][:])
                    xF.append(xf)
                hT = []
                for fg in range(NE):
                    w = pp_h.tile([P, 4, NE, P], BF16, tag="w4", name="w4", bufs=3)
                    nc.scalar.dma_start(out=w[:], in_=d_w1r.ap()[l, fg])
                    for fi in range(4):
                        f = fg * 4 + fi
                        ps = pacc.tile([P, S], F32, tag="ps_a", name="acc", bufs=3)
                        for k in range(NE):
                            nc.tensor.matmul(ps[:], lhsT=w[:, fi, k, :], rhs=xF[k][:],
                                             start=(k == 0), stop=(k == NE - 1))
                        h = pp_h.tile([P, S], BF16, tag=f"h{f}", name=f"h{f}")
                        nc.scalar.activation(h[:], ps[:], AF.Relu,
                                             bias=t_b1[l][:, f:f + 1])
                        hT.append(h)
                psm2, pss2 = _ln_start(pools)
                for m in range(NE):
                    w = pp_h.tile([P, NFT, P], BF16, tag="w32", name="w32", bufs=2)
                    nc.scalar.dma_start(out=w[:], in_=d_w2r.ap()[l, m])
                    ps = pacc.tile([P, S], F32, tag="ps_a", name="acc", bufs=3)
                    for f in range(NFT):
                        nc.tensor.matmul(ps[:], lhsT=w[:, f, :], rhs=hT[f][:],
                                         start=(f == 0), stop=(f == NFT - 1))
                    yv = tp.tile([P, S], F32, tag="yv", name="yv", bufs=2)
                    nc.scalar.add(yv[:], ps[:], t_b2[l][:, m:m + 1])
                    nc.vector.tensor_tensor(out=sum_t[m][:], in0=yv[:], in1=xT[m][:],
                                            op=OP.add)
                    _ln_feed(nc, pools, psm2, pss2, sum_t[m], m, ones_f)
            _ln_finish(nc, pools, psm2, pss2, sum_t, xT,
                       t_ln2g[l], t_ln2b[l], eps_t)

        # ---- final LN + lm_head (5 vocab tiles per weight DMA / output DMA) ----
        _ln_tiles(nc, tc, pools, xT, sum_t, t_lnfg, t_lnfb, ones_f, eps_t)
        with tc.tile_pool(name="lg", bufs=1) as pp_lg, \
             tc.tile_pool(name="xb", bufs=1) as pp_xb:
            xB = []
            for e in range(NE):
                xb = pp_xb.tile([P, S], BF16, tag=f"xb{e}", name=f"xb{e}")
                nc.vector.tensor_copy(xb[:], sum_t[e][:])
                xB.append(xb)
            for vg in range(NVG):
                w = pp_lg.tile([P, VGT, NE, P], BF16, tag="wlmt", name="wlmt", bufs=3)
                nc.scalar.dma_start(out=w[:], in_=d_wlm.ap()[vg])
                lgc = pp_lg.tile([P, VGT, S], BF16, tag="lg", name="lg", bufs=3)
                for u in range(VGT):
                    vt = vg * VGT + u
                    ps = pacc.tile([P, S], F32, tag="ps_a", name="acc", bufs=3)
                    for k in range(NE):
                        nc.tensor.matmul(ps[:], lhsT=w[:, u, k, :], rhs=xB[k][:],
                                         start=(k == 0), stop=(k == NE - 1))
                    nc.scalar.add(lgc[:, u, :], ps[:], t_blm[:, vt:vt + 1])
                nc.sync.dma_start(
                    out=d_out.ap()[vg * VGT * P:(vg + 1) * VGT * P, :]
                    .rearrange("(u p) s -> p u s", p=P),
                    in_=lgc[:])

    nc.compile()
    return nc


_CACHED = {}


def _sw(v):
    """[.., n*P] vector -> [.., P, n] partition-major fp32."""
    v = np.asarray(v, np.float32)
    return np.ascontiguousarray(
        v.reshape(*v.shape[:-1], v.shape[-1] // P, P).swapaxes(-1, -2))


def _prep_weights(inputs):
    f32 = np.float32
    bf = ml_dtypes.bfloat16
    Wq, Wk, Wv = inputs["Wq"], inputs["Wk"], inputs["Wv"]
    # [L,H,E,HD] -> [L,E,H*HD]
    wq_flat = np.ascontiguousarray(Wq.transpose(0, 2, 1, 3).reshape(L, E, H * HD))
    wk_flat = np.ascontiguousarray(Wk.transpose(0, 2, 1, 3).reshape(L, E, H * HD))
    wv_flat = np.ascontiguousarray(Wv.transpose(0, 2, 1, 3).reshape(L, E, H * HD))

    def sw_ee(w):  # [L, E_in, E_out] -> [L, m, p, k, f]
        return np.ascontiguousarray(
            w.reshape(L, NE, P, NE, P).transpose(0, 3, 2, 1, 4)).astype(bf)

    wqr = sw_ee(wq_flat)
    wkr = sw_ee(wk_flat)
    # Wo in per-head layout [l, m, d, h, f] so each head's 64 contraction rows
    # sit at partitions 0-63
    wor = np.ascontiguousarray(
        inputs["Wo"].reshape(L, H, HD, NE, P).transpose(0, 3, 2, 1, 4)).astype(bf)
    wvf = np.ascontiguousarray(
        wv_flat.reshape(L, NE, P, H * HD)).astype(bf)
    w1r = np.ascontiguousarray(
        inputs["W1"].reshape(L, NE, P, NE, 4, P).transpose(0, 3, 2, 4, 1, 5)).astype(bf)
    w2r = np.ascontiguousarray(
        inputs["W2"].reshape(L, NFT, P, NE, P).transpose(0, 3, 2, 1, 4)).astype(bf)
    wlmr = np.ascontiguousarray(
        inputs["Wlm"].reshape(NE, P, NVG, VGT, P).transpose(2, 1, 3, 0, 4)).astype(bf)
    return {
        "wqr": wqr, "wkr": wkr, "wvf": wvf, "wor": wor,
        "w1r": w1r, "w2r": w2r, "wlmr": wlmr,
        "emb": np.ascontiguousarray(inputs["emb"]).astype(bf),
        "ln1g": _sw(inputs["ln1_g"]), "ln1b": _sw(inputs["ln1_b"]),
        "ln2g": _sw(inputs["ln2_g"]), "ln2b": _sw(inputs["ln2_b"]),
        "bo": _sw(inputs["bo"]), "b1": _sw(inputs["b1"]), "b2": _sw(inputs["b2"]),
        "lnfg": _sw(inputs["lnf_g"]), "lnfb": _sw(inputs["lnf_b"]),
        "blm": _sw(inputs["blm"]),
    }


def kernel(**inputs):
    if "nc" not in _CACHED:
        _CACHED["nc"] = build_program()
    nc = _CACHED["nc"]

    shared = _prep_weights(inputs)
    index = np.asarray(inputs["index"])

    # per-core causal masks and token ids
    jpos = np.arange(S)
    ipos = np.arange(P)
    in_maps = []
    for c in range(NC):
        b, j = c // CH, c % CH
        q0 = j * S
        m = np.zeros((NSB, P, S), np.float32)
        for sb in range(NSB):
            m[sb] = ((sb * P + ipos)[:, None] <= (q0 + jpos)[None, :])
        im = dict(shared)
        im["maskp"] = m.astype(ml_dtypes.bfloat16)
        im["idx"] = np.ascontiguousarray(index[b, q0:q0 + S]).astype(np.int32)
        in_maps.append(im)

    res = bass_utils.run_bass_kernel_spmd(nc, in_maps, core_ids=list(range(NC)))
    out = np.zeros((B, T, V), np.float32)
    for c in range(NC):
        b, j = c // CH, c % CH
        out[b, j * S:(j + 1) * S, :] = res.results[c]["logt"].astype(np.float32).T
    return out


# revision 44
# speedup vs baseline: 1.3838x; 1.0122x over previous
"""Trainium2 Bass kernel for a 3-block GPT (B=2,T=2048,E=1024,H=16,V=32000).

Sharding: sequence-parallel over 8 cores (512 tokens each, weights replicated).
Per layer: QKV local, group-local AllGather of concat(K^T, V) (per-batch groups
[[0-3],[4-7]] so the SPMD program is identical on every core), attention in
scores-transposed layout (softmax denominator via ones-matmul), proj/FFN with
weights stationary, LayerNorm in transposed layout via ones-matmul partition
reductions. lm_head produces logits^T [V, 512] per core in bf16; host
reassembles. Weights are host-swizzled so every streamed tile is contiguous
per partition (>=2KB DMA elements), and DMAs are batched to relieve the
HWDGE/sequencer issue path.
"""

import numpy as np
import ml_dtypes
from contextlib import ExitStack

import concourse.bass as bass
import concourse.mybir as mybir
import concourse.tile as tile
from concourse import bacc
from concourse.masks import make_identity
from concourse import bass_utils

# model dims (hardcoded; harness contract)
B, T, E, H, V = 2, 2048, 1024, 16, 32000
HD, L = 64, 3
NC = 8
S = (B * T) // NC        # 512 tokens per core
CH = NC // B             # 4 chunks (cores) per batch
FF = 4 * E               # 4096
EPS = 1e-5
P = 128
NE = E // P              # 8 e-tiles
NHP = H // 2             # 8 head pairs
NSB = T // P             # 16 key blocks per batch
NVT = V // P             # 250 vocab tiles
NVG = 50                 # vocab groups (5 tiles each)
VGT = NVT // NVG         # 5
NFT = FF // P            # 32 f-tiles
CS = 2 * E * S           # flat kv elems per core (k: E*S, v: S*E)
VOFF = E * S
# pair-grouped kv transport: 3 collectives so attention can start early.
# group g covers head-pairs PGRP[g]; k rows + v cols for those pairs.
PGRP = [(0, 4), (4, 7), (7, 8)]          # [start_pair, end_pair)
GW = [(e - s) * P for s, e in PGRP]      # widths: 512, 384, 128
GKSZ = [w * S for w in GW]               # k elems per group
GSZ = [2 * w * S for w in GW]            # total elems per group (k + v)
F32 = mybir.dt.float32
BF16 = mybir.dt.bfloat16
FP8 = mybir.dt.float8e4
I32 = mybir.dt.int32
KVS = 64.0               # k/v fp8 transport scale (values ~0.013 at layer 1)
AF = mybir.ActivationFunctionType
OP = mybir.AluOpType


def _ln_start(pools):
    """Allocate psum accumulators for LN stats (sum, sum-of-squares)."""
    pacc = pools["pacc"]
    psm = pacc.tile([P, S], F32, tag="ps_b", name="ln_ps", bufs=2)
    pss = pacc.tile([P, S], F32, tag="ps_b", name="ln_ps", bufs=2)
    return psm, pss


def _ln_feed(nc, pools, psm, pss, src_e, e, ones_f):
    """Accumulate LN stats for e-tile `e` (call as each tile is produced)."""
    tp = pools["tp"]
    nc.tensor.matmul(psm[:], lhsT=ones_f[:], rhs=src_e[:],
                     start=(e == 0), stop=(e == NE - 1), skip_group_check=True)
    sq = tp.tile([P, S], F32, tag="ln_sq", name="ln_sq", bufs=2)
    nc.scalar.square(sq[:], src_e[:])
    nc.tensor.matmul(pss[:], lhsT=ones_f[:], rhs=sq[:],
                     start=(e == 0), stop=(e == NE - 1), skip_group_check=True)


def _ln_finish(nc, pools, psm, pss, src, out, g_t, b_t, eps_t):
    """Finalize LN: mean/var/rstd then per-tile normalize."""
    tp = pools["tp"]
    mean = tp.tile([P, S], F32, tag="ln_mean", name="ln_mean")
    nc.scalar.mul(mean[:], psm[:], 1.0 / E)
    msq = tp.tile([P, S], F32, tag="ln_msq", name="ln_msq")
    nc.scalar.square(msq[:], mean[:])
    var = tp.tile([P, S], F32, tag="ln_var", name="ln_var")
    nc.vector.tensor_scalar(var[:], pss[:], 1.0 / E, None, OP.mult)
    nc.vector.tensor_tensor(out=var[:], in0=var[:], in1=msq[:], op=OP.subtract)
    std = tp.tile([P, S], F32, tag="ln_std", name="ln_std")
    nc.scalar.activation(std[:], var[:], AF.Sqrt, bias=eps_t[:])
    rstd = tp.tile([P, S], F32, tag="ln_rstd", name="ln_rstd")
    nc.vector.reciprocal(rstd[:], std[:])
    for e in range(NE):
        t = tp.tile([P, S], F32, tag="ln_t", name="ln_t", bufs=2)
        nc.vector.tensor_tensor(out=t[:], in0=src[e][:], in1=mean[:], op=OP.subtract)
        nc.vector.tensor_tensor(out=t[:], in0=t[:], in1=rstd[:], op=OP.mult)
        nc.scalar.activation(out[e][:], t[:], AF.Identity,
                             bias=b_t[:, e:e + 1], scale=g_t[:, e:e + 1])


def _ln_tiles(nc, tc, pools, src, out, g_t, b_t, ones_f, eps_t):
    """LayerNorm over E (partition axis across the 8 [128,S] tiles of src)."""
    psm, pss = _ln_start(pools)
    for e in range(NE):
        _ln_feed(nc, pools, psm, pss, src[e], e, ones_f)
    _ln_finish(nc, pools, psm, pss, src, out, g_t, b_t, eps_t)


def build_program():
    nc = bacc.Bacc("TRN2", target_bir_lowering=False, debug=False, num_devices=NC)

    # ---- DRAM I/O (all streamed weights pre-swizzled on host so each tile is
    # contiguous per partition) ----
    d_wqr = nc.dram_tensor("wqr", [L, NE, P, NE, P], BF16, kind="ExternalInput")
    d_wkr = nc.dram_tensor("wkr", [L, NE, P, NE, P], BF16, kind="ExternalInput")
    d_wvf = nc.dram_tensor("wvf", [L, NE, P, H * HD], BF16, kind="ExternalInput")
    d_wor = nc.dram_tensor("wor", [L, NE, HD, H, P], BF16, kind="ExternalInput")
    d_w1r = nc.dram_tensor("w1r", [L, NE, P, 4, NE, P], BF16, kind="ExternalInput")
    d_w2r = nc.dram_tensor("w2r", [L, NE, P, NFT, P], BF16, kind="ExternalInput")
    d_wlm = nc.dram_tensor("wlmr", [NVG, P, VGT, NE, P], BF16, kind="ExternalInput")
    d_emb = nc.dram_tensor("emb", [V, E], BF16, kind="ExternalInput")
    d_idx = nc.dram_tensor("idx", [S], I32, kind="ExternalInput")
    d_msk = nc.dram_tensor("maskp", [NSB, P, S], BF16, kind="ExternalInput")
    d_ln1g = nc.dram_tensor("ln1g", [L, P, NE], F32, kind="ExternalInput")
    d_ln1b = nc.dram_tensor("ln1b", [L, P, NE], F32, kind="ExternalInput")
    d_ln2g = nc.dram_tensor("ln2g", [L, P, NE], F32, kind="ExternalInput")
    d_ln2b = nc.dram_tensor("ln2b", [L, P, NE], F32, kind="ExternalInput")
    d_bo = nc.dram_tensor("bo", [L, P, NE], F32, kind="ExternalInput")
    d_b1 = nc.dram_tensor("b1", [L, P, NFT], F32, kind="ExternalInput")
    d_b2 = nc.dram_tensor("b2", [L, P, NE], F32, kind="ExternalInput")
    d_lnfg = nc.dram_tensor("lnfg", [P, NE], F32, kind="ExternalInput")
    d_lnfb = nc.dram_tensor("lnfb", [P, NE], F32, kind="ExternalInput")
    d_blm = nc.dram_tensor("blm", [P, NVT], F32, kind="ExternalInput")
    d_out = nc.dram_tensor("logt", [V, S], BF16, kind="ExternalOutput")

    groups = [[0, 1, 2, 3], [4, 5, 6, 7]]

    with ExitStack() as ctx:
        tc = ctx.enter_context(tile.TileContext(nc, num_cores=NC))
        const = ctx.enter_context(tc.tile_pool(name="const", bufs=1))
        pp_x = ctx.enter_context(tc.tile_pool(name="xres", bufs=1))
        pp_sum = ctx.enter_context(tc.tile_pool(name="xsum", bufs=1))
        pp_msk = ctx.enter_context(tc.tile_pool(name="masks", bufs=1))
        tp = ctx.enter_context(tc.tile_pool(name="tp", bufs=1))
        pacc = ctx.enter_context(tc.tile_pool(name="pacc", bufs=1, space="PSUM"))
        dram = ctx.enter_context(tc.tile_pool(name="ccdram", bufs=2, space="DRAM"))
        pools = {"pacc": pacc, "tp": tp}

        # ---- embedding gather first: critical path at startup ----
        idx_t = const.tile([P, S // P], I32, name="idx_t")
        nc.sync.dma_start(out=idx_t[:], in_=d_idx.ap().rearrange("(g p) -> p g", p=P))

        # queue all four gathers on the Pool engine before anything else
        xgs = []
        for g in range(S // P):
            xg = tp.tile([P, E], BF16, tag="embg", name="embg", bufs=4)
            nc.gpsimd.indirect_dma_start(
                out=xg[:], out_offset=None, in_=d_emb.ap(),
                in_offset=bass.IndirectOffsetOnAxis(ap=idx_t[:, g:g + 1], axis=0))
            xgs.append(xg)

        ident = const.tile([P, P], F32, name="ident")
        make_identity(nc, ident[:])
        identb = const.tile([P, P], BF16, name="identb")
        nc.vector.tensor_copy(identb[:], ident[:])
        ones_f = const.tile([P, P], F32, name="ones_f")
        nc.vector.memset(ones_f[:], 1.0)
        ones_bc = const.tile([P, HD], F32, name="ones_bc")
        nc.vector.memset(ones_bc[:], 1.0)
        eps_t = const.tile([P, 1], F32, name="eps_t")
        nc.vector.memset(eps_t[:], EPS)

        # residual stream xT: 8 tiles [128, S]
        xT = [pp_x.tile([P, S], F32, tag=f"x{e}", name=f"x{e}") for e in range(NE)]
        sum_t = [pp_sum.tile([P, S], F32, tag=f"s{e}", name=f"s{e}") for e in range(NE)]

        for g in range(S // P):
            for e in range(NE):
                pst = pacc.tile([P, S], BF16, tag="ps_a", name="tpose", bufs=3)
                nc.tensor.transpose(pst[:, 0:P], xgs[g][:, e * P:(e + 1) * P],
                                    identb[:])
                nc.vector.tensor_copy(xT[e][:, g * P:(g + 1) * P], pst[:, 0:P])

        # ---- params -> [128, n] tiles (after the gather chain is queued) ----
        def ldvec(dt_ap, n, name):
            t = const.tile([P, n], F32, tag=name, name=name)
            nc.gpsimd.dma_start(out=t[:], in_=dt_ap)
            return t

        t_ln1g = [ldvec(d_ln1g.ap()[l], NE, f"ln1g{l}") for l in range(L)]
        t_ln1b = [ldvec(d_ln1b.ap()[l], NE, f"ln1b{l}") for l in range(L)]
        t_ln2g = [ldvec(d_ln2g.ap()[l], NE, f"ln2g{l}") for l in range(L)]
        t_ln2b = [ldvec(d_ln2b.ap()[l], NE, f"ln2b{l}") for l in range(L)]
        t_bo = [ldvec(d_bo.ap()[l], NE, f"bo{l}") for l in range(L)]
        t_b1 = [ldvec(d_b1.ap()[l], NFT, f"b1{l}") for l in range(L)]
        t_b2 = [ldvec(d_b2.ap()[l], NE, f"b2{l}") for l in range(L)]
        t_lnfg = ldvec(d_lnfg.ap(), NE, "lnfg")
        t_lnfb = ldvec(d_lnfb.ap(), NE, "lnfb")
        t_blm = ldvec(d_blm.ap(), NVT, "blm")

        # causal masks (bf16 multiplicative, per-core data): one DMA
        mask3 = pp_msk.tile([P, NSB * S], BF16, tag="msk", name="msk")
        nc.gpsimd.dma_start(out=mask3[:].rearrange("p (sb s) -> p sb s", sb=NSB),
                    in_=d_msk.ap().rearrange("sb p s -> p sb s"))

        # ---- transformer blocks ----
        for l in range(L):
            with tc.tile_pool(name="qT", bufs=1) as pp_q, \
                 tc.tile_pool(name="oT", bufs=1) as pp_o, \
                 tc.tile_pool(name="attn", bufs=1) as ap_t, \
                 tc.tile_pool(name="vslab", bufs=1) as vp:

                # bf16 activations
                xA = []
                for e in range(NE):
                    xa = ap_t.tile([P, S], BF16, tag=f"xa{e}", name=f"xa{e}")
                    nc.vector.tensor_copy(xa[:], xT[e][:])
                    xA.append(xa)

                # k/v projections feed pair-grouped transport buffers so the
                # AllGathers launch incrementally and overlap attention.
                # Group 0 ships k and v separately: scores for pairs 0-3 can
                # start as soon as their k rows arrive.
                kvcat = [dram.tile([GKSZ[0]], FP8, tag="kvcat0", name="kvcat0"),
                         dram.tile([GSZ[1]], FP8, tag="kvcat1", name="kvcat1"),
                         dram.tile([GSZ[2]], FP8, tag="kvcat2", name="kvcat2")]
                kvout = [dram.tile([CH * GKSZ[0]], FP8, tag="kvout0", name="kvout0"),
                         dram.tile([CH * GSZ[1]], FP8, tag="kvout1", name="kvout1"),
                         dram.tile([CH * GSZ[2]], FP8, tag="kvout2", name="kvout2")]
                c0v = dram.tile([S * GW[0]], FP8, tag="c0v", name="c0v")
                o0v = dram.tile([CH * S * GW[0]], FP8, tag="o0v", name="o0v")

                def kproj(m, g, lm):
                    w = ap_t.tile([P, NE, P], BF16, tag="wb", name="wb", bufs=3)
                    nc.scalar.dma_start(out=w[:], in_=d_wkr.ap()[l, m])
                    ps = pacc.tile([P, S], F32, tag="ps_a", name="acc", bufs=3)
                    for k in range(NE):
                        nc.tensor.matmul(ps[:], lhsT=w[:, k, :], rhs=xA[k][:],
                                         start=(k == 0), stop=(k == NE - 1))
                    kl = tp.tile([P, S], FP8, tag="klocal", name="klocal", bufs=2)
                    nc.vector.tensor_scalar(kl[:], ps[:], KVS, None, OP.mult)
                    nc.sync.dma_start(
                        out=kvcat[g][lm * P * S:(lm + 1) * P * S].rearrange(
                            "(p s) -> p s", p=P),
                        in_=kl[:])

                def vproj(n):
                    # psv halves cover tokens tb*128.., head cols n*512..n*512+512
                    ps2 = [pacc.tile([P, 2, 512], F32, tag="ps_a", name="acc", bufs=3)
                           for _ in range(2)]
                    psv = [ps2[0][:, 0, :], ps2[0][:, 1, :],
                           ps2[1][:, 0, :], ps2[1][:, 1, :]]
                    for k in range(NE):
                        wv = ap_t.tile([P, 512], BF16, tag="wv", name="wv", bufs=3)
                        nc.scalar.dma_start(
                            out=wv[:],
                            in_=d_wvf.ap()[l, k, :, n * 512:(n + 1) * 512])
                        for m in range(4):
                            nc.tensor.matmul(psv[m], lhsT=xA[k][:, m * P:(m + 1) * P],
                                             rhs=wv[:], start=(k == 0), stop=(k == NE - 1))
                    for tb in range(4):
                        vv = tp.tile([P, 512], FP8, tag="vlocal", name="vlocal", bufs=2)
                        nc.vector.tensor_scalar(vv[:], psv[tb], KVS, None, OP.mult)
                        if n == 0:  # all 512 cols belong to group 0
                            nc.sync.dma_start(
                                out=c0v[tb * P * GW[0]:(tb + 1) * P * GW[0]]
                                .rearrange("(p f) -> p f", p=P),
                                in_=vv[:])
                        else:  # cols 0-383 -> group 1, cols 384-511 -> group 2
                            nc.sync.dma_start(
                                out=kvcat[1][GKSZ[1] + tb * P * GW[1]:
                                             GKSZ[1] + (tb + 1) * P * GW[1]]
                                .rearrange("(p f) -> p f", p=P),
                                in_=vv[:, 0:GW[1]])
                            nc.sync.dma_start(
                                out=kvcat[2][GKSZ[2] + tb * P * GW[2]:
                                             GKSZ[2] + (tb + 1) * P * GW[2]]
                                .rearrange("(p f) -> p f", p=P),
                                in_=vv[:, GW[1]:512])

                for m in range(4):
                    kproj(m, 0, m)
                nc.gpsimd.collective_compute(
                    "AllGather", OP.bypass, replica_groups=groups,
                    ins=[kvcat[0][:]], outs=[kvout[0][:]])
                vproj(0)
                nc.gpsimd.collective_compute(
                    "AllGather", OP.bypass, replica_groups=groups,
                    ins=[c0v[:]], outs=[o0v[:]])
                for m in range(4, 7):
                    kproj(m, 1, m - 4)
                kproj(7, 2, 0)
                vproj(1)
                nc.gpsimd.collective_compute(
                    "AllGather", OP.bypass, replica_groups=groups,
                    ins=[kvcat[1][:]], outs=[kvout[1][:]])
                nc.gpsimd.collective_compute(
                    "AllGather", OP.bypass, replica_groups=groups,
                    ins=[kvcat[2][:]], outs=[kvout[2][:]])

                qT = []
                for m in range(NE):
                    w = ap_t.tile([P, NE, P], BF16, tag="wb", name="wb", bufs=3)
                    nc.scalar.dma_start(out=w[:], in_=d_wqr.ap()[l, m])
                    ps = pacc.tile([P, S], F32, tag="ps_a", name="acc", bufs=3)
                    for k in range(NE):
                        nc.tensor.matmul(ps[:], lhsT=w[:, k, :], rhs=xA[k][:],
                                         start=(k == 0), stop=(k == NE - 1))
                    q = pp_q.tile([P, S], BF16, tag=f"q{m}", name=f"q{m}")
                    nc.vector.tensor_copy(q[:], ps[:])
                    qT.append(q)

                # attention: scores pair-major, AV per head with the softmax
                # denominator fused as a ones column in the v slab
                oT = []
                for p in range(NHP):
                    g = 0 if p < 4 else (1 if p < 7 else 2)
                    lm = p - PGRP[g][0]
                    pav_e = pacc.tile([65, S], F32, tag="ps_b", name="pave", bufs=2)
                    pav_o = pacc.tile([65, S], F32, tag="ps_b", name="pavo", bufs=2)
                    vsl8 = vp.tile([P, CH, 4, P], FP8, tag="vsl8", name="vsl8", bufs=2)
                    for cc in range(CH):
                        if g == 0:
                            vsrc, vbase = o0v, cc * S * GW[0]
                        else:
                            vsrc, vbase = kvout[g], cc * GSZ[g] + GKSZ[g]
                        nc.sync.dma_start(
                            out=vsl8[:, cc, :, :],
                            in_=vsrc[vbase:vbase + S * GW[g]].rearrange(
                                "(tb q f) -> q tb f", tb=4, q=P)[
                                :, :, lm * P:(lm + 1) * P])
                    # [v_even(64) | 1 | v_odd(64) | 1] per key block
                    vslab = vp.tile([P, CH, 4, 130], BF16, tag="vsl", name="vsl",
                                    bufs=1)
                    nc.vector.tensor_scalar(vslab[:, :, :, 0:HD],
                                            vsl8[:, :, :, 0:HD],
                                            1.0 / KVS, None, OP.mult)
                    nc.vector.tensor_scalar(vslab[:, :, :, 65:65 + HD],
                                            vsl8[:, :, :, HD:P],
                                            1.0 / KVS, None, OP.mult)
                    nc.vector.memset(vslab[:, :, :, HD:HD + 1], 1.0)
                    nc.vector.memset(vslab[:, :, :, 129:130], 1.0)
                    for cc in range(CH):
                        kbase = (cc * GKSZ[0] if g == 0 else cc * GSZ[g]) + lm * P * S
                        ksl8 = ap_t.tile([P, S], FP8, tag="ksl8", name="ksl8", bufs=3)
                        nc.sync.dma_start(
                            out=ksl8[:],
                            in_=kvout[g][kbase:kbase + P * S]
                            .rearrange("(p s) -> p s", p=P))
                        ksl = ap_t.tile([P, S], BF16, tag="kslab", name="kslab", bufs=3)
                        nc.vector.tensor_scalar(ksl[:], ksl8[:], 1.0 / KVS, None, OP.mult)
                        for jh in range(2):
                            sb0 = 4 * cc + 2 * jh
                            sA = pacc.tile([P, 2, S], F32, tag="ps_a", name="sc",
                                           bufs=3)
                            sB = pacc.tile([P, 2, S], F32, tag="ps_a", name="sc",
                                           bufs=3)
                            for d in range(2):
                                j2 = 2 * jh + d
                                nc.tensor.matmul(
                                    sA[:, d, :], lhsT=ksl[0:HD, j2 * P:(j2 + 1) * P],
                                    rhs=qT[p][0:HD, :], tile_position=(0, 0))
                                nc.tensor.matmul(
                                    sB[:, d, :], lhsT=ksl[HD:P, j2 * P:(j2 + 1) * P],
                                    rhs=qT[p][HD:P, :], tile_position=(64, 0))
                            pa = ap_t.tile([P, 2 * S], BF16, tag="pt", name="pt", bufs=4)
                            pb = ap_t.tile([P, 2 * S], BF16, tag="pt", name="pt", bufs=4)
                            nc.scalar.activation(pa[:], sA[:], AF.Exp, scale=HD ** -0.5)
                            nc.scalar.activation(pb[:], sB[:], AF.Exp, scale=HD ** -0.5)
                            nc.vector.tensor_tensor(out=pa[:], in0=pa[:],
                                                    in1=mask3[:, sb0 * S:(sb0 + 2) * S],
                                                    op=OP.mult)
                            nc.vector.tensor_tensor(out=pb[:], in0=pb[:],
                                                    in1=mask3[:, sb0 * S:(sb0 + 2) * S],
                                                    op=OP.mult)
                            for d in range(2):
                                sb = sb0 + d
                                first, last = (sb == 0), (sb == NSB - 1)
                                j2 = 2 * jh + d
                                nc.tensor.matmul(
                                    pav_e[:], lhsT=vslab[:, cc, j2, 0:65],
                                    rhs=pa[:, d * S:(d + 1) * S], start=first,
                                    stop=last, skip_group_check=True)
                                nc.tensor.matmul(
                                    pav_o[:], lhsT=vslab[:, cc, j2, 65:130],
                                    rhs=pb[:, d * S:(d + 1) * S], start=first,
                                    stop=last, skip_group_check=True)
                    for hh, pav in ((0, pav_e), (1, pav_o)):
                        dd = tp.tile([65, S], F32, tag="dd", name="dd", bufs=2)
                        nc.vector.reciprocal(dd[64:65, :], pav[64:65, :])
                        rec_ps = pacc.tile([HD, S], F32, tag="ps_a", name="recp",
                                           bufs=3)
                        nc.tensor.matmul(rec_ps[:], lhsT=ones_bc[64:65, :],
                                         rhs=dd[64:65, :], tile_position=(64, 0))
                        rec_s = tp.tile([HD, S], F32, tag="rec_s", name="rec_s",
                                        bufs=2)
                        nc.scalar.copy(rec_s[:], rec_ps[:])
                        o = pp_o.tile([HD, S], BF16, tag=f"o{2 * p + hh}",
                                      name=f"o{2 * p + hh}")
                        nc.vector.tensor_tensor(out=o[:], in0=pav[0:HD, :],
                                                in1=rec_s[:], op=OP.mult)
                        oT.append(o)

                # output projection + bias + residual (LN stats fed per tile)
                psm1, pss1 = _ln_start(pools)
                for m in range(NE):
                    w = ap_t.tile([HD, H, P], BF16, tag="wo", name="wo", bufs=3)
                    nc.scalar.dma_start(out=w[:], in_=d_wor.ap()[l, m])
                    ps = pacc.tile([P, S], F32, tag="ps_a", name="acc", bufs=3)
                    for h in range(H):
                        nc.tensor.matmul(ps[:], lhsT=w[:, h, :], rhs=oT[h][:],
                                         start=(h == 0), stop=(h == H - 1))
                    yv = tp.tile([P, S], F32, tag="yv", name="yv", bufs=2)
                    nc.scalar.add(yv[:], ps[:], t_bo[l][:, m:m + 1])
                    nc.vector.tensor_tensor(out=sum_t[m][:], in0=yv[:], in1=xT[m][:],
                                            op=OP.add)
                    _ln_feed(nc, pools, psm1, pss1, sum_t[m], m, ones_f)

            _ln_finish(nc, pools, psm1, pss1, sum_t, xT,
                       t_ln1g[l], t_ln1b[l], eps_t)

            # FFN (bf16 matmuls, fp32 psum + residual)
            with tc.tile_pool(name="ht", bufs=1) as pp_h, \
                 tc.tile_pool(name="xbf", bufs=1) as pp_xbf:
                xF = []
                for e in range(NE):
                    xf = pp_xbf.tile([P, S], BF16, tag=f"xf{e}", name=f"xf{e}")
                    nc.vector.tensor_copy(xf[:], xT[e][:])
                    xF.append(xf)
                hT = []
                for fg in range(NE):
                    w = pp_h.tile([P, 4, NE, P], BF16, tag="w4", name="w4", bufs=3)
                    nc.scalar.dma_start(out=w[:], in_=d_w1r.ap()[l, fg])
                    for fi in range(4):
                        f = fg * 4 + fi
                        ps = pacc.tile([P, S], F32, tag="ps_a", name="acc", bufs=3)
                        for k in range(NE):
                            nc.tensor.matmul(ps[:], lhsT=w[:, fi, k, :], rhs=xF[k][:],
                                             start=(k == 0), stop=(k == NE - 1))
                        h = pp_h.tile([P, S], BF16, tag=f"h{f}", name=f"h{f}")
                        nc.scalar.activation(h[:], ps[:], AF.Relu,
                                             bias=t_b1[l][:, f:f + 1])
                        hT.append(h)
                psm2, pss2 = _ln_start(pools)
                for m in range(NE):
                    w = pp_h.tile([P, NFT, P], BF16, tag="w32", name="w32", bufs=2)
                    nc.scalar.dma_start(out=w[:], in_=d_w2r.ap()[l, m])
                    ps = pacc.tile([P, S], F32, tag="ps_a", name="acc", bufs=3)
                    for f in range(NFT):
                        nc.tensor.matmul(ps[:], lhsT=w[:, f, :], rhs=hT[f][:],
                                         start=(f == 0), stop=(f == NFT - 1))
                    yv = tp.tile([P, S], F32, tag="yv", name="yv", bufs=2)
                    nc.scalar.add(yv[:], ps[:], t_b2[l][:, m:m + 1])
                    nc.vector.tensor_tensor(out=sum_t[m][:], in0=yv[:], in1=xT[m][:],
                                            op=OP.add)
                    _ln_feed(nc, pools, psm2, pss2, sum_t[m], m, ones_f)
            _ln_finish(nc, pools, psm2, pss2, sum_t, xT,
                       t_ln2g[l], t_ln2b[l], eps_t)

        # ---- final LN + lm_head (5 vocab tiles per weight DMA / output DMA) ----
        _ln_tiles(nc, tc, pools, xT, sum_t, t_lnfg, t_lnfb, ones_f, eps_t)
        with tc.tile_pool(name="lg", bufs=1) as pp_lg, \
             tc.tile_pool(name="xb", bufs=1) as pp_xb:
            xB = []
            for e in range(NE):
                xb = pp_xb.tile([P, S], BF16, tag=f"xb{e}", name=f"xb{e}")
                nc.vector.tensor_copy(xb[:], sum_t[e][:])
                xB.append(xb)
            for vg in range(NVG):
                w = pp_lg.tile([P, VGT, NE, P], BF16, tag="wlmt", name="wlmt", bufs=3)
                nc.scalar.dma_start(out=w[:], in_=d_wlm.ap()[vg])
                lgc = pp_lg.tile([P, VGT, S], BF16, tag="lg", name="lg", bufs=3)
                for u in range(VGT):
                    vt = vg * VGT + u
                    ps = pacc.tile([P, S], F32, tag="ps_a", name="acc", bufs=3)
                    for k in range(NE):
                        nc.tensor.matmul(ps[:], lhsT=w[:, u, k, :], rhs=xB[k][:],
                                         start=(k == 0), stop=(k == NE - 1))
                    nc.scalar.add(lgc[:, u, :], ps[:], t_blm[:, vt:vt + 1])
                nc.sync.dma_start(
                    out=d_out.ap()[vg * VGT * P:(vg + 1) * VGT * P, :]
                    .rearrange("(u p) s -> p u s", p=P),
                    in_=lgc[:])

    nc.compile()
    return nc


_CACHED = {}


def _sw(v):
    """[.., n*P] vector -> [.., P, n] partition-major fp32."""
    v = np.asarray(v, np.float32)
    return np.ascontiguousarray(
        v.reshape(*v.shape[:-1], v.shape[-1] // P, P).swapaxes(-1, -2))


def _prep_weights(inputs):
    f32 = np.float32
    bf = ml_dtypes.bfloat16
    Wq, Wk, Wv = inputs["Wq"], inputs["Wk"], inputs["Wv"]
    # [L,H,E,HD] -> [L,E,H*HD]
    wq_flat = np.ascontiguousarray(Wq.transpose(0, 2, 1, 3).reshape(L, E, H * HD))
    wk_flat = np.ascontiguousarray(Wk.transpose(0, 2, 1, 3).reshape(L, E, H * HD))
    wv_flat = np.ascontiguousarray(Wv.transpose(0, 2, 1, 3).reshape(L, E, H * HD))

    def sw_ee(w):  # [L, E_in, E_out] -> [L, m, p, k, f]
        return np.ascontiguousarray(
            w.reshape(L, NE, P, NE, P).transpose(0, 3, 2, 1, 4)).astype(bf)

    wqr = sw_ee(wq_flat)
    wkr = sw_ee(wk_flat)
    # Wo in per-head layout [l, m, d, h, f] so each head's 64 contraction rows
    # sit at partitions 0-63
    wor = np.ascontiguousarray(
        inputs["Wo"].reshape(L, H, HD, NE, P).transpose(0, 3, 2, 1, 4)).astype(bf)
    wvf = np.ascontiguousarray(
        wv_flat.reshape(L, NE, P, H * HD)).astype(bf)
    w1r = np.ascontiguousarray(
        inputs["W1"].reshape(L, NE, P, NE, 4, P).transpose(0, 3, 2, 4, 1, 5)).astype(bf)
    w2r = np.ascontiguousarray(
        inputs["W2"].reshape(L, NFT, P, NE, P).transpose(0, 3, 2, 1, 4)).astype(bf)
    wlmr = np.ascontiguousarray(
        inputs["Wlm"].reshape(NE, P, NVG, VGT, P).transpose(2, 1, 3, 0, 4)).astype(bf)
    return {
        "wqr": wqr, "wkr": wkr, "wvf": wvf, "wor": wor,
        "w1r": w1r, "w2r": w2r, "wlmr": wlmr,
        "emb": np.ascontiguousarray(inputs["emb"]).astype(bf),
        "ln1g": _sw(inputs["ln1_g"]), "ln1b": _sw(inputs["ln1_b"]),
        "ln2g": _sw(inputs["ln2_g"]), "ln2b": _sw(inputs["ln2_b"]),
        "bo": _sw(inputs["bo"]), "b1": _sw(inputs["b1"]), "b2": _sw(inputs["b2"]),
        "lnfg": _sw(inputs["lnf_g"]), "lnfb": _sw(inputs["lnf_b"]),
        "blm": _sw(inputs["blm"]),
    }


def kernel(**inputs):
    if "nc" not in _CACHED:
        _CACHED["nc"] = build_program()
    nc = _CACHED["nc"]

    shared = _prep_weights(inputs)
    index = np.asarray(inputs["index"])

    # per-core causal masks and token ids
    jpos = np.arange(S)
    ipos = np.arange(P)
    in_maps = []
    for c in range(NC):
        b, j = c // CH, c % CH
        q0 = j * S
        m = np.zeros((NSB, P, S), np.float32)
        for sb in range(NSB):
            m[sb] = ((sb * P + ipos)[:, None] <= (q0 + jpos)[None, :])
        im = dict(shared)
        im["maskp"] = m.astype(ml_dtypes.bfloat16)
        im["idx"] = np.ascontiguousarray(index[b, q0:q0 + S]).astype(np.int32)
        in_maps.append(im)

    res = bass_utils.run_bass_kernel_spmd(nc, in_maps, core_ids=list(range(NC)))
    out = np.zeros((B, T, V), np.float32)
    for c in range(NC):
        b, j = c // CH, c % CH
        out[b, j * S:(j + 1) * S, :] = res.results[c]["logt"].astype(np.float32).T
    return out


# revision 53
# speedup vs baseline: 1.3870x; 1.0023x over previous
"""Trainium2 Bass kernel for a 3-block GPT (B=2,T=2048,E=1024,H=16,V=32000).

Sharding: sequence-parallel over 8 cores (512 tokens each, weights replicated).
Per layer: QKV computed locally; K^T and V are shipped through four
pair-grouped fp8 AllGathers (per-batch groups [[0-3],[4-7]]; k and v of the
first pair-group travel separately) so attention on early head pairs overlaps
the remaining transfers. K/V are scaled x64 into fp8e4 for transport and
descaled to bf16 on arrival. Attention runs scores pair-major with exp+mask
at two-key-block granularity ([128,1024] psum tiles); AV is per-head with the
softmax denominator fused as a ones column in the v slab, then a one-row
reciprocal is broadcast back over 64 partitions with a tiny matmul. The
output projection contracts per-head [64,128] weight slices, and LayerNorm
statistics (ones-matmul partition reductions) are accumulated inside the
producing loops so only the finalize tail is exposed. lm_head streams
host-swizzled bf16 weights in 5-vocab-tile chunks and emits bf16 logits^T
[V, 512] per core; the host reassembles and upcasts. All streamed weights are
host-swizzled so each DMA element is >=2KB-contiguous per partition.
"""

import numpy as np
import ml_dtypes
from contextlib import ExitStack

import concourse.bass as bass
import concourse.mybir as mybir
import concourse.tile as tile
from concourse import bacc
from concourse.masks import make_identity
from concourse import bass_utils

# model dims (hardcoded; harness contract)
B, T, E, H, V = 2, 2048, 1024, 16, 32000
HD, L = 64, 3
NC = 8
S = (B * T) // NC        # 512 tokens per core
CH = NC // B             # 4 chunks (cores) per batch
FF = 4 * E               # 4096
EPS = 1e-5
P = 128
NE = E // P              # 8 e-tiles
NHP = H // 2             # 8 head pairs
NSB = T // P             # 16 key blocks per batch
NVT = V // P             # 250 vocab tiles
NVG = 50                 # vocab groups (5 tiles each)
VGT = NVT // NVG         # 5
NFT = FF // P            # 32 f-tiles
CS = 2 * E * S           # flat kv elems per core (k: E*S, v: S*E)
VOFF = E * S
# pair-grouped kv transport: 3 collectives so attention can start early.
# group g covers head-pairs PGRP[g]; k rows + v cols for those pairs.
PGRP = [(0, 4), (4, 7), (7, 8)]          # [start_pair, end_pair)
GW = [(e - s) * P for s, e in PGRP]      # widths: 512, 384, 128
GKSZ = [w * S for w in GW]               # k elems per group
GSZ = [2 * w * S for w in GW]            # total elems per group (k + v)
F32 = mybir.dt.float32
BF16 = mybir.dt.bfloat16
FP8 = mybir.dt.float8e4
I32 = mybir.dt.int32
KVS = 64.0               # k/v fp8 transport scale (values ~0.013 at layer 1)
AF = mybir.ActivationFunctionType
OP = mybir.AluOpType


def _ln_start(pools):
    """Allocate psum accumulators for LN stats (sum, sum-of-squares)."""
    pacc = pools["pacc"]
    psm = pacc.tile([P, S], F32, tag="ps_b", name="ln_ps", bufs=2)
    pss = pacc.tile([P, S], F32, tag="ps_b", name="ln_ps", bufs=2)
    return psm, pss


def _ln_feed(nc, pools, psm, pss, src_e, e, ones_f):
    """Accumulate LN stats for e-tile `e` (call as each tile is produced)."""
    tp = pools["tp"]
    nc.tensor.matmul(psm[:], lhsT=ones_f[:], rhs=src_e[:],
                     start=(e == 0), stop=(e == NE - 1), skip_group_check=True)
    sq = tp.tile([P, S], F32, tag="ln_sq", name="ln_sq", bufs=2)
    nc.scalar.square(sq[:], src_e[:])
    nc.tensor.matmul(pss[:], lhsT=ones_f[:], rhs=sq[:],
                     start=(e == 0), stop=(e == NE - 1), skip_group_check=True)


def _ln_finish(nc, pools, psm, pss, src, out, g_t, b_t, eps_t):
    """Finalize LN: mean/var/rstd then per-tile normalize."""
    tp = pools["tp"]
    mean = tp.tile([P, S], F32, tag="ln_mean", name="ln_mean")
    nc.scalar.mul(mean[:], psm[:], 1.0 / E)
    msq = tp.tile([P, S], F32, tag="ln_msq", name="ln_msq")
    nc.scalar.square(msq[:], mean[:])
    var = tp.tile([P, S], F32, tag="ln_var", name="ln_var")
    nc.vector.tensor_scalar(var[:], pss[:], 1.0 / E, None, OP.mult)
    nc.vector.tensor_tensor(out=var[:], in0=var[:], in1=msq[:], op=OP.subtract)
    std = tp.tile([P, S], F32, tag="ln_std", name="ln_std")
    nc.scalar.activation(std[:], var[:], AF.Sqrt, bias=eps_t[:])
    rstd = tp.tile([P, S], F32, tag="ln_rstd", name="ln_rstd")
    nc.vector.reciprocal(rstd[:], std[:])
    for e in range(NE):
        t = tp.tile([P, S], F32, tag="ln_t", name="ln_t", bufs=2)
        nc.vector.tensor_tensor(out=t[:], in0=src[e][:], in1=mean[:], op=OP.subtract)
        nc.vector.tensor_tensor(out=t[:], in0=t[:], in1=rstd[:], op=OP.mult)
        nc.scalar.activation(out[e][:], t[:], AF.Identity,
                             bias=b_t[:, e:e + 1], scale=g_t[:, e:e + 1])


def _ln_tiles(nc, tc, pools, src, out, g_t, b_t, ones_f, eps_t):
    """LayerNorm over E (partition axis across the 8 [128,S] tiles of src)."""
    psm, pss = _ln_start(pools)
    for e in range(NE):
        _ln_feed(nc, pools, psm, pss, src[e], e, ones_f)
    _ln_finish(nc, pools, psm, pss, src, out, g_t, b_t, eps_t)


def build_program():
    nc = bacc.Bacc("TRN2", target_bir_lowering=False, debug=False, num_devices=NC)

    # ---- DRAM I/O (all streamed weights pre-swizzled on host so each tile is
    # contiguous per partition) ----
    d_wqr = nc.dram_tensor("wqr", [L, NE, P, NE, P], BF16, kind="ExternalInput")
    d_wkr = nc.dram_tensor("wkr", [L, NE, P, NE, P], BF16, kind="ExternalInput")
    d_wvf = nc.dram_tensor("wvf", [L, NE, P, H * HD], BF16, kind="ExternalInput")
    d_wor = nc.dram_tensor("wor", [L, NE, HD, H, P], BF16, kind="ExternalInput")
    d_w1r = nc.dram_tensor("w1r", [L, NE, P, 4, NE, P], BF16, kind="ExternalInput")
    d_w2r = nc.dram_tensor("w2r", [L, NE, P, NFT, P], BF16, kind="ExternalInput")
    d_wlm = nc.dram_tensor("wlmr", [NVG, P, VGT, NE, P], BF16, kind="ExternalInput")
    d_emb = nc.dram_tensor("emb", [V, E], BF16, kind="ExternalInput")
    d_idx = nc.dram_tensor("idx", [S], I32, kind="ExternalInput")
    d_msk = nc.dram_tensor("maskp", [NSB, P, S], BF16, kind="ExternalInput")
    d_ln1g = nc.dram_tensor("ln1g", [L, P, NE], F32, kind="ExternalInput")
    d_ln1b = nc.dram_tensor("ln1b", [L, P, NE], F32, kind="ExternalInput")
    d_ln2g = nc.dram_tensor("ln2g", [L, P, NE], F32, kind="ExternalInput")
    d_ln2b = nc.dram_tensor("ln2b", [L, P, NE], F32, kind="ExternalInput")
    d_bo = nc.dram_tensor("bo", [L, P, NE], F32, kind="ExternalInput")
    d_b1 = nc.dram_tensor("b1", [L, P, NFT], F32, kind="ExternalInput")
    d_b2 = nc.dram_tensor("b2", [L, P, NE], F32, kind="ExternalInput")
    d_lnfg = nc.dram_tensor("lnfg", [P, NE], F32, kind="ExternalInput")
    d_lnfb = nc.dram_tensor("lnfb", [P, NE], F32, kind="ExternalInput")
    d_blm = nc.dram_tensor("blm", [P, NVT], F32, kind="ExternalInput")
    d_out = nc.dram_tensor("logt", [V, S], BF16, kind="ExternalOutput")

    groups = [[0, 1, 2, 3], [4, 5, 6, 7]]

    with ExitStack() as ctx:
        tc = ctx.enter_context(tile.TileContext(nc, num_cores=NC))
        const = ctx.enter_context(tc.tile_pool(name="const", bufs=1))
        pp_x = ctx.enter_context(tc.tile_pool(name="xres", bufs=1))
        pp_sum = ctx.enter_context(tc.tile_pool(name="xsum", bufs=1))
        pp_msk = ctx.enter_context(tc.tile_pool(name="masks", bufs=1))
        tp = ctx.enter_context(tc.tile_pool(name="tp", bufs=1))
        pacc = ctx.enter_context(tc.tile_pool(name="pacc", bufs=1, space="PSUM"))
        dram = ctx.enter_context(tc.tile_pool(name="ccdram", bufs=2, space="DRAM"))
        pools = {"pacc": pacc, "tp": tp}

        # ---- embedding gather first: critical path at startup ----
        idx_t = const.tile([P, S // P], I32, name="idx_t")
        nc.sync.dma_start(out=idx_t[:], in_=d_idx.ap().rearrange("(g p) -> p g", p=P))

        # queue all four gathers on the Pool engine before anything else
        xgs = []
        for g in range(S // P):
            xg = tp.tile([P, E], BF16, tag="embg", name="embg", bufs=4)
            nc.gpsimd.indirect_dma_start(
                out=xg[:], out_offset=None, in_=d_emb.ap(),
                in_offset=bass.IndirectOffsetOnAxis(ap=idx_t[:, g:g + 1], axis=0))
            xgs.append(xg)

        ident = const.tile([P, P], F32, name="ident")
        make_identity(nc, ident[:])
        identb = const.tile([P, P], BF16, name="identb")
        nc.vector.tensor_copy(identb[:], ident[:])
        ones_f = const.tile([P, P], F32, name="ones_f")
        nc.vector.memset(ones_f[:], 1.0)
        ones_bc = const.tile([P, HD], F32, name="ones_bc")
        nc.vector.memset(ones_bc[:], 1.0)
        eps_t = const.tile([P, 1], F32, name="eps_t")
        nc.vector.memset(eps_t[:], EPS)

        # residual stream xT: 8 tiles [128, S]
        xT = [pp_x.tile([P, S], F32, tag=f"x{e}", name=f"x{e}") for e in range(NE)]
        sum_t = [pp_sum.tile([P, S], F32, tag=f"s{e}", name=f"s{e}") for e in range(NE)]

        for g in range(S // P):
            for e in range(NE):
                pst = pacc.tile([P, S], BF16, tag="ps_a", name="tpose", bufs=3)
                nc.tensor.transpose(pst[:, 0:P], xgs[g][:, e * P:(e + 1) * P],
                                    identb[:])
                nc.vector.tensor_copy(xT[e][:, g * P:(g + 1) * P], pst[:, 0:P])

        # ---- params -> [128, n] tiles (after the gather chain is queued) ----
        def ldvec(dt_ap, n, name):
            t = const.tile([P, n], F32, tag=name, name=name)
            nc.gpsimd.dma_start(out=t[:], in_=dt_ap)
            return t

        t_ln1g = [ldvec(d_ln1g.ap()[l], NE, f"ln1g{l}") for l in range(L)]
        t_ln1b = [ldvec(d_ln1b.ap()[l], NE, f"ln1b{l}") for l in range(L)]
        t_ln2g = [ldvec(d_ln2g.ap()[l], NE, f"ln2g{l}") for l in range(L)]
        t_ln2b = [ldvec(d_ln2b.ap()[l], NE, f"ln2b{l}") for l in range(L)]
        t_bo = [ldvec(d_bo.ap()[l], NE, f"bo{l}") for l in range(L)]
        t_b1 = [ldvec(d_b1.ap()[l], NFT, f"b1{l}") for l in range(L)]
        t_b2 = [ldvec(d_b2.ap()[l], NE, f"b2{l}") for l in range(L)]
        t_lnfg = ldvec(d_lnfg.ap(), NE, "lnfg")
        t_lnfb = ldvec(d_lnfb.ap(), NE, "lnfb")
        t_blm = ldvec(d_blm.ap(), NVT, "blm")

        # causal masks (bf16 multiplicative, per-core data): one DMA
        mask3 = pp_msk.tile([P, NSB * S], BF16, tag="msk", name="msk")
        nc.gpsimd.dma_start(out=mask3[:].rearrange("p (sb s) -> p sb s", sb=NSB),
                    in_=d_msk.ap().rearrange("sb p s -> p sb s"))

        # ---- transformer blocks ----
        for l in range(L):
            with tc.tile_pool(name="qT", bufs=1) as pp_q, \
                 tc.tile_pool(name="oT", bufs=1) as pp_o, \
                 tc.tile_pool(name="attn", bufs=1) as ap_t, \
                 tc.tile_pool(name="vslab", bufs=1) as vp:

                # bf16 activations
                xA = []
                for e in range(NE):
                    xa = ap_t.tile([P, S], BF16, tag=f"xa{e}", name=f"xa{e}")
                    if e % 2 == 0:
                        nc.vector.tensor_copy(xa[:], xT[e][:])
                    else:
                        nc.scalar.copy(xa[:], xT[e][:])
                    xA.append(xa)

                # k/v projections feed pair-grouped transport buffers so the
                # AllGathers launch incrementally and overlap attention.
                # Group 0 ships k and v separately: scores for pairs 0-3 can
                # start as soon as their k rows arrive.
                kvcat = [dram.tile([GKSZ[0]], FP8, tag="kvcat0", name="kvcat0"),
                         dram.tile([GSZ[1]], FP8, tag="kvcat1", name="kvcat1"),
                         dram.tile([GSZ[2]], FP8, tag="kvcat2", name="kvcat2")]
                kvout = [dram.tile([CH * GKSZ[0]], FP8, tag="kvout0", name="kvout0"),
                         dram.tile([CH * GSZ[1]], FP8, tag="kvout1", name="kvout1"),
                         dram.tile([CH * GSZ[2]], FP8, tag="kvout2", name="kvout2")]
                c0v = dram.tile([S * GW[0]], FP8, tag="c0v", name="c0v")
                o0v = dram.tile([CH * S * GW[0]], FP8, tag="o0v", name="o0v")

                def kproj(m, g, lm):
                    w = ap_t.tile([P, NE, P], BF16, tag="wb", name="wb", bufs=3)
                    nc.scalar.dma_start(out=w[:], in_=d_wkr.ap()[l, m])
                    ps = pacc.tile([P, S], F32, tag="ps_a", name="acc", bufs=3)
                    for k in range(NE):
                        nc.tensor.matmul(ps[:], lhsT=w[:, k, :], rhs=xA[k][:],
                                         start=(k == 0), stop=(k == NE - 1))
                    kl = tp.tile([P, S], FP8, tag="klocal", name="klocal", bufs=2)
                    nc.scalar.mul(kl[:], ps[:], KVS)
                    nc.sync.dma_start(
                        out=kvcat[g][lm * P * S:(lm + 1) * P * S].rearrange(
                            "(p s) -> p s", p=P),
                        in_=kl[:])

                def vproj(n):
                    # psv halves cover tokens tb*128.., head cols n*512..n*512+512
                    ps2 = [pacc.tile([P, 2, 512], F32, tag="ps_a", name="acc", bufs=3)
                           for _ in range(2)]
                    psv = [ps2[0][:, 0, :], ps2[0][:, 1, :],
                           ps2[1][:, 0, :], ps2[1][:, 1, :]]
                    for k in range(NE):
                        wv = ap_t.tile([P, 512], BF16, tag="wv", name="wv", bufs=3)
                        nc.scalar.dma_start(
                            out=wv[:],
                            in_=d_wvf.ap()[l, k, :, n * 512:(n + 1) * 512])
                        for m in range(4):
                            nc.tensor.matmul(psv[m], lhsT=xA[k][:, m * P:(m + 1) * P],
                                             rhs=wv[:], start=(k == 0), stop=(k == NE - 1))
                    for tb in range(4):
                        vv = tp.tile([P, 512], FP8, tag="vlocal", name="vlocal", bufs=2)
                        nc.scalar.mul(vv[:], psv[tb], KVS)
                        if n == 0:  # all 512 cols belong to group 0
                            nc.sync.dma_start(
                                out=c0v[tb * P * GW[0]:(tb + 1) * P * GW[0]]
                                .rearrange("(p f) -> p f", p=P),
                                in_=vv[:])
                        else:  # cols 0-383 -> group 1, cols 384-511 -> group 2
                            nc.sync.dma_start(
                                out=kvcat[1][GKSZ[1] + tb * P * GW[1]:
                                             GKSZ[1] + (tb + 1) * P * GW[1]]
                                .rearrange("(p f) -> p f", p=P),
                                in_=vv[:, 0:GW[1]])
                            nc.sync.dma_start(
                                out=kvcat[2][GKSZ[2] + tb * P * GW[2]:
                                             GKSZ[2] + (tb + 1) * P * GW[2]]
                                .rearrange("(p f) -> p f", p=P),
                                in_=vv[:, GW[1]:512])

                for m in range(4):
                    kproj(m, 0, m)
                nc.gpsimd.collective_compute(
                    "AllGather", OP.bypass, replica_groups=groups,
                    ins=[kvcat[0][:]], outs=[kvout[0][:]])
                vproj(0)
                nc.gpsimd.collective_compute(
                    "AllGather", OP.bypass, replica_groups=groups,
                    ins=[c0v[:]], outs=[o0v[:]])
                for m in range(4, 7):
                    kproj(m, 1, m - 4)
                kproj(7, 2, 0)
                vproj(1)
                nc.gpsimd.collective_compute(
                    "AllGather", OP.bypass, replica_groups=groups,
                    ins=[kvcat[1][:]], outs=[kvout[1][:]])
                nc.gpsimd.collective_compute(
                    "AllGather", OP.bypass, replica_groups=groups,
                    ins=[kvcat[2][:]], outs=[kvout[2][:]])

                qT = []
                for m in range(NE):
                    w = ap_t.tile([P, NE, P], BF16, tag="wb", name="wb", bufs=3)
                    nc.scalar.dma_start(out=w[:], in_=d_wqr.ap()[l, m])
                    ps = pacc.tile([P, S], F32, tag="ps_a", name="acc", bufs=3)
                    for k in range(NE):
                        nc.tensor.matmul(ps[:], lhsT=w[:, k, :], rhs=xA[k][:],
                                         start=(k == 0), stop=(k == NE - 1))
                    q = pp_q.tile([P, S], BF16, tag=f"q{m}", name=f"q{m}")
                    nc.scalar.copy(q[:], ps[:])
                    qT.append(q)

                # attention: scores pair-major, AV per head with the softmax
                # denominator fused as a ones column in the v slab
                oT = []
                for p in range(NHP):
                    g = 0 if p < 4 else (1 if p < 7 else 2)
                    lm = p - PGRP[g][0]
                    pav_e = pacc.tile([65, S], F32, tag="ps_b", name="pave", bufs=2)
                    pav_o = pacc.tile([65, S], F32, tag="ps_b", name="pavo", bufs=2)
                    vsl8 = vp.tile([P, CH, 4, P], FP8, tag="vsl8", name="vsl8", bufs=2)
                    for cc in range(CH):
                        if g == 0:
                            vsrc, vbase = o0v, cc * S * GW[0]
                        else:
                            vsrc, vbase = kvout[g], cc * GSZ[g] + GKSZ[g]
                        nc.sync.dma_start(
                            out=vsl8[:, cc, :, :],
                            in_=vsrc[vbase:vbase + S * GW[g]].rearrange(
                                "(tb q f) -> q tb f", tb=4, q=P)[
                                :, :, lm * P:(lm + 1) * P])
                    # [v_even(64) | 1 | v_odd(64) | 1] per key block
                    vslab = vp.tile([P, CH, 4, 130], BF16, tag="vsl", name="vsl",
                                    bufs=1)
                    nc.vector.tensor_scalar(vslab[:, :, :, 0:HD],
                                            vsl8[:, :, :, 0:HD],
                                            1.0 / KVS, None, OP.mult)
                    nc.vector.tensor_scalar(vslab[:, :, :, 65:65 + HD],
                                            vsl8[:, :, :, HD:P],
                                            1.0 / KVS, None, OP.mult)
                    nc.vector.memset(vslab[:, :, :, HD:HD + 1], 1.0)
                    nc.vector.memset(vslab[:, :, :, 129:130], 1.0)
                    for cc in range(CH):
                        kbase = (cc * GKSZ[0] if g == 0 else cc * GSZ[g]) + lm * P * S
                        ksl8 = ap_t.tile([P, S], FP8, tag="ksl8", name="ksl8", bufs=3)
                        nc.sync.dma_start(
                            out=ksl8[:],
                            in_=kvout[g][kbase:kbase + P * S]
                            .rearrange("(p s) -> p s", p=P))
                        ksl = ap_t.tile([P, S], BF16, tag="kslab", name="kslab", bufs=3)
                        nc.vector.tensor_scalar(ksl[:], ksl8[:], 1.0 / KVS, None, OP.mult)
                        for jh in range(2):
                            sb0 = 4 * cc + 2 * jh
                            sA = pacc.tile([P, 2, S], F32, tag="ps_a", name="sc",
                                           bufs=3)
                            sB = pacc.tile([P, 2, S], F32, tag="ps_a", name="sc",
                                           bufs=3)
                            for d in range(2):
                                j2 = 2 * jh + d
                                nc.tensor.matmul(
                                    sA[:, d, :], lhsT=ksl[0:HD, j2 * P:(j2 + 1) * P],
                                    rhs=qT[p][0:HD, :], tile_position=(0, 0))
                                nc.tensor.matmul(
                                    sB[:, d, :], lhsT=ksl[HD:P, j2 * P:(j2 + 1) * P],
                                    rhs=qT[p][HD:P, :], tile_position=(64, 0))
                            pa = ap_t.tile([P, 2 * S], BF16, tag="pt", name="pt", bufs=4)
                            pb = ap_t.tile([P, 2 * S], BF16, tag="pt", name="pt", bufs=4)
                            nc.scalar.activation(pa[:], sA[:], AF.Exp, scale=HD ** -0.5)
                            nc.scalar.activation(pb[:], sB[:], AF.Exp, scale=HD ** -0.5)
                            nc.vector.tensor_tensor(out=pa[:], in0=pa[:],
                                                    in1=mask3[:, sb0 * S:(sb0 + 2) * S],
                                                    op=OP.mult)
                            nc.vector.tensor_tensor(out=pb[:], in0=pb[:],
                                                    in1=mask3[:, sb0 * S:(sb0 + 2) * S],
                                                    op=OP.mult)
                            for d in range(2):
                                sb = sb0 + d
                                first, last = (sb == 0), (sb == NSB - 1)
                                j2 = 2 * jh + d
                                nc.tensor.matmul(
                                    pav_e[:], lhsT=vslab[:, cc, j2, 0:65],
                                    rhs=pa[:, d * S:(d + 1) * S], start=first,
                                    stop=last, skip_group_check=True)
                                nc.tensor.matmul(
                                    pav_o[:], lhsT=vslab[:, cc, j2, 65:130],
                                    rhs=pb[:, d * S:(d + 1) * S], start=first,
                                    stop=last, skip_group_check=True)
                    for hh, pav in ((0, pav_e), (1, pav_o)):
                        dd = tp.tile([65, S], F32, tag="dd", name="dd", bufs=2)
                        nc.vector.reciprocal(dd[64:65, :], pav[64:65, :])
                        rec_ps = pacc.tile([HD, S], F32, tag="ps_a", name="recp",
                                           bufs=3)
                        nc.tensor.matmul(rec_ps[:], lhsT=ones_bc[64:65, :],
                                         rhs=dd[64:65, :], tile_position=(64, 0))
                        rec_s = tp.tile([HD, S], F32, tag="rec_s", name="rec_s",
                                        bufs=2)
                        nc.vector.tensor_copy(rec_s[:], rec_ps[:])
                        o = pp_o.tile([HD, S], BF16, tag=f"o{2 * p + hh}",
                                      name=f"o{2 * p + hh}")
                        nc.vector.tensor_tensor(out=o[:], in0=pav[0:HD, :],
                                                in1=rec_s[:], op=OP.mult)
                        oT.append(o)

                # output projection + bias + residual (LN stats fed per tile)
                psm1, pss1 = _ln_start(pools)
                for m in range(NE):
                    w = ap_t.tile([HD, H, P], BF16, tag="wo", name="wo", bufs=3)
                    nc.scalar.dma_start(out=w[:], in_=d_wor.ap()[l, m])
                    ps = pacc.tile([P, S], F32, tag="ps_a", name="acc", bufs=3)
                    for h in range(H):
                        nc.tensor.matmul(ps[:], lhsT=w[:, h, :], rhs=oT[h][:],
                                         start=(h == 0), stop=(h == H - 1))
                    yv = tp.tile([P, S], F32, tag="yv", name="yv", bufs=2)
                    nc.scalar.add(yv[:], ps[:], t_bo[l][:, m:m + 1])
                    nc.vector.tensor_tensor(out=sum_t[m][:], in0=yv[:], in1=xT[m][:],
                                            op=OP.add)
                    _ln_feed(nc, pools, psm1, pss1, sum_t[m], m, ones_f)

            _ln_finish(nc, pools, psm1, pss1, sum_t, xT,
                       t_ln1g[l], t_ln1b[l], eps_t)

            # FFN (bf16 matmuls, fp32 psum + residual)
            with tc.tile_pool(name="ht", bufs=1) as pp_h, \
                 tc.tile_pool(name="xbf", bufs=1) as pp_xbf:
                xF = []
                for e in range(NE):
                    xf = pp_xbf.tile([P, S], BF16, tag=f"xf{e}", name=f"xf{e}")
                    if e % 2 == 0:
                        nc.vector.tensor_copy(xf[:], xT[e][:])
                    else:
                        nc.scalar.copy(xf[:], xT[e][:])
                    xF.append(xf)
                hT = []
                for fg in range(NE):
                    w = pp_h.tile([P, 4, NE, P], BF16, tag="w4", name="w4", bufs=3)
                    nc.scalar.dma_start(out=w[:], in_=d_w1r.ap()[l, fg])
                    for fi in range(4):
                        f = fg * 4 + fi
                        ps = pacc.tile([P, S], F32, tag="ps_a", name="acc", bufs=3)
                        for k in range(NE):
                            nc.tensor.matmul(ps[:], lhsT=w[:, fi, k, :], rhs=xF[k][:],
                                             start=(k == 0), stop=(k == NE - 1))
                        h = pp_h.tile([P, S], BF16, tag=f"h{f}", name=f"h{f}")
                        nc.scalar.activation(h[:], ps[:], AF.Relu,
                                             bias=t_b1[l][:, f:f + 1])
                        hT.append(h)
                psm2, pss2 = _ln_start(pools)
                for m in range(NE):
                    w = pp_h.tile([P, NFT, P], BF16, tag="w32", name="w32", bufs=2)
                    nc.scalar.dma_start(out=w[:], in_=d_w2r.ap()[l, m])
                    ps = pacc.tile([P, S], F32, tag="ps_a", name="acc", bufs=3)
                    for f in range(NFT):
                        nc.tensor.matmul(ps[:], lhsT=w[:, f, :], rhs=hT[f][:],
                                         start=(f == 0), stop=(f == NFT - 1))
                    yv = tp.tile([P, S], F32, tag="yv", name="yv", bufs=2)
                    nc.scalar.add(yv[:], ps[:], t_b2[l][:, m:m + 1])
                    nc.vector.tensor_tensor(out=sum_t[m][:], in0=yv[:], in1=xT[m][:],
                                            op=OP.add)
                    _ln_feed(nc, pools, psm2, pss2, sum_t[m], m, ones_f)
            _ln_finish(nc, pools, psm2, pss2, sum_t, xT,
                       t_ln2g[l], t_ln2b[l], eps_t)

        # ---- final LN + lm_head (5 vocab tiles per weight DMA / output DMA) ----
        _ln_tiles(nc, tc, pools, xT, sum_t, t_lnfg, t_lnfb, ones_f, eps_t)
        with tc.tile_pool(name="lg", bufs=1) as pp_lg, \
             tc.tile_pool(name="xb", bufs=1) as pp_xb:
            xB = []
            for e in range(NE):
                xb = pp_xb.tile([P, S], BF16, tag=f"xb{e}", name=f"xb{e}")
                if e % 2 == 0:
                    nc.vector.tensor_copy(xb[:], sum_t[e][:])
                else:
                    nc.scalar.copy(xb[:], sum_t[e][:])
                xB.append(xb)
            for vg in range(NVG):
                w = pp_lg.tile([P, VGT, NE, P], BF16, tag="wlmt", name="wlmt", bufs=3)
                nc.scalar.dma_start(out=w[:], in_=d_wlm.ap()[vg])
                lgc = pp_lg.tile([P, VGT, S], BF16, tag="lg", name="lg", bufs=3)
                for u in range(VGT):
                    vt = vg * VGT + u
                    ps = pacc.tile([P, S], F32, tag="ps_a", name="acc", bufs=3)
                    for k in range(NE):
                        nc.tensor.matmul(ps[:], lhsT=w[:, u, k, :], rhs=xB[k][:],
                                         start=(k == 0), stop=(k == NE - 1))
                    nc.scalar.add(lgc[:, u, :], ps[:], t_blm[:, vt:vt + 1])
                nc.sync.dma_start(
                    out=d_out.ap()[vg * VGT * P:(vg + 1) * VGT * P, :]
                    .rearrange("(u p) s -> p u s", p=P),
                    in_=lgc[:])

    nc.compile()
    return nc


_CACHED = {}


def _sw(v):
    """[.., n*P] vector -> [.., P, n] partition-major fp32."""
    v = np.asarray(v, np.float32)
    return np.ascontiguousarray(
        v.reshape(*v.shape[:-1], v.shape[-1] // P, P).swapaxes(-1, -2))


def _prep_weights(inputs):
    f32 = np.float32
    bf = ml_dtypes.bfloat16
    Wq, Wk, Wv = inputs["Wq"], inputs["Wk"], inputs["Wv"]
    # [L,H,E,HD] -> [L,E,H*HD]
    wq_flat = np.ascontiguousarray(Wq.transpose(0, 2, 1, 3).reshape(L, E, H * HD))
    wk_flat = np.ascontiguousarray(Wk.transpose(0, 2, 1, 3).reshape(L, E, H * HD))
    wv_flat = np.ascontiguousarray(Wv.transpose(0, 2, 1, 3).reshape(L, E, H * HD))

    def sw_ee(w):  # [L, E_in, E_out] -> [L, m, p, k, f]
        return np.ascontiguousarray(
            w.reshape(L, NE, P, NE, P).transpose(0, 3, 2, 1, 4)).astype(bf)

    wqr = sw_ee(wq_flat)
    wkr = sw_ee(wk_flat)
    # Wo in per-head layout [l, m, d, h, f] so each head's 64 contraction rows
    # sit at partitions 0-63
    wor = np.ascontiguousarray(
        inputs["Wo"].reshape(L, H, HD, NE, P).transpose(0, 3, 2, 1, 4)).astype(bf)
    wvf = np.ascontiguousarray(
        wv_flat.reshape(L, NE, P, H * HD)).astype(bf)
    w1r = np.ascontiguousarray(
        inputs["W1"].reshape(L, NE, P, NE, 4, P).transpose(0, 3, 2, 4, 1, 5)).astype(bf)
    w2r = np.ascontiguousarray(
        inputs["W2"].reshape(L, NFT, P, NE, P).transpose(0, 3, 2, 1, 4)).astype(bf)
    wlmr = np.ascontiguousarray(
        inputs["Wlm"].reshape(NE, P, NVG, VGT, P).transpose(2, 1, 3, 0, 4)).astype(bf)
    return {
        "wqr": wqr, "wkr": wkr, "wvf": wvf, "wor": wor,
        "w1r": w1r, "w2r": w2r, "wlmr": wlmr,
        "emb": np.ascontiguousarray(inputs["emb"]).astype(bf),
        "ln1g": _sw(inputs["ln1_g"]), "ln1b": _sw(inputs["ln1_b"]),
        "ln2g": _sw(inputs["ln2_g"]), "ln2b": _sw(inputs["ln2_b"]),
        "bo": _sw(inputs["bo"]), "b1": _sw(inputs["b1"]), "b2": _sw(inputs["b2"]),
        "lnfg": _sw(inputs["lnf_g"]), "lnfb": _sw(inputs["lnf_b"]),
        "blm": _sw(inputs["blm"]),
    }


def kernel(**inputs):
    if "nc" not in _CACHED:
        _CACHED["nc"] = build_program()
    nc = _CACHED["nc"]

    shared = _prep_weights(inputs)
    index = np.asarray(inputs["index"])

    # per-core causal masks and token ids
    jpos = np.arange(S)
    ipos = np.arange(P)
    in_maps = []
    for c in range(NC):
        b, j = c // CH, c % CH
        q0 = j * S
        m = np.zeros((NSB, P, S), np.float32)
        for sb in range(NSB):
            m[sb] = ((sb * P + ipos)[:, None] <= (q0 + jpos)[None, :])
        im = dict(shared)
        im["maskp"] = m.astype(ml_dtypes.bfloat16)
        im["idx"] = np.ascontiguousarray(index[b, q0:q0 + S]).astype(np.int32)
        in_maps.append(im)

    res = bass_utils.run_bass_kernel_spmd(nc, in_maps, core_ids=list(range(NC)))
    out = np.zeros((B, T, V), np.float32)
    for c in range(NC):
        b, j = c // CH, c % CH
        out[b, j * S:(j + 1) * S, :] = res.results[c]["logt"].astype(np.float32).T
    return out


# revision 54
# speedup vs baseline: 1.4614x; 1.0537x over previous
"""Trainium2 Bass kernel for a 3-block GPT (B=2,T=2048,E=1024,H=16,V=32000).

Sharding: sequence-parallel over 8 cores (512 tokens each, weights replicated).
Per layer: QKV computed locally; K^T and V are shipped through four
pair-grouped fp8 AllGathers (per-batch groups [[0-3],[4-7]]; k and v of the
first pair-group travel separately) so attention on early head pairs overlaps
the remaining transfers. K/V are scaled x64 into fp8e4 for transport and
descaled to bf16 on arrival. Attention runs scores pair-major with exp+mask
at two-key-block granularity ([128,1024] psum tiles); AV is per-head with the
softmax denominator fused as a ones column in the v slab, then a one-row
reciprocal is broadcast back over 64 partitions with a tiny matmul. The
output projection contracts per-head [64,128] weight slices, and LayerNorm
statistics (ones-matmul partition reductions) are accumulated inside the
producing loops so only the finalize tail is exposed. lm_head streams
host-swizzled bf16 weights in 5-vocab-tile chunks and emits bf16 logits^T
[V, 512] per core; the host reassembles and upcasts. All streamed weights are
host-swizzled so each DMA element is >=2KB-contiguous per partition.
"""

import numpy as np
import ml_dtypes
from contextlib import ExitStack

import concourse.bass as bass
import concourse.mybir as mybir
import concourse.tile as tile
from concourse import bacc
from concourse.masks import make_identity
from concourse import bass_utils

# model dims (hardcoded; harness contract)
B, T, E, H, V = 2, 2048, 1024, 16, 32000
HD, L = 64, 3
NC = 8
S = (B * T) // NC        # 512 tokens per core
CH = NC // B             # 4 chunks (cores) per batch
FF = 4 * E               # 4096
EPS = 1e-5
P = 128
NE = E // P              # 8 e-tiles
NHP = H // 2             # 8 head pairs
NSB = T // P             # 16 key blocks per batch
NVT = V // P             # 250 vocab tiles
NVG = 50                 # vocab groups (5 tiles each)
VGT = NVT // NVG         # 5
NFT = FF // P            # 32 f-tiles
CS = 2 * E * S           # flat kv elems per core (k: E*S, v: S*E)
VOFF = E * S
# pair-grouped kv transport: 3 collectives so attention can start early.
# group g covers head-pairs PGRP[g]; k rows + v cols for those pairs.
PGRP = [(0, 4), (4, 7), (7, 8)]          # [start_pair, end_pair)
GW = [(e - s) * P for s, e in PGRP]      # widths: 512, 384, 128
GKSZ = [w * S for w in GW]               # k elems per group
GSZ = [2 * w * S for w in GW]            # total elems per group (k + v)
F32 = mybir.dt.float32
BF16 = mybir.dt.bfloat16
FP8 = mybir.dt.float8e4
I32 = mybir.dt.int32
KVS = 64.0               # k/v fp8 transport scale (values ~0.013 at layer 1)
AF = mybir.ActivationFunctionType
OP = mybir.AluOpType


def _ln_start(pools):
    """Allocate psum accumulators for LN stats (sum, sum-of-squares)."""
    pacc = pools["pacc"]
    psm = pacc.tile([P, S], F32, tag="ps_b", name="ln_ps", bufs=2)
    pss = pacc.tile([P, S], F32, tag="ps_b", name="ln_ps", bufs=2)
    return psm, pss


def _ln_feed(nc, pools, psm, pss, src_e, e, ones_f):
    """Accumulate LN stats for e-tile `e` (call as each tile is produced)."""
    tp = pools["tp"]
    nc.tensor.matmul(psm[:], lhsT=ones_f[:], rhs=src_e[:],
                     start=(e == 0), stop=(e == NE - 1), skip_group_check=True)
    sq = tp.tile([P, S], F32, tag="ln_sq", name="ln_sq", bufs=2)
    nc.scalar.square(sq[:], src_e[:])
    nc.tensor.matmul(pss[:], lhsT=ones_f[:], rhs=sq[:],
                     start=(e == 0), stop=(e == NE - 1), skip_group_check=True)


def _ln_finish(nc, pools, psm, pss, src, out, g_t, b_t, eps_t):
    """Finalize LN: mean/var/rstd then per-tile normalize."""
    tp = pools["tp"]
    mean = tp.tile([P, S], F32, tag="ln_mean", name="ln_mean")
    nc.scalar.mul(mean[:], psm[:], 1.0 / E)
    msq = tp.tile([P, S], F32, tag="ln_msq", name="ln_msq")
    nc.scalar.square(msq[:], mean[:])
    var = tp.tile([P, S], F32, tag="ln_var", name="ln_var")
    nc.vector.tensor_scalar(var[:], pss[:], 1.0 / E, None, OP.mult)
    nc.vector.tensor_tensor(out=var[:], in0=var[:], in1=msq[:], op=OP.subtract)
    std = tp.tile([P, S], F32, tag="ln_std", name="ln_std")
    nc.scalar.activation(std[:], var[:], AF.Sqrt, bias=eps_t[:])
    rstd = tp.tile([P, S], F32, tag="ln_rstd", name="ln_rstd")
    nc.vector.reciprocal(rstd[:], std[:])
    for e in range(NE):
        t = tp.tile([P, S], F32, tag="ln_t", name="ln_t", bufs=2)
        nc.vector.tensor_tensor(out=t[:], in0=src[e][:], in1=mean[:], op=OP.subtract)
        nc.vector.tensor_tensor(out=t[:], in0=t[:], in1=rstd[:], op=OP.mult)
        nc.scalar.activation(out[e][:], t[:], AF.Identity,
                             bias=b_t[:, e:e + 1], scale=g_t[:, e:e + 1])


def _ln_tiles(nc, tc, pools, src, out, g_t, b_t, ones_f, eps_t):
    """LayerNorm over E (partition axis across the 8 [128,S] tiles of src)."""
    psm, pss = _ln_start(pools)
    for e in range(NE):
        _ln_feed(nc, pools, psm, pss, src[e], e, ones_f)
    _ln_finish(nc, pools, psm, pss, src, out, g_t, b_t, eps_t)


def build_program():
    nc = bacc.Bacc("TRN2", target_bir_lowering=False, debug=False, num_devices=NC)

    # ---- DRAM I/O (all streamed weights pre-swizzled on host so each tile is
    # contiguous per partition) ----
    d_wqr = nc.dram_tensor("wqr", [L, NE, P, NE, P], BF16, kind="ExternalInput")
    d_wkr = nc.dram_tensor("wkr", [L, NE, P, NE, P], BF16, kind="ExternalInput")
    d_wvf = nc.dram_tensor("wvf", [L, NE, P, H * HD], BF16, kind="ExternalInput")
    d_wor = nc.dram_tensor("wor", [L, NE, HD, H, P], BF16, kind="ExternalInput")
    d_w1r = nc.dram_tensor("w1r", [L, NE, P, 4, NE, P], BF16, kind="ExternalInput")
    d_w2r = nc.dram_tensor("w2r", [L, NE, P, NFT, P], BF16, kind="ExternalInput")
    d_wlm = nc.dram_tensor("wlmr", [NVG, P, VGT, NE, 2, P], FP8,
                           kind="ExternalInput")
    d_emb = nc.dram_tensor("emb", [V, E], BF16, kind="ExternalInput")
    d_idx = nc.dram_tensor("idx", [S], I32, kind="ExternalInput")
    d_msk = nc.dram_tensor("maskp", [NSB, P, S], BF16, kind="ExternalInput")
    d_ln1g = nc.dram_tensor("ln1g", [L, P, NE], F32, kind="ExternalInput")
    d_ln1b = nc.dram_tensor("ln1b", [L, P, NE], F32, kind="ExternalInput")
    d_ln2g = nc.dram_tensor("ln2g", [L, P, NE], F32, kind="ExternalInput")
    d_ln2b = nc.dram_tensor("ln2b", [L, P, NE], F32, kind="ExternalInput")
    d_bo = nc.dram_tensor("bo", [L, P, NE], F32, kind="ExternalInput")
    d_b1 = nc.dram_tensor("b1", [L, P, NFT], F32, kind="ExternalInput")
    d_b2 = nc.dram_tensor("b2", [L, P, NE], F32, kind="ExternalInput")
    d_lnfg = nc.dram_tensor("lnfg", [P, NE], F32, kind="ExternalInput")
    d_lnfb = nc.dram_tensor("lnfb", [P, NE], F32, kind="ExternalInput")
    d_blm = nc.dram_tensor("blm", [P, NVT], F32, kind="ExternalInput")
    d_out = nc.dram_tensor("logt", [V, S], BF16, kind="ExternalOutput")

    groups = [[0, 1, 2, 3], [4, 5, 6, 7]]

    with ExitStack() as ctx:
        tc = ctx.enter_context(tile.TileContext(nc, num_cores=NC))
        const = ctx.enter_context(tc.tile_pool(name="const", bufs=1))
        pp_x = ctx.enter_context(tc.tile_pool(name="xres", bufs=1))
        pp_sum = ctx.enter_context(tc.tile_pool(name="xsum", bufs=1))
        pp_msk = ctx.enter_context(tc.tile_pool(name="masks", bufs=1))
        tp = ctx.enter_context(tc.tile_pool(name="tp", bufs=1))
        pacc = ctx.enter_context(tc.tile_pool(name="pacc", bufs=1, space="PSUM"))
        dram = ctx.enter_context(tc.tile_pool(name="ccdram", bufs=2, space="DRAM"))
        pools = {"pacc": pacc, "tp": tp}

        # ---- embedding gather first: critical path at startup ----
        idx_t = const.tile([P, S // P], I32, name="idx_t")
        nc.sync.dma_start(out=idx_t[:], in_=d_idx.ap().rearrange("(g p) -> p g", p=P))

        # queue all four gathers on the Pool engine before anything else
        xgs = []
        for g in range(S // P):
            xg = tp.tile([P, E], BF16, tag="embg", name="embg", bufs=4)
            nc.gpsimd.indirect_dma_start(
                out=xg[:], out_offset=None, in_=d_emb.ap(),
                in_offset=bass.IndirectOffsetOnAxis(ap=idx_t[:, g:g + 1], axis=0))
            xgs.append(xg)

        ident = const.tile([P, P], F32, name="ident")
        make_identity(nc, ident[:])
        identb = const.tile([P, P], BF16, name="identb")
        nc.vector.tensor_copy(identb[:], ident[:])
        ones_f = const.tile([P, P], F32, name="ones_f")
        nc.vector.memset(ones_f[:], 1.0)
        ones_bc = const.tile([P, HD], F32, name="ones_bc")
        nc.vector.memset(ones_bc[:], 1.0)
        eps_t = const.tile([P, 1], F32, name="eps_t")
        nc.vector.memset(eps_t[:], EPS)

        # residual stream xT: 8 tiles [128, S]
        xT = [pp_x.tile([P, S], F32, tag=f"x{e}", name=f"x{e}") for e in range(NE)]
        sum_t = [pp_sum.tile([P, S], F32, tag=f"s{e}", name=f"s{e}") for e in range(NE)]

        for g in range(S // P):
            for e in range(NE):
                pst = pacc.tile([P, S], BF16, tag="ps_a", name="tpose", bufs=3)
                nc.tensor.transpose(pst[:, 0:P], xgs[g][:, e * P:(e + 1) * P],
                                    identb[:])
                nc.vector.tensor_copy(xT[e][:, g * P:(g + 1) * P], pst[:, 0:P])

        # ---- params -> [128, n] tiles (after the gather chain is queued) ----
        def ldvec(dt_ap, n, name):
            t = const.tile([P, n], F32, tag=name, name=name)
            nc.gpsimd.dma_start(out=t[:], in_=dt_ap)
            return t

        t_ln1g = [ldvec(d_ln1g.ap()[l], NE, f"ln1g{l}") for l in range(L)]
        t_ln1b = [ldvec(d_ln1b.ap()[l], NE, f"ln1b{l}") for l in range(L)]
        t_ln2g = [ldvec(d_ln2g.ap()[l], NE, f"ln2g{l}") for l in range(L)]
        t_ln2b = [ldvec(d_ln2b.ap()[l], NE, f"ln2b{l}") for l in range(L)]
        t_bo = [ldvec(d_bo.ap()[l], NE, f"bo{l}") for l in range(L)]
        t_b1 = [ldvec(d_b1.ap()[l], NFT, f"b1{l}") for l in range(L)]
        t_b2 = [ldvec(d_b2.ap()[l], NE, f"b2{l}") for l in range(L)]
        t_lnfg = ldvec(d_lnfg.ap(), NE, "lnfg")
        t_lnfb = ldvec(d_lnfb.ap(), NE, "lnfb")
        t_blm = ldvec(d_blm.ap(), NVT, "blm")

        # causal masks (bf16 multiplicative, per-core data): one DMA
        mask3 = pp_msk.tile([P, NSB * S], BF16, tag="msk", name="msk")
        nc.gpsimd.dma_start(out=mask3[:].rearrange("p (sb s) -> p sb s", sb=NSB),
                    in_=d_msk.ap().rearrange("sb p s -> p sb s"))

        # ---- transformer blocks ----
        for l in range(L):
            with tc.tile_pool(name="qT", bufs=1) as pp_q, \
                 tc.tile_pool(name="oT", bufs=1) as pp_o, \
                 tc.tile_pool(name="attn", bufs=1) as ap_t, \
                 tc.tile_pool(name="vslab", bufs=1) as vp:

                # bf16 activations
                xA = []
                for e in range(NE):
                    xa = ap_t.tile([P, S], BF16, tag=f"xa{e}", name=f"xa{e}")
                    if e % 2 == 0:
                        nc.vector.tensor_copy(xa[:], xT[e][:])
                    else:
                        nc.scalar.copy(xa[:], xT[e][:])
                    xA.append(xa)

                # k/v projections feed pair-grouped transport buffers so the
                # AllGathers launch incrementally and overlap attention.
                # Group 0 ships k and v separately: scores for pairs 0-3 can
                # start as soon as their k rows arrive.
                kvcat = [dram.tile([GKSZ[0]], FP8, tag="kvcat0", name="kvcat0"),
                         dram.tile([GSZ[1]], FP8, tag="kvcat1", name="kvcat1"),
                         dram.tile([GSZ[2]], FP8, tag="kvcat2", name="kvcat2")]
                kvout = [dram.tile([CH * GKSZ[0]], FP8, tag="kvout0", name="kvout0"),
                         dram.tile([CH * GSZ[1]], FP8, tag="kvout1", name="kvout1"),
                         dram.tile([CH * GSZ[2]], FP8, tag="kvout2", name="kvout2")]
                c0v = dram.tile([S * GW[0]], FP8, tag="c0v", name="c0v")
                o0v = dram.tile([CH * S * GW[0]], FP8, tag="o0v", name="o0v")

                def kproj(m, g, lm):
                    w = ap_t.tile([P, NE, P], BF16, tag="wb", name="wb", bufs=3)
                    nc.scalar.dma_start(out=w[:], in_=d_wkr.ap()[l, m])
                    ps = pacc.tile([P, S], F32, tag="ps_a", name="acc", bufs=3)
                    for k in range(NE):
                        nc.tensor.matmul(ps[:], lhsT=w[:, k, :], rhs=xA[k][:],
                                         start=(k == 0), stop=(k == NE - 1))
                    kl = tp.tile([P, S], FP8, tag="klocal", name="klocal", bufs=2)
                    nc.scalar.mul(kl[:], ps[:], KVS)
                    nc.sync.dma_start(
                        out=kvcat[g][lm * P * S:(lm + 1) * P * S].rearrange(
                            "(p s) -> p s", p=P),
                        in_=kl[:])

                def vproj(n):
                    # psv halves cover tokens tb*128.., head cols n*512..n*512+512
                    ps2 = [pacc.tile([P, 2, 512], F32, tag="ps_a", name="acc", bufs=3)
                           for _ in range(2)]
                    psv = [ps2[0][:, 0, :], ps2[0][:, 1, :],
                           ps2[1][:, 0, :], ps2[1][:, 1, :]]
                    for k in range(NE):
                        wv = ap_t.tile([P, 512], BF16, tag="wv", name="wv", bufs=3)
                        nc.scalar.dma_start(
                            out=wv[:],
                            in_=d_wvf.ap()[l, k, :, n * 512:(n + 1) * 512])
                        for m in range(4):
                            nc.tensor.matmul(psv[m], lhsT=xA[k][:, m * P:(m + 1) * P],
                                             rhs=wv[:], start=(k == 0), stop=(k == NE - 1))
                    for tb in range(4):
                        vv = tp.tile([P, 512], FP8, tag="vlocal", name="vlocal", bufs=2)
                        nc.scalar.mul(vv[:], psv[tb], KVS)
                        if n == 0:  # all 512 cols belong to group 0
                            nc.sync.dma_start(
                                out=c0v[tb * P * GW[0]:(tb + 1) * P * GW[0]]
                                .rearrange("(p f) -> p f", p=P),
                                in_=vv[:])
                        else:  # cols 0-383 -> group 1, cols 384-511 -> group 2
                            nc.sync.dma_start(
                                out=kvcat[1][GKSZ[1] + tb * P * GW[1]:
                                             GKSZ[1] + (tb + 1) * P * GW[1]]
                                .rearrange("(p f) -> p f", p=P),
                                in_=vv[:, 0:GW[1]])
                            nc.sync.dma_start(
                                out=kvcat[2][GKSZ[2] + tb * P * GW[2]:
                                             GKSZ[2] + (tb + 1) * P * GW[2]]
                                .rearrange("(p f) -> p f", p=P),
                                in_=vv[:, GW[1]:512])

                for m in range(4):
                    kproj(m, 0, m)
                nc.gpsimd.collective_compute(
                    "AllGather", OP.bypass, replica_groups=groups,
                    ins=[kvcat[0][:]], outs=[kvout[0][:]])
                vproj(0)
                nc.gpsimd.collective_compute(
                    "AllGather", OP.bypass, replica_groups=groups,
                    ins=[c0v[:]], outs=[o0v[:]])
                for m in range(4, 7):
                    kproj(m, 1, m - 4)
                kproj(7, 2, 0)
                vproj(1)
                nc.gpsimd.collective_compute(
                    "AllGather", OP.bypass, replica_groups=groups,
                    ins=[kvcat[1][:]], outs=[kvout[1][:]])
                nc.gpsimd.collective_compute(
                    "AllGather", OP.bypass, replica_groups=groups,
                    ins=[kvcat[2][:]], outs=[kvout[2][:]])

                qT = []
                for m in range(NE):
                    w = ap_t.tile([P, NE, P], BF16, tag="wb", name="wb", bufs=3)
                    nc.scalar.dma_start(out=w[:], in_=d_wqr.ap()[l, m])
                    ps = pacc.tile([P, S], F32, tag="ps_a", name="acc", bufs=3)
                    for k in range(NE):
                        nc.tensor.matmul(ps[:], lhsT=w[:, k, :], rhs=xA[k][:],
                                         start=(k == 0), stop=(k == NE - 1))
                    q = pp_q.tile([P, S], BF16, tag=f"q{m}", name=f"q{m}")
                    nc.scalar.copy(q[:], ps[:])
                    qT.append(q)

                # attention: scores pair-major, AV per head with the softmax
                # denominator fused as a ones column in the v slab
                oT = []
                for p in range(NHP):
                    g = 0 if p < 4 else (1 if p < 7 else 2)
                    lm = p - PGRP[g][0]
                    pav_e = pacc.tile([65, S], F32, tag="ps_b", name="pave", bufs=2)
                    pav_o = pacc.tile([65, S], F32, tag="ps_b", name="pavo", bufs=2)
                    vsl8 = vp.tile([P, CH, 4, P], FP8, tag="vsl8", name="vsl8", bufs=2)
                    for cc in range(CH):
                        if g == 0:
                            vsrc, vbase = o0v, cc * S * GW[0]
                        else:
                            vsrc, vbase = kvout[g], cc * GSZ[g] + GKSZ[g]
                        nc.sync.dma_start(
                            out=vsl8[:, cc, :, :],
                            in_=vsrc[vbase:vbase + S * GW[g]].rearrange(
                                "(tb q f) -> q tb f", tb=4, q=P)[
                                :, :, lm * P:(lm + 1) * P])
                    # [v_even(64) | 1 | v_odd(64) | 1] per key block
                    vslab = vp.tile([P, CH, 4, 130], BF16, tag="vsl", name="vsl",
                                    bufs=1)
                    nc.vector.tensor_scalar(vslab[:, :, :, 0:HD],
                                            vsl8[:, :, :, 0:HD],
                                            1.0 / KVS, None, OP.mult)
                    nc.vector.tensor_scalar(vslab[:, :, :, 65:65 + HD],
                                            vsl8[:, :, :, HD:P],
                                            1.0 / KVS, None, OP.mult)
                    nc.vector.memset(vslab[:, :, :, HD:HD + 1], 1.0)
                    nc.vector.memset(vslab[:, :, :, 129:130], 1.0)
                    for cc in range(CH):
                        kbase = (cc * GKSZ[0] if g == 0 else cc * GSZ[g]) + lm * P * S
                        ksl8 = ap_t.tile([P, S], FP8, tag="ksl8", name="ksl8", bufs=3)
                        nc.sync.dma_start(
                            out=ksl8[:],
                            in_=kvout[g][kbase:kbase + P * S]
                            .rearrange("(p s) -> p s", p=P))
                        ksl = ap_t.tile([P, S], BF16, tag="kslab", name="kslab", bufs=3)
                        nc.vector.tensor_scalar(ksl[:], ksl8[:], 1.0 / KVS, None, OP.mult)
                        for jh in range(2):
                            sb0 = 4 * cc + 2 * jh
                            sA = pacc.tile([P, 2, S], F32, tag="ps_a", name="sc",
                                           bufs=3)
                            sB = pacc.tile([P, 2, S], F32, tag="ps_a", name="sc",
                                           bufs=3)
                            for d in range(2):
                                j2 = 2 * jh + d
                                nc.tensor.matmul(
                                    sA[:, d, :], lhsT=ksl[0:HD, j2 * P:(j2 + 1) * P],
                                    rhs=qT[p][0:HD, :], tile_position=(0, 0))
                                nc.tensor.matmul(
                                    sB[:, d, :], lhsT=ksl[HD:P, j2 * P:(j2 + 1) * P],
                                    rhs=qT[p][HD:P, :], tile_position=(64, 0))
                            pa = ap_t.tile([P, 2 * S], BF16, tag="pt", name="pt", bufs=4)
                            pb = ap_t.tile([P, 2 * S], BF16, tag="pt", name="pt", bufs=4)
                            nc.scalar.activation(pa[:], sA[:], AF.Exp, scale=HD ** -0.5)
                            nc.scalar.activation(pb[:], sB[:], AF.Exp, scale=HD ** -0.5)
                            nc.vector.tensor_tensor(out=pa[:], in0=pa[:],
                                                    in1=mask3[:, sb0 * S:(sb0 + 2) * S],
                                                    op=OP.mult)
                            nc.vector.tensor_tensor(out=pb[:], in0=pb[:],
                                                    in1=mask3[:, sb0 * S:(sb0 + 2) * S],
                                                    op=OP.mult)
                            for d in range(2):
                                sb = sb0 + d
                                first, last = (sb == 0), (sb == NSB - 1)
                                j2 = 2 * jh + d
                                nc.tensor.matmul(
                                    pav_e[:], lhsT=vslab[:, cc, j2, 0:65],
                                    rhs=pa[:, d * S:(d + 1) * S], start=first,
                                    stop=last, skip_group_check=True)
                                nc.tensor.matmul(
                                    pav_o[:], lhsT=vslab[:, cc, j2, 65:130],
                                    rhs=pb[:, d * S:(d + 1) * S], start=first,
                                    stop=last, skip_group_check=True)
                    for hh, pav in ((0, pav_e), (1, pav_o)):
                        dd = tp.tile([65, S], F32, tag="dd", name="dd", bufs=2)
                        nc.vector.reciprocal(dd[64:65, :], pav[64:65, :])
                        rec_ps = pacc.tile([HD, S], F32, tag="ps_a", name="recp",
                                           bufs=3)
                        nc.tensor.matmul(rec_ps[:], lhsT=ones_bc[64:65, :],
                                         rhs=dd[64:65, :], tile_position=(64, 0))
                        rec_s = tp.tile([HD, S], F32, tag="rec_s", name="rec_s",
                                        bufs=2)
                        nc.vector.tensor_copy(rec_s[:], rec_ps[:])
                        o = pp_o.tile([HD, S], BF16, tag=f"o{2 * p + hh}",
                                      name=f"o{2 * p + hh}")
                        nc.vector.tensor_tensor(out=o[:], in0=pav[0:HD, :],
                                                in1=rec_s[:], op=OP.mult)
                        oT.append(o)

                # output projection + bias + residual (LN stats fed per tile)
                psm1, pss1 = _ln_start(pools)
                for m in range(NE):
                    w = ap_t.tile([HD, H, P], BF16, tag="wo", name="wo", bufs=3)
                    nc.scalar.dma_start(out=w[:], in_=d_wor.ap()[l, m])
                    ps = pacc.tile([P, S], F32, tag="ps_a", name="acc", bufs=3)
                    for h in range(H):
                        nc.tensor.matmul(ps[:], lhsT=w[:, h, :], rhs=oT[h][:],
                                         start=(h == 0), stop=(h == H - 1))
                    yv = tp.tile([P, S], F32, tag="yv", name="yv", bufs=2)
                    nc.scalar.add(yv[:], ps[:], t_bo[l][:, m:m + 1])
                    nc.vector.tensor_tensor(out=sum_t[m][:], in0=yv[:], in1=xT[m][:],
                                            op=OP.add)
                    _ln_feed(nc, pools, psm1, pss1, sum_t[m], m, ones_f)

            _ln_finish(nc, pools, psm1, pss1, sum_t, xT,
                       t_ln1g[l], t_ln1b[l], eps_t)

            # FFN (bf16 matmuls, fp32 psum + residual)
            with tc.tile_pool(name="ht", bufs=1) as pp_h, \
                 tc.tile_pool(name="xbf", bufs=1) as pp_xbf:
                xF = []
                for e in range(NE):
                    xf = pp_xbf.tile([P, S], BF16, tag=f"xf{e}", name=f"xf{e}")
                    if e % 2 == 0:
                        nc.vector.tensor_copy(xf[:], xT[e][:])
                    else:
                        nc.scalar.copy(xf[:], xT[e][:])
                    xF.append(xf)
                hT = []
                for fg in range(NE):
                    w = pp_h.tile([P, 4, NE, P], BF16, tag="w4", name="w4", bufs=3)
                    nc.scalar.dma_start(out=w[:], in_=d_w1r.ap()[l, fg])
                    for fi in range(4):
                        f = fg * 4 + fi
                        ps = pacc.tile([P, S], F32, tag="ps_a", name="acc", bufs=3)
                        for k in range(NE):
                            nc.tensor.matmul(ps[:], lhsT=w[:, fi, k, :], rhs=xF[k][:],
                                             start=(k == 0), stop=(k == NE - 1))
                        h = pp_h.tile([P, S], BF16, tag=f"h{f}", name=f"h{f}")
                        nc.scalar.activation(h[:], ps[:], AF.Relu,
                                             bias=t_b1[l][:, f:f + 1])
                        hT.append(h)
                psm2, pss2 = _ln_start(pools)
                for m in range(NE):
                    w = pp_h.tile([P, NFT, P], BF16, tag="w32", name="w32", bufs=2)
                    nc.scalar.dma_start(out=w[:], in_=d_w2r.ap()[l, m])
                    ps = pacc.tile([P, S], F32, tag="ps_a", name="acc", bufs=3)
                    for f in range(NFT):
                        nc.tensor.matmul(ps[:], lhsT=w[:, f, :], rhs=hT[f][:],
                                         start=(f == 0), stop=(f == NFT - 1))
                    yv = tp.tile([P, S], F32, tag="yv", name="yv", bufs=2)
                    nc.scalar.add(yv[:], ps[:], t_b2[l][:, m:m + 1])
                    nc.vector.tensor_tensor(out=sum_t[m][:], in0=yv[:], in1=xT[m][:],
                                            op=OP.add)
                    _ln_feed(nc, pools, psm2, pss2, sum_t[m], m, ones_f)
            _ln_finish(nc, pools, psm2, pss2, sum_t, xT,
                       t_ln2g[l], t_ln2b[l], eps_t)

        # ---- final LN + lm_head ----
        # Error-compensated DoubleRow: W is shipped as interleaved fp8 pairs
        # (W8 = fp8(64W), Rw8 = fp8(16*(64W - W8))); x as fp8 x8 plus residual
        # r8 = fp8(x - x8). logits = [x8.(W8 + Rw8/16) + r8.W8]/64 via 12
        # half-rate DR matmuls per vocab tile (vs 8 full-rate bf16).
        _ln_tiles(nc, tc, pools, xT, sum_t, t_lnfg, t_lnfb, ones_f, eps_t)
        with tc.tile_pool(name="lg", bufs=1) as pp_lg, \
             tc.tile_pool(name="xb", bufs=1) as pp_xb:
            xpair = [pp_xb.tile([P, 2, S], FP8, tag=f"xp{k}", name=f"xp{k}")
                     for k in range(NE)]
            rpair = [pp_xb.tile([P, 2, S], FP8, tag=f"rp{kp}", name=f"rp{kp}")
                     for kp in range(NE // 2)]
            for e in range(NE):
                nc.vector.tensor_copy(xpair[e][:, 0, :], sum_t[e][:])
                x8d = tp.tile([P, S], F32, tag="x8d", name="x8d", bufs=2)
                nc.scalar.copy(x8d[:], xpair[e][:, 0, :])
                rt = tp.tile([P, S], F32, tag="rt", name="rt", bufs=2)
                nc.vector.tensor_tensor(out=rt[:], in0=sum_t[e][:], in1=x8d[:],
                                        op=OP.subtract)
                nc.vector.tensor_copy(rpair[e // 2][:, e % 2, :], rt[:])
                nc.vector.tensor_scalar(xpair[e][:, 1, :], xpair[e][:, 0, :],
                                        1.0 / 16.0, None, OP.mult)
            for vg in range(NVG):
                w = pp_lg.tile([P, VGT, NE, 2, P], FP8, tag="wlmt", name="wlmt",
                               bufs=3)
                nc.scalar.dma_start(out=w[:], in_=d_wlm.ap()[vg])
                lgc = pp_lg.tile([P, VGT, S], BF16, tag="lg", name="lg", bufs=3)
                for u in range(VGT):
                    vt = vg * VGT + u
                    ps = pacc.tile([P, S], F32, tag="ps_a", name="acc", bufs=3)
                    for k in range(NE):
                        nc.tensor.matmul(
                            ps[:], lhsT=w[:, u, k, :, :], rhs=xpair[k][:],
                            start=(k == 0), stop=False,
                            perf_mode=mybir.MatmulPerfMode.DoubleRow)
                    for kp in range(NE // 2):
                        nc.tensor.matmul(
                            ps[:], lhsT=w[:, u, 2 * kp:2 * kp + 2, 0, :],
                            rhs=rpair[kp][:],
                            start=False, stop=(kp == NE // 2 - 1),
                            perf_mode=mybir.MatmulPerfMode.DoubleRow)
                    nc.scalar.activation(lgc[:, u, :], ps[:], AF.Identity,
                                         bias=t_blm[:, vt:vt + 1], scale=1.0 / 64.0)
                nc.sync.dma_start(
                    out=d_out.ap()[vg * VGT * P:(vg + 1) * VGT * P, :]
                    .rearrange("(u p) s -> p u s", p=P),
                    in_=lgc[:])

    nc.compile()
    return nc


_CACHED = {}


def _sw(v):
    """[.., n*P] vector -> [.., P, n] partition-major fp32."""
    v = np.asarray(v, np.float32)
    return np.ascontiguousarray(
        v.reshape(*v.shape[:-1], v.shape[-1] // P, P).swapaxes(-1, -2))


def _prep_weights(inputs):
    f32 = np.float32
    bf = ml_dtypes.bfloat16
    Wq, Wk, Wv = inputs["Wq"], inputs["Wk"], inputs["Wv"]
    # [L,H,E,HD] -> [L,E,H*HD]
    wq_flat = np.ascontiguousarray(Wq.transpose(0, 2, 1, 3).reshape(L, E, H * HD))
    wk_flat = np.ascontiguousarray(Wk.transpose(0, 2, 1, 3).reshape(L, E, H * HD))
    wv_flat = np.ascontiguousarray(Wv.transpose(0, 2, 1, 3).reshape(L, E, H * HD))

    def sw_ee(w):  # [L, E_in, E_out] -> [L, m, p, k, f]
        return np.ascontiguousarray(
            w.reshape(L, NE, P, NE, P).transpose(0, 3, 2, 1, 4)).astype(bf)

    wqr = sw_ee(wq_flat)
    wkr = sw_ee(wk_flat)
    # Wo in per-head layout [l, m, d, h, f] so each head's 64 contraction rows
    # sit at partitions 0-63
    wor = np.ascontiguousarray(
        inputs["Wo"].reshape(L, H, HD, NE, P).transpose(0, 3, 2, 1, 4)).astype(bf)
    wvf = np.ascontiguousarray(
        wv_flat.reshape(L, NE, P, H * HD)).astype(bf)
    w1r = np.ascontiguousarray(
        inputs["W1"].reshape(L, NE, P, NE, 4, P).transpose(0, 3, 2, 4, 1, 5)).astype(bf)
    w2r = np.ascontiguousarray(
        inputs["W2"].reshape(L, NFT, P, NE, P).transpose(0, 3, 2, 1, 4)).astype(bf)
    f8 = ml_dtypes.float8_e4m3
    w64 = (64.0 * inputs["Wlm"]).astype(f32)
    w8 = w64.astype(f8)
    rw8 = (16.0 * (w64 - w8.astype(f32))).astype(f8)
    # interleave [k, p, g, u, f] pairs -> [g, p, u, k, pair, f]
    wlmr = np.ascontiguousarray(
        np.stack([w8.reshape(NE, P, NVG, VGT, P), rw8.reshape(NE, P, NVG, VGT, P)],
                 axis=4).transpose(2, 1, 3, 0, 4, 5))
    return {
        "wqr": wqr, "wkr": wkr, "wvf": wvf, "wor": wor,
        "w1r": w1r, "w2r": w2r, "wlmr": wlmr,
        "emb": np.ascontiguousarray(inputs["emb"]).astype(bf),
        "ln1g": _sw(inputs["ln1_g"]), "ln1b": _sw(inputs["ln1_b"]),
        "ln2g": _sw(inputs["ln2_g"]), "ln2b": _sw(inputs["ln2_b"]),
        "bo": _sw(inputs["bo"]), "b1": _sw(inputs["b1"]), "b2": _sw(inputs["b2"]),
        "lnfg": _sw(inputs["lnf_g"]), "lnfb": _sw(inputs["lnf_b"]),
        "blm": _sw(inputs["blm"]),
    }


def kernel(**inputs):
    if "nc" not in _CACHED:
        _CACHED["nc"] = build_program()
    nc = _CACHED["nc"]

    shared = _prep_weights(inputs)
    index = np.asarray(inputs["index"])

    # per-core causal masks and token ids
    jpos = np.arange(S)
    ipos = np.arange(P)
    in_maps = []
    for c in range(NC):
        b, j = c // CH, c % CH
        q0 = j * S
        m = np.zeros((NSB, P, S), np.float32)
        for sb in range(NSB):
            m[sb] = ((sb * P + ipos)[:, None] <= (q0 + jpos)[None, :])
        im = dict(shared)
        im["maskp"] = m.astype(ml_dtypes.bfloat16)
        im["idx"] = np.ascontiguousarray(index[b, q0:q0 + S]).astype(np.int32)
        in_maps.append(im)

    res = bass_utils.run_bass_kernel_spmd(nc, in_maps, core_ids=list(range(NC)))
    out = np.zeros((B, T, V), np.float32)
    for c in range(NC):
        b, j = c // CH, c % CH
        out[b, j * S:(j + 1) * S, :] = res.results[c]["logt"].astype(np.float32).T
    return out


# revision 58
# speedup vs baseline: 1.4618x; 1.0002x over previous
"""Trainium2 Bass kernel for a 3-block GPT (B=2,T=2048,E=1024,H=16,V=32000).

Sharding: sequence-parallel over 8 cores (512 tokens each, weights replicated).
Per layer: QKV computed locally; K^T and V are shipped through four
pair-grouped fp8 AllGathers (per-batch groups [[0-3],[4-7]]; k and v of the
first pair-group travel separately) so attention on early head pairs overlaps
the remaining transfers. K/V are scaled x64 into fp8e4 for transport and
descaled to bf16 on arrival. Attention runs scores pair-major with exp+mask
at two-key-block granularity ([128,1024] psum tiles); AV is per-head with the
softmax denominator fused as a ones column in the v slab, then a one-row
reciprocal is broadcast back over 64 partitions with a tiny matmul. The
output projection contracts per-head [64,128] weight slices, and LayerNorm
statistics (ones-matmul partition reductions) are accumulated inside the
producing loops so only the finalize tail is exposed. lm_head streams
host-swizzled bf16 weights in 5-vocab-tile chunks and emits bf16 logits^T
[V, 512] per core; the host reassembles and upcasts. All streamed weights are
host-swizzled so each DMA element is >=2KB-contiguous per partition.
"""

import numpy as np
import ml_dtypes
from contextlib import ExitStack

import concourse.bass as bass
import concourse.mybir as mybir
import concourse.tile as tile
from concourse import bacc
from concourse.masks import make_identity
from concourse import bass_utils

# model dims (hardcoded; harness contract)
B, T, E, H, V = 2, 2048, 1024, 16, 32000
HD, L = 64, 3
NC = 8
S = (B * T) // NC        # 512 tokens per core
CH = NC // B             # 4 chunks (cores) per batch
FF = 4 * E               # 4096
EPS = 1e-5
P = 128
NE = E // P              # 8 e-tiles
NHP = H // 2             # 8 head pairs
NSB = T // P             # 16 key blocks per batch
NVT = V // P             # 250 vocab tiles
NVG = 50                 # vocab groups (5 tiles each)
VGT = NVT // NVG         # 5
NFT = FF // P            # 32 f-tiles
CS = 2 * E * S           # flat kv elems per core (k: E*S, v: S*E)
VOFF = E * S
# pair-grouped kv transport: 3 collectives so attention can start early.
# group g covers head-pairs PGRP[g]; k rows + v cols for those pairs.
PGRP = [(0, 4), (4, 7), (7, 8)]          # [start_pair, end_pair)
GW = [(e - s) * P for s, e in PGRP]      # widths: 512, 384, 128
GKSZ = [w * S for w in GW]               # k elems per group
GSZ = [2 * w * S for w in GW]            # total elems per group (k + v)
F32 = mybir.dt.float32
BF16 = mybir.dt.bfloat16
FP8 = mybir.dt.float8e4
I32 = mybir.dt.int32
KVS = 64.0               # k/v fp8 transport scale (values ~0.013 at layer 1)
AF = mybir.ActivationFunctionType
OP = mybir.AluOpType


def _ln_start(pools):
    """Allocate psum accumulators for LN stats (sum, sum-of-squares)."""
    pacc = pools["pacc"]
    psm = pacc.tile([P, S], F32, tag="ps_b", name="ln_ps", bufs=2)
    pss = pacc.tile([P, S], F32, tag="ps_b", name="ln_ps", bufs=2)
    return psm, pss


def _ln_feed(nc, pools, psm, pss, src_e, e, ones_f):
    """Accumulate LN stats for e-tile `e` (call as each tile is produced)."""
    tp = pools["tp"]
    nc.tensor.matmul(psm[:], lhsT=ones_f[:], rhs=src_e[:],
                     start=(e == 0), stop=(e == NE - 1), skip_group_check=True)
    sq = tp.tile([P, S], F32, tag="ln_sq", name="ln_sq", bufs=2)
    nc.scalar.square(sq[:], src_e[:])
    nc.tensor.matmul(pss[:], lhsT=ones_f[:], rhs=sq[:],
                     start=(e == 0), stop=(e == NE - 1), skip_group_check=True)


def _ln_finish(nc, pools, psm, pss, src, out, g_t, b_t, eps_t):
    """Finalize LN: mean/var/rstd then per-tile normalize."""
    tp = pools["tp"]
    mean = tp.tile([P, S], F32, tag="ln_mean", name="ln_mean")
    nc.scalar.mul(mean[:], psm[:], 1.0 / E)
    msq = tp.tile([P, S], F32, tag="ln_msq", name="ln_msq")
    nc.scalar.square(msq[:], mean[:])
    var = tp.tile([P, S], F32, tag="ln_var", name="ln_var")
    nc.vector.tensor_scalar(var[:], pss[:], 1.0 / E, None, OP.mult)
    nc.vector.tensor_tensor(out=var[:], in0=var[:], in1=msq[:], op=OP.subtract)
    std = tp.tile([P, S], F32, tag="ln_std", name="ln_std")
    nc.scalar.activation(std[:], var[:], AF.Sqrt, bias=eps_t[:])
    rstd = tp.tile([P, S], F32, tag="ln_rstd", name="ln_rstd")
    nc.vector.reciprocal(rstd[:], std[:])
    for e in range(NE):
        t = tp.tile([P, S], F32, tag="ln_t", name="ln_t", bufs=2)
        nc.vector.tensor_tensor(out=t[:], in0=src[e][:], in1=mean[:], op=OP.subtract)
        nc.vector.tensor_tensor(out=t[:], in0=t[:], in1=rstd[:], op=OP.mult)
        nc.scalar.activation(out[e][:], t[:], AF.Identity,
                             bias=b_t[:, e:e + 1], scale=g_t[:, e:e + 1])


def _ln_tiles(nc, tc, pools, src, out, g_t, b_t, ones_f, eps_t):
    """LayerNorm over E (partition axis across the 8 [128,S] tiles of src)."""
    psm, pss = _ln_start(pools)
    for e in range(NE):
        _ln_feed(nc, pools, psm, pss, src[e], e, ones_f)
    _ln_finish(nc, pools, psm, pss, src, out, g_t, b_t, eps_t)


def build_program():
    nc = bacc.Bacc("TRN2", target_bir_lowering=False, debug=False, num_devices=NC)

    # ---- DRAM I/O (all streamed weights pre-swizzled on host so each tile is
    # contiguous per partition) ----
    d_wqr = nc.dram_tensor("wqr", [L, NE, P, NE, P], BF16, kind="ExternalInput")
    d_wkr = nc.dram_tensor("wkr", [L, NE, P, NE, P], BF16, kind="ExternalInput")
    d_wvf = nc.dram_tensor("wvf", [L, NE, P, H * HD], BF16, kind="ExternalInput")
    d_wor = nc.dram_tensor("wor", [L, NE, HD, H, P], BF16, kind="ExternalInput")
    d_w1r = nc.dram_tensor("w1r", [L, NE, P, 4, NE, P], BF16, kind="ExternalInput")
    d_w2r = nc.dram_tensor("w2r", [L, NE, P, NFT, P], BF16, kind="ExternalInput")
    d_wlm = nc.dram_tensor("wlmr", [NVG, P, VGT, NE, 2, P], FP8,
                           kind="ExternalInput")
    d_emb = nc.dram_tensor("emb", [V, E], BF16, kind="ExternalInput")
    d_idx = nc.dram_tensor("idx", [S], I32, kind="ExternalInput")
    d_msk = nc.dram_tensor("maskp", [NSB, P, S], BF16, kind="ExternalInput")
    d_ln1g = nc.dram_tensor("ln1g", [L, P, NE], F32, kind="ExternalInput")
    d_ln1b = nc.dram_tensor("ln1b", [L, P, NE], F32, kind="ExternalInput")
    d_ln2g = nc.dram_tensor("ln2g", [L, P, NE], F32, kind="ExternalInput")
    d_ln2b = nc.dram_tensor("ln2b", [L, P, NE], F32, kind="ExternalInput")
    d_bo = nc.dram_tensor("bo", [L, P, NE], F32, kind="ExternalInput")
    d_b1 = nc.dram_tensor("b1", [L, P, NFT], F32, kind="ExternalInput")
    d_b2 = nc.dram_tensor("b2", [L, P, NE], F32, kind="ExternalInput")
    d_lnfg = nc.dram_tensor("lnfg", [P, NE], F32, kind="ExternalInput")
    d_lnfb = nc.dram_tensor("lnfb", [P, NE], F32, kind="ExternalInput")
    d_blm = nc.dram_tensor("blm", [P, NVT], F32, kind="ExternalInput")
    d_out = nc.dram_tensor("logt", [V, S], BF16, kind="ExternalOutput")

    groups = [[0, 1, 2, 3], [4, 5, 6, 7]]

    with ExitStack() as ctx:
        tc = ctx.enter_context(tile.TileContext(nc, num_cores=NC))
        const = ctx.enter_context(tc.tile_pool(name="const", bufs=1))
        pp_x = ctx.enter_context(tc.tile_pool(name="xres", bufs=1))
        pp_sum = ctx.enter_context(tc.tile_pool(name="xsum", bufs=1))
        pp_msk = ctx.enter_context(tc.tile_pool(name="masks", bufs=1))
        tp = ctx.enter_context(tc.tile_pool(name="tp", bufs=1))
        pacc = ctx.enter_context(tc.tile_pool(name="pacc", bufs=1, space="PSUM"))
        dram = ctx.enter_context(tc.tile_pool(name="ccdram", bufs=2, space="DRAM"))
        pools = {"pacc": pacc, "tp": tp}

        # ---- embedding gather first: critical path at startup ----
        idx_t = const.tile([P, S // P], I32, name="idx_t")
        nc.gpsimd.dma_start(out=idx_t[:], in_=d_idx.ap().rearrange("(g p) -> p g", p=P))

        # queue all four gathers on the Pool engine before anything else
        xgs = []
        for g in range(S // P):
            xg = tp.tile([P, E], BF16, tag="embg", name="embg", bufs=4)
            nc.gpsimd.indirect_dma_start(
                out=xg[:], out_offset=None, in_=d_emb.ap(),
                in_offset=bass.IndirectOffsetOnAxis(ap=idx_t[:, g:g + 1], axis=0))
            xgs.append(xg)

        ident = const.tile([P, P], F32, name="ident")
        make_identity(nc, ident[:])
        identb = const.tile([P, P], BF16, name="identb")
        nc.vector.tensor_copy(identb[:], ident[:])
        ones_f = const.tile([P, P], F32, name="ones_f")
        nc.vector.memset(ones_f[:], 1.0)
        ones_bc = const.tile([P, HD], F32, name="ones_bc")
        nc.vector.memset(ones_bc[:], 1.0)
        eps_t = const.tile([P, 1], F32, name="eps_t")
        nc.vector.memset(eps_t[:], EPS)

        # residual stream xT: 8 tiles [128, S]
        xT = [pp_x.tile([P, S], F32, tag=f"x{e}", name=f"x{e}") for e in range(NE)]
        sum_t = [pp_sum.tile([P, S], F32, tag=f"s{e}", name=f"s{e}") for e in range(NE)]

        for g in range(S // P):
            for e in range(NE):
                pst = pacc.tile([P, S], BF16, tag="ps_a", name="tpose", bufs=3)
                nc.tensor.transpose(pst[:, 0:P], xgs[g][:, e * P:(e + 1) * P],
                                    identb[:])
                nc.vector.tensor_copy(xT[e][:, g * P:(g + 1) * P], pst[:, 0:P])

        # ---- params -> [128, n] tiles (after the gather chain is queued) ----
        def ldvec(dt_ap, n, name):
            t = const.tile([P, n], F32, tag=name, name=name)
            nc.gpsimd.dma_start(out=t[:], in_=dt_ap)
            return t

        t_ln1g = [ldvec(d_ln1g.ap()[l], NE, f"ln1g{l}") for l in range(L)]
        t_ln1b = [ldvec(d_ln1b.ap()[l], NE, f"ln1b{l}") for l in range(L)]
        t_ln2g = [ldvec(d_ln2g.ap()[l], NE, f"ln2g{l}") for l in range(L)]
        t_ln2b = [ldvec(d_ln2b.ap()[l], NE, f"ln2b{l}") for l in range(L)]
        t_bo = [ldvec(d_bo.ap()[l], NE, f"bo{l}") for l in range(L)]
        t_b1 = [ldvec(d_b1.ap()[l], NFT, f"b1{l}") for l in range(L)]
        t_b2 = [ldvec(d_b2.ap()[l], NE, f"b2{l}") for l in range(L)]
        t_lnfg = ldvec(d_lnfg.ap(), NE, "lnfg")
        t_lnfb = ldvec(d_lnfb.ap(), NE, "lnfb")
        t_blm = ldvec(d_blm.ap(), NVT, "blm")

        # causal masks (bf16 multiplicative, per-core data): one DMA
        mask3 = pp_msk.tile([P, NSB * S], BF16, tag="msk", name="msk")
        nc.gpsimd.dma_start(out=mask3[:].rearrange("p (sb s) -> p sb s", sb=NSB),
                    in_=d_msk.ap().rearrange("sb p s -> p sb s"))

        # ---- transformer blocks ----
        for l in range(L):
            with tc.tile_pool(name="qT", bufs=1) as pp_q, \
                 tc.tile_pool(name="oT", bufs=1) as pp_o, \
                 tc.tile_pool(name="attn", bufs=1) as ap_t, \
                 tc.tile_pool(name="vslab", bufs=1) as vp:

                # bf16 activations
                xA = []
                for e in range(NE):
                    xa = ap_t.tile([P, S], BF16, tag=f"xa{e}", name=f"xa{e}")
                    if e % 2 == 0:
                        nc.vector.tensor_copy(xa[:], xT[e][:])
                    else:
                        nc.scalar.copy(xa[:], xT[e][:])
                    xA.append(xa)

                # k/v projections feed pair-grouped transport buffers so the
                # AllGathers launch incrementally and overlap attention.
                # Group 0 ships k and v separately: scores for pairs 0-3 can
                # start as soon as their k rows arrive.
                kvcat = [dram.tile([GKSZ[0]], FP8, tag="kvcat0", name="kvcat0"),
                         dram.tile([GSZ[1]], FP8, tag="kvcat1", name="kvcat1"),
                         dram.tile([GSZ[2]], FP8, tag="kvcat2", name="kvcat2")]
                kvout = [dram.tile([CH * GKSZ[0]], FP8, tag="kvout0", name="kvout0"),
                         dram.tile([CH * GSZ[1]], FP8, tag="kvout1", name="kvout1"),
                         dram.tile([CH * GSZ[2]], FP8, tag="kvout2", name="kvout2")]
                c0v = dram.tile([S * GW[0]], FP8, tag="c0v", name="c0v")
                o0v = dram.tile([CH * S * GW[0]], FP8, tag="o0v", name="o0v")

                def kproj(m, g, lm):
                    w = ap_t.tile([P, NE, P], BF16, tag="wb", name="wb", bufs=3)
                    nc.scalar.dma_start(out=w[:], in_=d_wkr.ap()[l, m])
                    ps = pacc.tile([P, S], F32, tag="ps_a", name="acc", bufs=3)
                    for k in range(NE):
                        nc.tensor.matmul(ps[:], lhsT=w[:, k, :], rhs=xA[k][:],
                                         start=(k == 0), stop=(k == NE - 1))
                    kl = tp.tile([P, S], FP8, tag="klocal", name="klocal", bufs=2)
                    nc.scalar.mul(kl[:], ps[:], KVS)
                    nc.sync.dma_start(
                        out=kvcat[g][lm * P * S:(lm + 1) * P * S].rearrange(
                            "(p s) -> p s", p=P),
                        in_=kl[:])

                def vproj(n):
                    # psv halves cover tokens tb*128.., head cols n*512..n*512+512
                    ps2 = [pacc.tile([P, 2, 512], F32, tag="ps_a", name="acc", bufs=3)
                           for _ in range(2)]
                    psv = [ps2[0][:, 0, :], ps2[0][:, 1, :],
                           ps2[1][:, 0, :], ps2[1][:, 1, :]]
                    for k in range(NE):
                        wv = ap_t.tile([P, 512], BF16, tag="wv", name="wv", bufs=3)
                        nc.scalar.dma_start(
                            out=wv[:],
                            in_=d_wvf.ap()[l, k, :, n * 512:(n + 1) * 512])
                        for m in range(4):
                            nc.tensor.matmul(psv[m], lhsT=xA[k][:, m * P:(m + 1) * P],
                                             rhs=wv[:], start=(k == 0), stop=(k == NE - 1))
                    for tb in range(4):
                        vv = tp.tile([P, 512], FP8, tag="vlocal", name="vlocal", bufs=2)
                        nc.scalar.mul(vv[:], psv[tb], KVS)
                        if n == 0:  # all 512 cols belong to group 0
                            nc.sync.dma_start(
                                out=c0v[tb * P * GW[0]:(tb + 1) * P * GW[0]]
                                .rearrange("(p f) -> p f", p=P),
                                in_=vv[:])
                        else:  # cols 0-383 -> group 1, cols 384-511 -> group 2
                            nc.sync.dma_start(
                                out=kvcat[1][GKSZ[1] + tb * P * GW[1]:
                                             GKSZ[1] + (tb + 1) * P * GW[1]]
                                .rearrange("(p f) -> p f", p=P),
                                in_=vv[:, 0:GW[1]])
                            nc.sync.dma_start(
                                out=kvcat[2][GKSZ[2] + tb * P * GW[2]:
                                             GKSZ[2] + (tb + 1) * P * GW[2]]
                                .rearrange("(p f) -> p f", p=P),
                                in_=vv[:, GW[1]:512])

                for m in range(4):
                    kproj(m, 0, m)
                nc.gpsimd.collective_compute(
                    "AllGather", OP.bypass, replica_groups=groups,
                    ins=[kvcat[0][:]], outs=[kvout[0][:]])
                vproj(0)
                nc.gpsimd.collective_compute(
                    "AllGather", OP.bypass, replica_groups=groups,
                    ins=[c0v[:]], outs=[o0v[:]])
                for m in range(4, 7):
                    kproj(m, 1, m - 4)
                kproj(7, 2, 0)
                vproj(1)
                nc.gpsimd.collective_compute(
                    "AllGather", OP.bypass, replica_groups=groups,
                    ins=[kvcat[1][:]], outs=[kvout[1][:]])
                nc.gpsimd.collective_compute(
                    "AllGather", OP.bypass, replica_groups=groups,
                    ins=[kvcat[2][:]], outs=[kvout[2][:]])

                qT = []
                for m in range(NE):
                    w = ap_t.tile([P, NE, P], BF16, tag="wb", name="wb", bufs=3)
                    nc.scalar.dma_start(out=w[:], in_=d_wqr.ap()[l, m])
                    ps = pacc.tile([P, S], F32, tag="ps_a", name="acc", bufs=3)
                    for k in range(NE):
                        nc.tensor.matmul(ps[:], lhsT=w[:, k, :], rhs=xA[k][:],
                                         start=(k == 0), stop=(k == NE - 1))
                    q = pp_q.tile([P, S], BF16, tag=f"q{m}", name=f"q{m}")
                    nc.scalar.copy(q[:], ps[:])
                    qT.append(q)

                # attention: scores pair-major, AV per head with the softmax
                # denominator fused as a ones column in the v slab
                oT = []
                for p in range(NHP):
                    g = 0 if p < 4 else (1 if p < 7 else 2)
                    lm = p - PGRP[g][0]
                    pav_e = pacc.tile([65, S], F32, tag="ps_b", name="pave", bufs=2)
                    pav_o = pacc.tile([65, S], F32, tag="ps_b", name="pavo", bufs=2)
                    vsl8 = vp.tile([P, CH, 4, P], FP8, tag="vsl8", name="vsl8", bufs=2)
                    for cc in range(CH):
                        if g == 0:
                            vsrc, vbase = o0v, cc * S * GW[0]
                        else:
                            vsrc, vbase = kvout[g], cc * GSZ[g] + GKSZ[g]
                        nc.sync.dma_start(
                            out=vsl8[:, cc, :, :],
                            in_=vsrc[vbase:vbase + S * GW[g]].rearrange(
                                "(tb q f) -> q tb f", tb=4, q=P)[
                                :, :, lm * P:(lm + 1) * P])
                    # [v_even(64) | 1 | v_odd(64) | 1] per key block
                    vslab = vp.tile([P, CH, 4, 130], BF16, tag="vsl", name="vsl",
                                    bufs=1)
                    nc.vector.tensor_scalar(vslab[:, :, :, 0:HD],
                                            vsl8[:, :, :, 0:HD],
                                            1.0 / KVS, None, OP.mult)
                    nc.vector.tensor_scalar(vslab[:, :, :, 65:65 + HD],
                                            vsl8[:, :, :, HD:P],
                                            1.0 / KVS, None, OP.mult)
                    nc.vector.memset(vslab[:, :, :, HD:HD + 1], 1.0)
                    nc.vector.memset(vslab[:, :, :, 129:130], 1.0)
                    for cc in range(CH):
                        kbase = (cc * GKSZ[0] if g == 0 else cc * GSZ[g]) + lm * P * S
                        ksl8 = ap_t.tile([P, S], FP8, tag="ksl8", name="ksl8", bufs=3)
                        nc.sync.dma_start(
                            out=ksl8[:],
                            in_=kvout[g][kbase:kbase + P * S]
                            .rearrange("(p s) -> p s", p=P))
                        ksl = ap_t.tile([P, S], BF16, tag="kslab", name="kslab", bufs=3)
                        nc.vector.tensor_scalar(ksl[:], ksl8[:], 1.0 / KVS, None, OP.mult)
                        for jh in range(2):
                            sb0 = 4 * cc + 2 * jh
                            sA = pacc.tile([P, 2, S], F32, tag="ps_a", name="sc",
                                           bufs=3)
                            sB = pacc.tile([P, 2, S], F32, tag="ps_a", name="sc",
                                           bufs=3)
                            for d in range(2):
                                j2 = 2 * jh + d
                                nc.tensor.matmul(
                                    sA[:, d, :], lhsT=ksl[0:HD, j2 * P:(j2 + 1) * P],
                                    rhs=qT[p][0:HD, :], tile_position=(0, 0))
                                nc.tensor.matmul(
                                    sB[:, d, :], lhsT=ksl[HD:P, j2 * P:(j2 + 1) * P],
                                    rhs=qT[p][HD:P, :], tile_position=(64, 0))
                            pa = ap_t.tile([P, 2 * S], BF16, tag="pt", name="pt", bufs=4)
                            pb = ap_t.tile([P, 2 * S], BF16, tag="pt", name="pt", bufs=4)
                            nc.scalar.activation(pa[:], sA[:], AF.Exp, scale=HD ** -0.5)
                            nc.scalar.activation(pb[:], sB[:], AF.Exp, scale=HD ** -0.5)
                            nc.vector.tensor_tensor(out=pa[:], in0=pa[:],
                                                    in1=mask3[:, sb0 * S:(sb0 + 2) * S],
                                                    op=OP.mult)
                            nc.vector.tensor_tensor(out=pb[:], in0=pb[:],
                                                    in1=mask3[:, sb0 * S:(sb0 + 2) * S],
                                                    op=OP.mult)
                            for d in range(2):
                                sb = sb0 + d
                                first, last = (sb == 0), (sb == NSB - 1)
                                j2 = 2 * jh + d
                                nc.tensor.matmul(
                                    pav_e[:], lhsT=vslab[:, cc, j2, 0:65],
                                    rhs=pa[:, d * S:(d + 1) * S], start=first,
                                    stop=last, skip_group_check=True)
                                nc.tensor.matmul(
                                    pav_o[:], lhsT=vslab[:, cc, j2, 65:130],
                                    rhs=pb[:, d * S:(d + 1) * S], start=first,
                                    stop=last, skip_group_check=True)
                    for hh, pav in ((0, pav_e), (1, pav_o)):
                        dd = tp.tile([65, S], F32, tag="dd", name="dd", bufs=2)
                        nc.vector.reciprocal(dd[64:65, :], pav[64:65, :])
                        rec_ps = pacc.tile([HD, S], F32, tag="ps_a", name="recp",
                                           bufs=3)
                        nc.tensor.matmul(rec_ps[:], lhsT=ones_bc[64:65, :],
                                         rhs=dd[64:65, :], tile_position=(64, 0))
                        rec_s = tp.tile([HD, S], F32, tag="rec_s", name="rec_s",
                                        bufs=2)
                        nc.vector.tensor_copy(rec_s[:], rec_ps[:])
                        o = pp_o.tile([HD, S], BF16, tag=f"o{2 * p + hh}",
                                      name=f"o{2 * p + hh}")
                        nc.vector.tensor_tensor(out=o[:], in0=pav[0:HD, :],
                                                in1=rec_s[:], op=OP.mult)
                        oT.append(o)

                # output projection + bias + residual (LN stats fed per tile)
                psm1, pss1 = _ln_start(pools)
                for m in range(NE):
                    w = ap_t.tile([HD, H, P], BF16, tag="wo", name="wo", bufs=3)
                    nc.scalar.dma_start(out=w[:], in_=d_wor.ap()[l, m])
                    ps = pacc.tile([P, S], F32, tag="ps_a", name="acc", bufs=3)
                    for h in range(H):
                        nc.tensor.matmul(ps[:], lhsT=w[:, h, :], rhs=oT[h][:],
                                         start=(h == 0), stop=(h == H - 1))
                    yv = tp.tile([P, S], F32, tag="yv", name="yv", bufs=2)
                    nc.scalar.add(yv[:], ps[:], t_bo[l][:, m:m + 1])
                    nc.vector.tensor_tensor(out=sum_t[m][:], in0=yv[:], in1=xT[m][:],
                                            op=OP.add)
                    _ln_feed(nc, pools, psm1, pss1, sum_t[m], m, ones_f)

            _ln_finish(nc, pools, psm1, pss1, sum_t, xT,
                       t_ln1g[l], t_ln1b[l], eps_t)

            # FFN (bf16 matmuls, fp32 psum + residual)
            with tc.tile_pool(name="ht", bufs=1) as pp_h, \
                 tc.tile_pool(name="xbf", bufs=1) as pp_xbf:
                xF = []
                for e in range(NE):
                    xf = pp_xbf.tile([P, S], BF16, tag=f"xf{e}", name=f"xf{e}")
                    if e % 2 == 0:
                        nc.vector.tensor_copy(xf[:], xT[e][:])
                    else:
                        nc.scalar.copy(xf[:], xT[e][:])
                    xF.append(xf)
                hT = []
                for fg in range(NE):
                    w = pp_h.tile([P, 4, NE, P], BF16, tag="w4", name="w4", bufs=3)
                    nc.scalar.dma_start(out=w[:], in_=d_w1r.ap()[l, fg])
                    for fi in range(4):
                        f = fg * 4 + fi
                        ps = pacc.tile([P, S], F32, tag="ps_a", name="acc", bufs=3)
                        for k in range(NE):
                            nc.tensor.matmul(ps[:], lhsT=w[:, fi, k, :], rhs=xF[k][:],
                                             start=(k == 0), stop=(k == NE - 1))
                        h = pp_h.tile([P, S], BF16, tag=f"h{f}", name=f"h{f}")
                        nc.scalar.activation(h[:], ps[:], AF.Relu,
                                             bias=t_b1[l][:, f:f + 1])
                        hT.append(h)
                psm2, pss2 = _ln_start(pools)
                for m in range(NE):
                    w = pp_h.tile([P, NFT, P], BF16, tag="w32", name="w32", bufs=2)
                    nc.scalar.dma_start(out=w[:], in_=d_w2r.ap()[l, m])
                    ps = pacc.tile([P, S], F32, tag="ps_a", name="acc", bufs=3)
                    for f in range(NFT):
                        nc.tensor.matmul(ps[:], lhsT=w[:, f, :], rhs=hT[f][:],
                                         start=(f == 0), stop=(f == NFT - 1))
                    yv = tp.tile([P, S], F32, tag="yv", name="yv", bufs=2)
                    nc.scalar.add(yv[:], ps[:], t_b2[l][:, m:m + 1])
                    nc.vector.tensor_tensor(out=sum_t[m][:], in0=yv[:], in1=xT[m][:],
                                            op=OP.add)
                    _ln_feed(nc, pools, psm2, pss2, sum_t[m], m, ones_f)
            _ln_finish(nc, pools, psm2, pss2, sum_t, xT,
                       t_ln2g[l], t_ln2b[l], eps_t)

        # ---- final LN + lm_head ----
        # Error-compensated DoubleRow: W is shipped as interleaved fp8 pairs
        # (W8 = fp8(64W), Rw8 = fp8(16*(64W - W8))); x as fp8 x8 plus residual
        # r8 = fp8(x - x8). logits = [x8.(W8 + Rw8/16) + r8.W8]/64 via 12
        # half-rate DR matmuls per vocab tile (vs 8 full-rate bf16).
        _ln_tiles(nc, tc, pools, xT, sum_t, t_lnfg, t_lnfb, ones_f, eps_t)
        with tc.tile_pool(name="lg", bufs=1) as pp_lg, \
             tc.tile_pool(name="xb", bufs=1) as pp_xb:
            xpair = [pp_xb.tile([P, 2, S], FP8, tag=f"xp{k}", name=f"xp{k}")
                     for k in range(NE)]
            rpair = [pp_xb.tile([P, 2, S], FP8, tag=f"rp{kp}", name=f"rp{kp}")
                     for kp in range(NE // 2)]
            for e in range(NE):
                nc.vector.tensor_copy(xpair[e][:, 0, :], sum_t[e][:])
                x8d = tp.tile([P, S], F32, tag="x8d", name="x8d", bufs=2)
                nc.scalar.copy(x8d[:], xpair[e][:, 0, :])
                rt = tp.tile([P, S], F32, tag="rt", name="rt", bufs=2)
                nc.vector.tensor_tensor(out=rt[:], in0=sum_t[e][:], in1=x8d[:],
                                        op=OP.subtract)
                nc.vector.tensor_copy(rpair[e // 2][:, e % 2, :], rt[:])
                nc.vector.tensor_scalar(xpair[e][:, 1, :], xpair[e][:, 0, :],
                                        1.0 / 16.0, None, OP.mult)
            for vg in range(NVG):
                w = pp_lg.tile([P, VGT, NE, 2, P], FP8, tag="wlmt", name="wlmt",
                               bufs=3)
                nc.scalar.dma_start(out=w[:], in_=d_wlm.ap()[vg])
                lgc = pp_lg.tile([P, VGT, S], BF16, tag="lg", name="lg", bufs=3)
                for u in range(VGT):
                    vt = vg * VGT + u
                    ps = pacc.tile([P, S], F32, tag="ps_a", name="acc", bufs=3)
                    for k in range(NE):
                        nc.tensor.matmul(
                            ps[:], lhsT=w[:, u, k, :, :], rhs=xpair[k][:],
                            start=(k == 0), stop=False,
                            perf_mode=mybir.MatmulPerfMode.DoubleRow)
                    for kp in range(NE // 2):
                        nc.tensor.matmul(
                            ps[:], lhsT=w[:, u, 2 * kp:2 * kp + 2, 0, :],
                            rhs=rpair[kp][:],
                            start=False, stop=(kp == NE // 2 - 1),
                            perf_mode=mybir.MatmulPerfMode.DoubleRow)
                    nc.scalar.activation(lgc[:, u, :], ps[:], AF.Identity,
                                         bias=t_blm[:, vt:vt + 1], scale=1.0 / 64.0)
                nc.sync.dma_start(
                    out=d_out.ap()[vg * VGT * P:(vg + 1) * VGT * P, :]
                    .rearrange("(u p) s -> p u s", p=P),
                    in_=lgc[:])

    nc.compile()
    return nc


_CACHED = {}


def _sw(v):
    """[.., n*P] vector -> [.., P, n] partition-major fp32."""
    v = np.asarray(v, np.float32)
    return np.ascontiguousarray(
        v.reshape(*v.shape[:-1], v.shape[-1] // P, P).swapaxes(-1, -2))


def _prep_weights(inputs):
    f32 = np.float32
    bf = ml_dtypes.bfloat16
    Wq, Wk, Wv = inputs["Wq"], inputs["Wk"], inputs["Wv"]
    # [L,H,E,HD] -> [L,E,H*HD]
    wq_flat = np.ascontiguousarray(Wq.transpose(0, 2, 1, 3).reshape(L, E, H * HD))
    wk_flat = np.ascontiguousarray(Wk.transpose(0, 2, 1, 3).reshape(L, E, H * HD))
    wv_flat = np.ascontiguousarray(Wv.transpose(0, 2, 1, 3).reshape(L, E, H * HD))

    def sw_ee(w):  # [L, E_in, E_out] -> [L, m, p, k, f]
        return np.ascontiguousarray(
            w.reshape(L, NE, P, NE, P).transpose(0, 3, 2, 1, 4)).astype(bf)

    wqr = sw_ee(wq_flat)
    wkr = sw_ee(wk_flat)
    # Wo in per-head layout [l, m, d, h, f] so each head's 64 contraction rows
    # sit at partitions 0-63
    wor = np.ascontiguousarray(
        inputs["Wo"].reshape(L, H, HD, NE, P).transpose(0, 3, 2, 1, 4)).astype(bf)
    wvf = np.ascontiguousarray(
        wv_flat.reshape(L, NE, P, H * HD)).astype(bf)
    w1r = np.ascontiguousarray(
        inputs["W1"].reshape(L, NE, P, NE, 4, P).transpose(0, 3, 2, 4, 1, 5)).astype(bf)
    w2r = np.ascontiguousarray(
        inputs["W2"].reshape(L, NFT, P, NE, P).transpose(0, 3, 2, 1, 4)).astype(bf)
    f8 = ml_dtypes.float8_e4m3
    w64 = (64.0 * inputs["Wlm"]).astype(f32)
    w8 = w64.astype(f8)
    rw8 = (16.0 * (w64 - w8.astype(f32))).astype(f8)
    # interleave [k, p, g, u, f] pairs -> [g, p, u, k, pair, f]
    wlmr = np.ascontiguousarray(
        np.stack([w8.reshape(NE, P, NVG, VGT, P), rw8.reshape(NE, P, NVG, VGT, P)],
                 axis=4).transpose(2, 1, 3, 0, 4, 5))
    return {
        "wqr": wqr, "wkr": wkr, "wvf": wvf, "wor": wor,
        "w1r": w1r, "w2r": w2r, "wlmr": wlmr,
        "emb": np.ascontiguousarray(inputs["emb"]).astype(bf),
        "ln1g": _sw(inputs["ln1_g"]), "ln1b": _sw(inputs["ln1_b"]),
        "ln2g": _sw(inputs["ln2_g"]), "ln2b": _sw(inputs["ln2_b"]),
        "bo": _sw(inputs["bo"]), "b1": _sw(inputs["b1"]), "b2": _sw(inputs["b2"]),
        "lnfg": _sw(inputs["lnf_g"]), "lnfb": _sw(inputs["lnf_b"]),
        "blm": _sw(inputs["blm"]),
    }


def kernel(**inputs):
    if "nc" not in _CACHED:
        _CACHED["nc"] = build_program()
    nc = _CACHED["nc"]

    shared = _prep_weights(inputs)
    index = np.asarray(inputs["index"])

    # per-core causal masks and token ids
    jpos = np.arange(S)
    ipos = np.arange(P)
    in_maps = []
    for c in range(NC):
        b, j = c // CH, c % CH
        q0 = j * S
        m = np.zeros((NSB, P, S), np.float32)
        for sb in range(NSB):
            m[sb] = ((sb * P + ipos)[:, None] <= (q0 + jpos)[None, :])
        im = dict(shared)
        im["maskp"] = m.astype(ml_dtypes.bfloat16)
        im["idx"] = np.ascontiguousarray(index[b, q0:q0 + S]).astype(np.int32)
        in_maps.append(im)

    res = bass_utils.run_bass_kernel_spmd(nc, in_maps, core_ids=list(range(NC)))
    out = np.zeros((B, T, V), np.float32)
    for c in range(NC):
        b, j = c // CH, c % CH
        out[b, j * S:(j + 1) * S, :] = res.results[c]["logt"].astype(np.float32).T
    return out


# revision 64
# speedup vs baseline: 1.4692x; 1.0051x over previous
"""Trainium2 Bass kernel for a 3-block GPT (B=2,T=2048,E=1024,H=16,V=32000).

Sharding: sequence-parallel over 8 cores (512 tokens each, weights replicated).
Per layer: QKV computed locally; K^T and V are shipped through four
pair-grouped fp8 AllGathers (per-batch groups [[0-3],[4-7]]; k and v of the
first pair-group travel separately) so attention on early head pairs overlaps
the remaining transfers. K/V are scaled x64 into fp8e4 for transport and
descaled to bf16 on arrival. Attention runs scores pair-major with exp+mask
at two-key-block granularity ([128,1024] psum tiles); AV is per-head with the
softmax denominator fused as a ones column in the v slab, then a one-row
reciprocal is broadcast back over 64 partitions with a tiny matmul. The
output projection contracts per-head [64,128] weight slices, and LayerNorm
statistics (ones-matmul partition reductions) are accumulated inside the
producing loops so only the finalize tail is exposed. lm_head uses
error-compensated fp8 DoubleRow matmuls: W ships as interleaved fp8 pairs
(W8 = fp8(64W), Rw8 = fp8(16(64W - W8))) and x as fp8 x8 plus residual
r8 = fp8(x - x8); 12 half-rate DR matmuls per vocab tile compute
x8.W8 + x8.dW + r8.W8, cancelling quantization error to ~1e-3 while
halving PE time vs bf16 at identical DMA bytes. Logits emit as bf16
logits^T [V, 512] per core; the host reassembles and upcasts. All streamed
weights are host-swizzled so each DMA element is >=2KB-contiguous per
partition.
"""

import numpy as np
import ml_dtypes
from contextlib import ExitStack

import concourse.bass as bass
import concourse.mybir as mybir
import concourse.tile as tile
from concourse import bacc
from concourse.masks import make_identity
from concourse import bass_utils

# model dims (hardcoded; harness contract)
B, T, E, H, V = 2, 2048, 1024, 16, 32000
HD, L = 64, 3
NC = 8
S = (B * T) // NC        # 512 tokens per core
CH = NC // B             # 4 chunks (cores) per batch
FF = 4 * E               # 4096
EPS = 1e-5
P = 128
NE = E // P              # 8 e-tiles
NHP = H // 2             # 8 head pairs
NSB = T // P             # 16 key blocks per batch
NVT = V // P             # 250 vocab tiles
NVG = 50                 # vocab groups (5 tiles each)
VGT = NVT // NVG         # 5
NFT = FF // P            # 32 f-tiles
CS = 2 * E * S           # flat kv elems per core (k: E*S, v: S*E)
VOFF = E * S
# pair-grouped kv transport: 3 collectives so attention can start early.
# group g covers head-pairs PGRP[g]; k rows + v cols for those pairs.
PGRP = [(0, 4), (4, 7), (7, 8)]          # [start_pair, end_pair)
GW = [(e - s) * P for s, e in PGRP]      # widths: 512, 384, 128
GKSZ = [w * S for w in GW]               # k elems per group
GSZ = [2 * w * S for w in GW]            # total elems per group (k + v)
F32 = mybir.dt.float32
BF16 = mybir.dt.bfloat16
FP8 = mybir.dt.float8e4
I32 = mybir.dt.int32
KVS = 64.0               # k/v fp8 transport scale (values ~0.013 at layer 1)
AF = mybir.ActivationFunctionType
OP = mybir.AluOpType


def _ln_start(pools):
    """Allocate psum accumulators for LN stats (sum, sum-of-squares)."""
    pacc = pools["pacc"]
    psm = pacc.tile([P, S], F32, tag="ps_b", name="ln_ps", bufs=2)
    pss = pacc.tile([P, S], F32, tag="ps_b", name="ln_ps", bufs=2)
    return psm, pss


def _ln_feed(nc, pools, psm, pss, src_e, e, ones_f):
    """Accumulate LN stats for e-tile `e` (call as each tile is produced)."""
    tp = pools["tp"]
    nc.tensor.matmul(psm[:], lhsT=ones_f[:], rhs=src_e[:],
                     start=(e == 0), stop=(e == NE - 1), skip_group_check=True)
    sq = tp.tile([P, S], F32, tag="ln_sq", name="ln_sq", bufs=2)
    nc.scalar.square(sq[:], src_e[:])
    nc.tensor.matmul(pss[:], lhsT=ones_f[:], rhs=sq[:],
                     start=(e == 0), stop=(e == NE - 1), skip_group_check=True)


def _ln_finish(nc, pools, psm, pss, src, out, g_t, b_t, eps_t):
    """Finalize LN: mean/var/rstd then per-tile normalize."""
    tp = pools["tp"]
    mean = tp.tile([P, S], F32, tag="ln_mean", name="ln_mean")
    nc.scalar.mul(mean[:], psm[:], 1.0 / E)
    msq = tp.tile([P, S], F32, tag="ln_msq", name="ln_msq")
    nc.scalar.square(msq[:], mean[:])
    var = tp.tile([P, S], F32, tag="ln_var", name="ln_var")
    nc.vector.tensor_scalar(var[:], pss[:], 1.0 / E, None, OP.mult)
    nc.vector.tensor_tensor(out=var[:], in0=var[:], in1=msq[:], op=OP.subtract)
    std = tp.tile([P, S], F32, tag="ln_std", name="ln_std")
    nc.scalar.activation(std[:], var[:], AF.Sqrt, bias=eps_t[:])
    rstd = tp.tile([P, S], F32, tag="ln_rstd", name="ln_rstd")
    nc.vector.reciprocal(rstd[:], std[:])
    for e in range(NE):
        t = tp.tile([P, S], F32, tag="ln_t", name="ln_t", bufs=2)
        nc.vector.tensor_tensor(out=t[:], in0=src[e][:], in1=mean[:], op=OP.subtract)
        nc.vector.tensor_tensor(out=t[:], in0=t[:], in1=rstd[:], op=OP.mult)
        nc.scalar.activation(out[e][:], t[:], AF.Identity,
                             bias=b_t[:, e:e + 1], scale=g_t[:, e:e + 1])


def _ln_tiles(nc, tc, pools, src, out, g_t, b_t, ones_f, eps_t):
    """LayerNorm over E (partition axis across the 8 [128,S] tiles of src)."""
    psm, pss = _ln_start(pools)
    for e in range(NE):
        _ln_feed(nc, pools, psm, pss, src[e], e, ones_f)
    _ln_finish(nc, pools, psm, pss, src, out, g_t, b_t, eps_t)


def build_program():
    nc = bacc.Bacc("TRN2", target_bir_lowering=False, debug=False, num_devices=NC)

    # ---- DRAM I/O (all streamed weights pre-swizzled on host so each tile is
    # contiguous per partition) ----
    d_wqr = nc.dram_tensor("wqr", [L, NE, P, NE, P], BF16, kind="ExternalInput")
    d_wkr = nc.dram_tensor("wkr", [L, NE, P, NE, P], BF16, kind="ExternalInput")
    d_wvf = nc.dram_tensor("wvf", [L, NE, P, H * HD], BF16, kind="ExternalInput")
    d_wor = nc.dram_tensor("wor", [L, NE, HD, H, P], BF16, kind="ExternalInput")
    d_w1r = nc.dram_tensor("w1r", [L, NE, P, 4, NE, 2, P], FP8,
                           kind="ExternalInput")
    d_w2r = nc.dram_tensor("w2r", [L, NE, P, NFT, P], BF16, kind="ExternalInput")
    d_wlm = nc.dram_tensor("wlmr", [NVG, P, VGT, NE, 2, P], FP8,
                           kind="ExternalInput")
    d_emb = nc.dram_tensor("emb", [V, E], BF16, kind="ExternalInput")
    d_idx = nc.dram_tensor("idx", [S], I32, kind="ExternalInput")
    d_msk = nc.dram_tensor("maskp", [NSB, P, S], BF16, kind="ExternalInput")
    d_ln1g = nc.dram_tensor("ln1g", [L, P, NE], F32, kind="ExternalInput")
    d_ln1b = nc.dram_tensor("ln1b", [L, P, NE], F32, kind="ExternalInput")
    d_ln2g = nc.dram_tensor("ln2g", [L, P, NE], F32, kind="ExternalInput")
    d_ln2b = nc.dram_tensor("ln2b", [L, P, NE], F32, kind="ExternalInput")
    d_bo = nc.dram_tensor("bo", [L, P, NE], F32, kind="ExternalInput")
    d_b1 = nc.dram_tensor("b1", [L, P, NFT], F32, kind="ExternalInput")
    d_b2 = nc.dram_tensor("b2", [L, P, NE], F32, kind="ExternalInput")
    d_lnfg = nc.dram_tensor("lnfg", [P, NE], F32, kind="ExternalInput")
    d_lnfb = nc.dram_tensor("lnfb", [P, NE], F32, kind="ExternalInput")
    d_blm = nc.dram_tensor("blm", [P, NVT], F32, kind="ExternalInput")
    d_out = nc.dram_tensor("logt", [V, S], BF16, kind="ExternalOutput")

    groups = [[0, 1, 2, 3], [4, 5, 6, 7]]

    with ExitStack() as ctx:
        tc = ctx.enter_context(tile.TileContext(nc, num_cores=NC))
        const = ctx.enter_context(tc.tile_pool(name="const", bufs=1))
        pp_x = ctx.enter_context(tc.tile_pool(name="xres", bufs=1))
        pp_sum = ctx.enter_context(tc.tile_pool(name="xsum", bufs=1))
        pp_msk = ctx.enter_context(tc.tile_pool(name="masks", bufs=1))
        tp = ctx.enter_context(tc.tile_pool(name="tp", bufs=1))
        pacc = ctx.enter_context(tc.tile_pool(name="pacc", bufs=1, space="PSUM"))
        dram = ctx.enter_context(tc.tile_pool(name="ccdram", bufs=2, space="DRAM"))
        pools = {"pacc": pacc, "tp": tp}

        # ---- embedding gather first: critical path at startup ----
        idx_t = const.tile([P, S // P], I32, name="idx_t")
        nc.gpsimd.dma_start(out=idx_t[:], in_=d_idx.ap().rearrange("(g p) -> p g", p=P))

        # queue all four gathers on the Pool engine before anything else
        xgs = []
        for g in range(S // P):
            xg = tp.tile([P, E], BF16, tag="embg", name="embg", bufs=4)
            nc.gpsimd.indirect_dma_start(
                out=xg[:], out_offset=None, in_=d_emb.ap(),
                in_offset=bass.IndirectOffsetOnAxis(ap=idx_t[:, g:g + 1], axis=0))
            xgs.append(xg)

        ident = const.tile([P, P], F32, name="ident")
        make_identity(nc, ident[:])
        identb = const.tile([P, P], BF16, name="identb")
        nc.vector.tensor_copy(identb[:], ident[:])
        ones_f = const.tile([P, P], F32, name="ones_f")
        nc.vector.memset(ones_f[:], 1.0)
        ones_bc = const.tile([P, HD], F32, name="ones_bc")
        nc.vector.memset(ones_bc[:], 1.0)
        eps_t = const.tile([P, 1], F32, name="eps_t")
        nc.vector.memset(eps_t[:], EPS)

        # residual stream xT: 8 tiles [128, S]
        xT = [pp_x.tile([P, S], F32, tag=f"x{e}", name=f"x{e}") for e in range(NE)]
        sum_t = [pp_sum.tile([P, S], F32, tag=f"s{e}", name=f"s{e}") for e in range(NE)]

        for g in range(S // P):
            for e in range(NE):
                pst = pacc.tile([P, S], BF16, tag="ps_a", name="tpose", bufs=3)
                nc.tensor.transpose(pst[:, 0:P], xgs[g][:, e * P:(e + 1) * P],
                                    identb[:])
                nc.vector.tensor_copy(xT[e][:, g * P:(g + 1) * P], pst[:, 0:P])

        # ---- params -> [128, n] tiles (after the gather chain is queued) ----
        def ldvec(dt_ap, n, name):
            t = const.tile([P, n], F32, tag=name, name=name)
            nc.gpsimd.dma_start(out=t[:], in_=dt_ap)
            return t

        t_ln1g = [ldvec(d_ln1g.ap()[l], NE, f"ln1g{l}") for l in range(L)]
        t_ln1b = [ldvec(d_ln1b.ap()[l], NE, f"ln1b{l}") for l in range(L)]
        t_ln2g = [ldvec(d_ln2g.ap()[l], NE, f"ln2g{l}") for l in range(L)]
        t_ln2b = [ldvec(d_ln2b.ap()[l], NE, f"ln2b{l}") for l in range(L)]
        t_bo = [ldvec(d_bo.ap()[l], NE, f"bo{l}") for l in range(L)]
        t_b1 = [ldvec(d_b1.ap()[l], NFT, f"b1{l}") for l in range(L)]
        t_b2 = [ldvec(d_b2.ap()[l], NE, f"b2{l}") for l in range(L)]
        t_lnfg = ldvec(d_lnfg.ap(), NE, "lnfg")
        t_lnfb = ldvec(d_lnfb.ap(), NE, "lnfb")
        t_blm = ldvec(d_blm.ap(), NVT, "blm")

        # causal masks (bf16 multiplicative, per-core data): one DMA
        mask3 = pp_msk.tile([P, NSB * S], BF16, tag="msk", name="msk")
        nc.gpsimd.dma_start(out=mask3[:].rearrange("p (sb s) -> p sb s", sb=NSB),
                    in_=d_msk.ap().rearrange("sb p s -> p sb s"))

        # ---- transformer blocks ----
        for l in range(L):
            with tc.tile_pool(name="qT", bufs=1) as pp_q, \
                 tc.tile_pool(name="oT", bufs=1) as pp_o, \
                 tc.tile_pool(name="attn", bufs=1) as ap_t, \
                 tc.tile_pool(name="vslab", bufs=1) as vp:

                # bf16 activations
                xA = []
                for e in range(NE):
                    xa = ap_t.tile([P, S], BF16, tag=f"xa{e}", name=f"xa{e}")
                    if e % 2 == 0:
                        nc.vector.tensor_copy(xa[:], xT[e][:])
                    else:
                        nc.scalar.copy(xa[:], xT[e][:])
                    xA.append(xa)

                # k/v projections feed pair-grouped transport buffers so the
                # AllGathers launch incrementally and overlap attention.
                # Group 0 ships k and v separately: scores for pairs 0-3 can
                # start as soon as their k rows arrive.
                kvcat = [dram.tile([GKSZ[0]], FP8, tag="kvcat0", name="kvcat0"),
                         dram.tile([GSZ[1]], FP8, tag="kvcat1", name="kvcat1"),
                         dram.tile([GSZ[2]], FP8, tag="kvcat2", name="kvcat2")]
                kvout = [dram.tile([CH * GKSZ[0]], FP8, tag="kvout0", name="kvout0"),
                         dram.tile([CH * GSZ[1]], FP8, tag="kvout1", name="kvout1"),
                         dram.tile([CH * GSZ[2]], FP8, tag="kvout2", name="kvout2")]
                c0v = dram.tile([S * GW[0]], FP8, tag="c0v", name="c0v")
                o0v = dram.tile([CH * S * GW[0]], FP8, tag="o0v", name="o0v")

                def kproj(m, g, lm):
                    w = ap_t.tile([P, NE, P], BF16, tag="wb", name="wb", bufs=3)
                    nc.scalar.dma_start(out=w[:], in_=d_wkr.ap()[l, m])
                    ps = pacc.tile([P, S], F32, tag="ps_a", name="acc", bufs=3)
                    for k in range(NE):
                        nc.tensor.matmul(ps[:], lhsT=w[:, k, :], rhs=xA[k][:],
                                         start=(k == 0), stop=(k == NE - 1))
                    kl = tp.tile([P, S], FP8, tag="klocal", name="klocal", bufs=2)
                    nc.scalar.mul(kl[:], ps[:], KVS)
                    nc.sync.dma_start(
                        out=kvcat[g][lm * P * S:(lm + 1) * P * S].rearrange(
                            "(p s) -> p s", p=P),
                        in_=kl[:])

                def vproj(n):
                    # psv halves cover tokens tb*128.., head cols n*512..n*512+512
                    ps2 = [pacc.tile([P, 2, 512], F32, tag="ps_a", name="acc", bufs=3)
                           for _ in range(2)]
                    psv = [ps2[0][:, 0, :], ps2[0][:, 1, :],
                           ps2[1][:, 0, :], ps2[1][:, 1, :]]
                    for k in range(NE):
                        wv = ap_t.tile([P, 512], BF16, tag="wv", name="wv", bufs=3)
                        nc.scalar.dma_start(
                            out=wv[:],
                            in_=d_wvf.ap()[l, k, :, n * 512:(n + 1) * 512])
                        for m in range(4):
                            nc.tensor.matmul(psv[m], lhsT=xA[k][:, m * P:(m + 1) * P],
                                             rhs=wv[:], start=(k == 0), stop=(k == NE - 1))
                    for tb in range(4):
                        vv = tp.tile([P, 512], FP8, tag="vlocal", name="vlocal", bufs=2)
                        nc.scalar.mul(vv[:], psv[tb], KVS)
                        if n == 0:  # all 512 cols belong to group 0
                            nc.sync.dma_start(
                                out=c0v[tb * P * GW[0]:(tb + 1) * P * GW[0]]
                                .rearrange("(p f) -> p f", p=P),
                                in_=vv[:])
                        else:  # cols 0-383 -> group 1, cols 384-511 -> group 2
                            nc.sync.dma_start(
                                out=kvcat[1][GKSZ[1] + tb * P * GW[1]:
                                             GKSZ[1] + (tb + 1) * P * GW[1]]
                                .rearrange("(p f) -> p f", p=P),
                                in_=vv[:, 0:GW[1]])
                            nc.sync.dma_start(
                                out=kvcat[2][GKSZ[2] + tb * P * GW[2]:
                                             GKSZ[2] + (tb + 1) * P * GW[2]]
                                .rearrange("(p f) -> p f", p=P),
                                in_=vv[:, GW[1]:512])

                for m in range(4):
                    kproj(m, 0, m)
                nc.gpsimd.collective_compute(
                    "AllGather", OP.bypass, replica_groups=groups,
                    ins=[kvcat[0][:]], outs=[kvout[0][:]])
                vproj(0)
                nc.gpsimd.collective_compute(
                    "AllGather", OP.bypass, replica_groups=groups,
                    ins=[c0v[:]], outs=[o0v[:]])
                for m in range(4, 7):
                    kproj(m, 1, m - 4)
                kproj(7, 2, 0)
                vproj(1)
                nc.gpsimd.collective_compute(
                    "AllGather", OP.bypass, replica_groups=groups,
                    ins=[kvcat[1][:]], outs=[kvout[1][:]])
                nc.gpsimd.collective_compute(
                    "AllGather", OP.bypass, replica_groups=groups,
                    ins=[kvcat[2][:]], outs=[kvout[2][:]])

                qT = []
                for m in range(NE):
                    w = ap_t.tile([P, NE, P], BF16, tag="wb", name="wb", bufs=3)
                    nc.scalar.dma_start(out=w[:], in_=d_wqr.ap()[l, m])
                    ps = pacc.tile([P, S], F32, tag="ps_a", name="acc", bufs=3)
                    for k in range(NE):
                        nc.tensor.matmul(ps[:], lhsT=w[:, k, :], rhs=xA[k][:],
                                         start=(k == 0), stop=(k == NE - 1))
                    q = pp_q.tile([P, S], BF16, tag=f"q{m}", name=f"q{m}")
                    nc.scalar.copy(q[:], ps[:])
                    qT.append(q)

                # attention: scores pair-major, AV per head with the softmax
                # denominator fused as a ones column in the v slab
                oT = []
                for p in range(NHP):
                    g = 0 if p < 4 else (1 if p < 7 else 2)
                    lm = p - PGRP[g][0]
                    pav_e = pacc.tile([65, S], F32, tag="ps_b", name="pave", bufs=2)
                    pav_o = pacc.tile([65, S], F32, tag="ps_b", name="pavo", bufs=2)
                    vsl8 = vp.tile([P, CH, 4, P], FP8, tag="vsl8", name="vsl8", bufs=2)
                    for cc in range(CH):
                        if g == 0:
                            vsrc, vbase = o0v, cc * S * GW[0]
                        else:
                            vsrc, vbase = kvout[g], cc * GSZ[g] + GKSZ[g]
                        nc.sync.dma_start(
                            out=vsl8[:, cc, :, :],
                            in_=vsrc[vbase:vbase + S * GW[g]].rearrange(
                                "(tb q f) -> q tb f", tb=4, q=P)[
                                :, :, lm * P:(lm + 1) * P])
                    # [v_even(64) | 1 | v_odd(64) | 1] per key block
                    vslab = vp.tile([P, CH, 4, 130], BF16, tag="vsl", name="vsl",
                                    bufs=1)
                    nc.vector.tensor_scalar(vslab[:, :, :, 0:HD],
                                            vsl8[:, :, :, 0:HD],
                                            1.0 / KVS, None, OP.mult)
                    nc.vector.tensor_scalar(vslab[:, :, :, 65:65 + HD],
                                            vsl8[:, :, :, HD:P],
                                            1.0 / KVS, None, OP.mult)
                    nc.vector.memset(vslab[:, :, :, HD:HD + 1], 1.0)
                    nc.vector.memset(vslab[:, :, :, 129:130], 1.0)
                    for cc in range(CH):
                        kbase = (cc * GKSZ[0] if g == 0 else cc * GSZ[g]) + lm * P * S
                        ksl8 = ap_t.tile([P, S], FP8, tag="ksl8", name="ksl8", bufs=3)
                        nc.sync.dma_start(
                            out=ksl8[:],
                            in_=kvout[g][kbase:kbase + P * S]
                            .rearrange("(p s) -> p s", p=P))
                        ksl = ap_t.tile([P, S], BF16, tag="kslab", name="kslab", bufs=3)
                        nc.vector.tensor_scalar(ksl[:], ksl8[:], 1.0 / KVS, None, OP.mult)
                        for jh in range(2):
                            sb0 = 4 * cc + 2 * jh
                            sA = pacc.tile([P, 2, S], F32, tag="ps_a", name="sc",
                                           bufs=3)
                            sB = pacc.tile([P, 2, S], F32, tag="ps_a", name="sc",
                                           bufs=3)
                            for d in range(2):
                                j2 = 2 * jh + d
                                nc.tensor.matmul(
                                    sA[:, d, :], lhsT=ksl[0:HD, j2 * P:(j2 + 1) * P],
                                    rhs=qT[p][0:HD, :], tile_position=(0, 0))
                                nc.tensor.matmul(
                                    sB[:, d, :], lhsT=ksl[HD:P, j2 * P:(j2 + 1) * P],
                                    rhs=qT[p][HD:P, :], tile_position=(64, 0))
                            pa = ap_t.tile([P, 2 * S], BF16, tag="pt", name="pt", bufs=4)
                            pb = ap_t.tile([P, 2 * S], BF16, tag="pt", name="pt", bufs=4)
                            nc.scalar.activation(pa[:], sA[:], AF.Exp, scale=HD ** -0.5)
                            nc.scalar.activation(pb[:], sB[:], AF.Exp, scale=HD ** -0.5)
                            nc.vector.tensor_tensor(out=pa[:], in0=pa[:],
                                                    in1=mask3[:, sb0 * S:(sb0 + 2) * S],
                                                    op=OP.mult)
                            nc.vector.tensor_tensor(out=pb[:], in0=pb[:],
                                                    in1=mask3[:, sb0 * S:(sb0 + 2) * S],
                                                    op=OP.mult)
                            for d in range(2):
                                sb = sb0 + d
                                first, last = (sb == 0), (sb == NSB - 1)
                                j2 = 2 * jh + d
                                nc.tensor.matmul(
                                    pav_e[:], lhsT=vslab[:, cc, j2, 0:65],
                                    rhs=pa[:, d * S:(d + 1) * S], start=first,
                                    stop=last, skip_group_check=True)
                                nc.tensor.matmul(
                                    pav_o[:], lhsT=vslab[:, cc, j2, 65:130],
                                    rhs=pb[:, d * S:(d + 1) * S], start=first,
                                    stop=last, skip_group_check=True)
                    for hh, pav in ((0, pav_e), (1, pav_o)):
                        dd = tp.tile([65, S], F32, tag="dd", name="dd", bufs=2)
                        nc.vector.reciprocal(dd[64:65, :], pav[64:65, :])
                        rec_ps = pacc.tile([HD, S], F32, tag="ps_a", name="recp",
                                           bufs=3)
                        nc.tensor.matmul(rec_ps[:], lhsT=ones_bc[64:65, :],
                                         rhs=dd[64:65, :], tile_position=(64, 0))
                        rec_s = tp.tile([HD, S], F32, tag="rec_s", name="rec_s",
                                        bufs=2)
                        nc.vector.tensor_copy(rec_s[:], rec_ps[:])
                        o = pp_o.tile([HD, S], BF16, tag=f"o{2 * p + hh}",
                                      name=f"o{2 * p + hh}")
                        nc.vector.tensor_tensor(out=o[:], in0=pav[0:HD, :],
                                                in1=rec_s[:], op=OP.mult)
                        oT.append(o)

                # output projection + bias + residual (LN stats fed per tile)
                psm1, pss1 = _ln_start(pools)
                for m in range(NE):
                    w = ap_t.tile([HD, H, P], BF16, tag="wo", name="wo", bufs=3)
                    nc.scalar.dma_start(out=w[:], in_=d_wor.ap()[l, m])
                    ps = pacc.tile([P, S], F32, tag="ps_a", name="acc", bufs=3)
                    for h in range(H):
                        nc.tensor.matmul(ps[:], lhsT=w[:, h, :], rhs=oT[h][:],
                                         start=(h == 0), stop=(h == H - 1))
                    yv = tp.tile([P, S], F32, tag="yv", name="yv", bufs=2)
                    nc.scalar.add(yv[:], ps[:], t_bo[l][:, m:m + 1])
                    nc.vector.tensor_tensor(out=sum_t[m][:], in0=yv[:], in1=xT[m][:],
                                            op=OP.add)
                    _ln_feed(nc, pools, psm1, pss1, sum_t[m], m, ones_f)

            _ln_finish(nc, pools, psm1, pss1, sum_t, xT,
                       t_ln1g[l], t_ln1b[l], eps_t)

            # FFN (bf16 matmuls, fp32 psum + residual)
            with tc.tile_pool(name="ht", bufs=1) as pp_h, \
                 tc.tile_pool(name="xbf", bufs=1) as pp_xbf:
                # compensated-DR inputs for FFN1 (x is post-LN1, unit scale)
                xp1 = [pp_xbf.tile([P, 2, S], FP8, tag=f"xf{k}", name=f"xf{k}")
                       for k in range(NE)]
                rp1 = [pp_xbf.tile([P, 2, S], FP8, tag=f"rf{kp}", name=f"rf{kp}")
                       for kp in range(NE // 2)]
                for e in range(NE):
                    nc.vector.tensor_copy(xp1[e][:, 0, :], xT[e][:])
                    x8d = tp.tile([P, S], F32, tag="x8d", name="x8d", bufs=2)
                    nc.scalar.copy(x8d[:], xp1[e][:, 0, :])
                    rt = tp.tile([P, S], F32, tag="rt", name="rt", bufs=2)
                    nc.vector.tensor_tensor(out=rt[:], in0=xT[e][:], in1=x8d[:],
                                            op=OP.subtract)
                    nc.scalar.copy(rp1[e // 2][:, e % 2, :], rt[:])
                    nc.vector.tensor_scalar(xp1[e][:, 1, :], xp1[e][:, 0, :],
                                            1.0 / 16.0, None, OP.mult)
                hT = []
                for fg in range(NE):
                    w = pp_h.tile([P, 4, NE, 2, P], FP8, tag="w4", name="w4", bufs=3)
                    nc.scalar.dma_start(out=w[:], in_=d_w1r.ap()[l, fg])
                    for fi in range(4):
                        f = fg * 4 + fi
                        ps = pacc.tile([P, S], F32, tag="ps_a", name="acc", bufs=3)
                        for k in range(NE):
                            nc.tensor.matmul(
                                ps[:], lhsT=w[:, fi, k, :, :], rhs=xp1[k][:],
                                start=(k == 0), stop=False,
                                perf_mode=mybir.MatmulPerfMode.DoubleRow)
                        for kp in range(NE // 2):
                            nc.tensor.matmul(
                                ps[:], lhsT=w[:, fi, 2 * kp:2 * kp + 2, 0, :],
                                rhs=rp1[kp][:],
                                start=False, stop=(kp == NE // 2 - 1),
                                perf_mode=mybir.MatmulPerfMode.DoubleRow)
                        h = pp_h.tile([P, S], BF16, tag=f"h{f}", name=f"h{f}")
                        nc.scalar.activation(h[:], ps[:], AF.Relu,
                                             bias=t_b1[l][:, f:f + 1],
                                             scale=1.0 / 64.0)
                        hT.append(h)
                psm2, pss2 = _ln_start(pools)
                for m in range(NE):
                    w = pp_h.tile([P, NFT, P], BF16, tag="w32", name="w32", bufs=2)
                    nc.scalar.dma_start(out=w[:], in_=d_w2r.ap()[l, m])
                    ps = pacc.tile([P, S], F32, tag="ps_a", name="acc", bufs=3)
                    for f in range(NFT):
                        nc.tensor.matmul(ps[:], lhsT=w[:, f, :], rhs=hT[f][:],
                                         start=(f == 0), stop=(f == NFT - 1))
                    yv = tp.tile([P, S], F32, tag="yv", name="yv", bufs=2)
                    nc.scalar.add(yv[:], ps[:], t_b2[l][:, m:m + 1])
                    nc.vector.tensor_tensor(out=sum_t[m][:], in0=yv[:], in1=xT[m][:],
                                            op=OP.add)
                    _ln_feed(nc, pools, psm2, pss2, sum_t[m], m, ones_f)
            _ln_finish(nc, pools, psm2, pss2, sum_t, xT,
                       t_ln2g[l], t_ln2b[l], eps_t)

        # ---- final LN + lm_head ----
        # Error-compensated DoubleRow: W is shipped as interleaved fp8 pairs
        # (W8 = fp8(64W), Rw8 = fp8(16*(64W - W8))); x as fp8 x8 plus residual
        # r8 = fp8(x - x8). logits = [x8.(W8 + Rw8/16) + r8.W8]/64 via 12
        # half-rate DR matmuls per vocab tile (vs 8 full-rate bf16).
        _ln_tiles(nc, tc, pools, xT, sum_t, t_lnfg, t_lnfb, ones_f, eps_t)
        with tc.tile_pool(name="lg", bufs=1) as pp_lg, \
             tc.tile_pool(name="xb", bufs=1) as pp_xb:
            xpair = [pp_xb.tile([P, 2, S], FP8, tag=f"xp{k}", name=f"xp{k}")
                     for k in range(NE)]
            rpair = [pp_xb.tile([P, 2, S], FP8, tag=f"rp{kp}", name=f"rp{kp}")
                     for kp in range(NE // 2)]
            for e in range(NE):
                nc.vector.tensor_copy(xpair[e][:, 0, :], sum_t[e][:])
                x8d = tp.tile([P, S], F32, tag="x8d", name="x8d", bufs=2)
                nc.scalar.copy(x8d[:], xpair[e][:, 0, :])
                rt = tp.tile([P, S], F32, tag="rt", name="rt", bufs=2)
                nc.vector.tensor_tensor(out=rt[:], in0=sum_t[e][:], in1=x8d[:],
                                        op=OP.subtract)
                nc.vector.tensor_copy(rpair[e // 2][:, e % 2, :], rt[:])
                nc.vector.tensor_scalar(xpair[e][:, 1, :], xpair[e][:, 0, :],
                                        1.0 / 16.0, None, OP.mult)
            for vg in range(NVG):
                w = pp_lg.tile([P, VGT, NE, 2, P], FP8, tag="wlmt", name="wlmt",
                               bufs=3)
                nc.scalar.dma_start(out=w[:], in_=d_wlm.ap()[vg])
                lgc = pp_lg.tile([P, VGT, S], BF16, tag="lg", name="lg", bufs=3)
                for u in range(VGT):
                    vt = vg * VGT + u
                    ps = pacc.tile([P, S], F32, tag="ps_a", name="acc", bufs=3)
                    for k in range(NE):
                        nc.tensor.matmul(
                            ps[:], lhsT=w[:, u, k, :, :], rhs=xpair[k][:],
                            start=(k == 0), stop=False,
                            perf_mode=mybir.MatmulPerfMode.DoubleRow)
                    for kp in range(NE // 2):
                        nc.tensor.matmul(
                            ps[:], lhsT=w[:, u, 2 * kp:2 * kp + 2, 0, :],
                            rhs=rpair[kp][:],
                            start=False, stop=(kp == NE // 2 - 1),
                            perf_mode=mybir.MatmulPerfMode.DoubleRow)
                    nc.scalar.activation(lgc[:, u, :], ps[:], AF.Identity,
                                         bias=t_blm[:, vt:vt + 1], scale=1.0 / 64.0)
                nc.sync.dma_start(
                    out=d_out.ap()[vg * VGT * P:(vg + 1) * VGT * P, :]
                    .rearrange("(u p) s -> p u s", p=P),
                    in_=lgc[:])

    nc.compile()
    return nc


_CACHED = {}


def _sw(v):
    """[.., n*P] vector -> [.., P, n] partition-major fp32."""
    v = np.asarray(v, np.float32)
    return np.ascontiguousarray(
        v.reshape(*v.shape[:-1], v.shape[-1] // P, P).swapaxes(-1, -2))


def _prep_weights(inputs):
    f32 = np.float32
    bf = ml_dtypes.bfloat16
    Wq, Wk, Wv = inputs["Wq"], inputs["Wk"], inputs["Wv"]
    # [L,H,E,HD] -> [L,E,H*HD]
    wq_flat = np.ascontiguousarray(Wq.transpose(0, 2, 1, 3).reshape(L, E, H * HD))
    wk_flat = np.ascontiguousarray(Wk.transpose(0, 2, 1, 3).reshape(L, E, H * HD))
    wv_flat = np.ascontiguousarray(Wv.transpose(0, 2, 1, 3).reshape(L, E, H * HD))

    def sw_ee(w):  # [L, E_in, E_out] -> [L, m, p, k, f]
        return np.ascontiguousarray(
            w.reshape(L, NE, P, NE, P).transpose(0, 3, 2, 1, 4)).astype(bf)

    wqr = sw_ee(wq_flat)
    wkr = sw_ee(wk_flat)
    # Wo in per-head layout [l, m, d, h, f] so each head's 64 contraction rows
    # sit at partitions 0-63
    wor = np.ascontiguousarray(
        inputs["Wo"].reshape(L, H, HD, NE, P).transpose(0, 3, 2, 1, 4)).astype(bf)
    wvf = np.ascontiguousarray(
        wv_flat.reshape(L, NE, P, H * HD)).astype(bf)
    f8 = ml_dtypes.float8_e4m3
    w1_64 = (64.0 * inputs["W1"]).astype(f32)
    w1_8 = w1_64.astype(f8)
    rw1_8 = (16.0 * (w1_64 - w1_8.astype(f32))).astype(f8)
    w1r = np.ascontiguousarray(
        np.stack([w1_8.reshape(L, NE, P, NE, 4, P),
                  rw1_8.reshape(L, NE, P, NE, 4, P)],
                 axis=5).transpose(0, 3, 2, 4, 1, 5, 6))
    w2r = np.ascontiguousarray(
        inputs["W2"].reshape(L, NFT, P, NE, P).transpose(0, 3, 2, 1, 4)).astype(bf)
    f8 = ml_dtypes.float8_e4m3
    w64 = (64.0 * inputs["Wlm"]).astype(f32)
    w8 = w64.astype(f8)
    rw8 = (16.0 * (w64 - w8.astype(f32))).astype(f8)
    # interleave [k, p, g, u, f] pairs -> [g, p, u, k, pair, f]
    wlmr = np.ascontiguousarray(
        np.stack([w8.reshape(NE, P, NVG, VGT, P), rw8.reshape(NE, P, NVG, VGT, P)],
                 axis=4).transpose(2, 1, 3, 0, 4, 5))
    return {
        "wqr": wqr, "wkr": wkr, "wvf": wvf, "wor": wor,
        "w1r": w1r, "w2r": w2r, "wlmr": wlmr,
        "emb": np.ascontiguousarray(inputs["emb"]).astype(bf),
        "ln1g": _sw(inputs["ln1_g"]), "ln1b": _sw(inputs["ln1_b"]),
        "ln2g": _sw(inputs["ln2_g"]), "ln2b": _sw(inputs["ln2_b"]),
        "bo": _sw(inputs["bo"]), "b1": _sw(inputs["b1"]), "b2": _sw(inputs["b2"]),
        "lnfg": _sw(inputs["lnf_g"]), "lnfb": _sw(inputs["lnf_b"]),
        "blm": _sw(inputs["blm"]),
    }


def kernel(**inputs):
    if "nc" not in _CACHED:
        _CACHED["nc"] = build_program()
    nc = _CACHED["nc"]

    shared = _prep_weights(inputs)
    index = np.asarray(inputs["index"])

    # per-core causal masks and token ids
    jpos = np.arange(S)
    ipos = np.arange(P)
    in_maps = []
    for c in range(NC):
        b, j = c // CH, c % CH
        q0 = j * S
        m = np.zeros((NSB, P, S), np.float32)
        for sb in range(NSB):
            m[sb] = ((sb * P + ipos)[:, None] <= (q0 + jpos)[None, :])
        im = dict(shared)
        im["maskp"] = m.astype(ml_dtypes.bfloat16)
        im["idx"] = np.ascontiguousarray(index[b, q0:q0 + S]).astype(np.int32)
        in_maps.append(im)

    res = bass_utils.run_bass_kernel_spmd(nc, in_maps, core_ids=list(range(NC)))
    out = np.zeros((B, T, V), np.float32)
    for c in range(NC):
        b, j = c // CH, c % CH
        out[b, j * S:(j + 1) * S, :] = res.results[c]["logt"].astype(np.float32).T
    return out


# revision 69
# speedup vs baseline: 1.4723x; 1.0021x over previous
"""Trainium2 Bass kernel for a 3-block GPT (B=2,T=2048,E=1024,H=16,V=32000).

Sharding: sequence-parallel over 8 cores (512 tokens each, weights replicated).
Per layer: QKV computed locally; K^T and V are shipped through four
pair-grouped fp8 AllGathers (per-batch groups [[0-3],[4-7]]; k and v of the
first pair-group travel separately) so attention on early head pairs overlaps
the remaining transfers. K/V are scaled x64 into fp8e4 for transport and
descaled to bf16 on arrival. Attention runs scores pair-major with exp+mask
at two-key-block granularity ([128,1024] psum tiles); AV is per-head with the
softmax denominator fused as a ones column in the v slab, then a one-row
reciprocal is broadcast back over 64 partitions with a tiny matmul. The
output projection contracts per-head [64,128] weight slices, and LayerNorm
statistics (ones-matmul partition reductions) are accumulated inside the
producing loops so only the finalize tail is exposed. lm_head uses
error-compensated fp8 DoubleRow matmuls: W ships as interleaved fp8 pairs
(W8 = fp8(64W), Rw8 = fp8(16(64W - W8))) and x as fp8 x8 plus residual
r8 = fp8(x - x8); 12 half-rate DR matmuls per vocab tile compute
x8.W8 + x8.dW + r8.W8, cancelling quantization error to ~1e-3 while
halving PE time vs bf16 at identical DMA bytes (FFN1 uses the same
scheme). Logits emit as bf16
logits^T [V, 512] per core; the host reassembles and upcasts. All streamed
weights are host-swizzled so each DMA element is >=2KB-contiguous per
partition.
"""

import numpy as np
import ml_dtypes
from contextlib import ExitStack

import concourse.bass as bass
import concourse.mybir as mybir
import concourse.tile as tile
from concourse import bacc
from concourse.masks import make_identity
from concourse import bass_utils

# model dims (hardcoded; harness contract)
B, T, E, H, V = 2, 2048, 1024, 16, 32000
HD, L = 64, 3
NC = 8
S = (B * T) // NC        # 512 tokens per core
CH = NC // B             # 4 chunks (cores) per batch
FF = 4 * E               # 4096
EPS = 1e-5
P = 128
NE = E // P              # 8 e-tiles
NHP = H // 2             # 8 head pairs
NSB = T // P             # 16 key blocks per batch
NVT = V // P             # 250 vocab tiles
NVG = 50                 # vocab groups (5 tiles each)
VGT = NVT // NVG         # 5
NFT = FF // P            # 32 f-tiles
CS = 2 * E * S           # flat kv elems per core (k: E*S, v: S*E)
VOFF = E * S
# pair-grouped kv transport: 3 collectives so attention can start early.
# group g covers head-pairs PGRP[g]; k rows + v cols for those pairs.
PGRP = [(0, 4), (4, 7), (7, 8)]          # [start_pair, end_pair)
GW = [(e - s) * P for s, e in PGRP]      # widths: 512, 384, 128
GKSZ = [w * S for w in GW]               # k elems per group
GSZ = [2 * w * S for w in GW]            # total elems per group (k + v)
F32 = mybir.dt.float32
BF16 = mybir.dt.bfloat16
FP8 = mybir.dt.float8e4
I32 = mybir.dt.int32
KVS = 64.0               # k/v fp8 transport scale (values ~0.013 at layer 1)
AF = mybir.ActivationFunctionType
OP = mybir.AluOpType


def _ln_start(pools):
    """Allocate psum accumulators for LN stats (sum, sum-of-squares)."""
    pacc = pools["pacc"]
    psm = pacc.tile([P, S], F32, tag="ps_b", name="ln_ps", bufs=2)
    pss = pacc.tile([P, S], F32, tag="ps_b", name="ln_ps", bufs=2)
    return psm, pss


def _ln_feed(nc, pools, psm, pss, src_e, e, ones_f):
    """Accumulate LN stats for e-tile `e` (call as each tile is produced)."""
    tp = pools["tp"]
    nc.tensor.matmul(psm[:], lhsT=ones_f[:], rhs=src_e[:],
                     start=(e == 0), stop=(e == NE - 1), skip_group_check=True)
    sq = tp.tile([P, S], F32, tag="ln_sq", name="ln_sq", bufs=2)
    nc.scalar.square(sq[:], src_e[:])
    nc.tensor.matmul(pss[:], lhsT=ones_f[:], rhs=sq[:],
                     start=(e == 0), stop=(e == NE - 1), skip_group_check=True)


def _ln_finish(nc, pools, psm, pss, src, out, g_t, b_t, eps_t):
    """Finalize LN: mean/var/rstd then per-tile normalize."""
    tp = pools["tp"]
    mean = tp.tile([P, S], F32, tag="ln_mean", name="ln_mean")
    nc.scalar.mul(mean[:], psm[:], 1.0 / E)
    msq = tp.tile([P, S], F32, tag="ln_msq", name="ln_msq")
    nc.scalar.square(msq[:], mean[:])
    var = tp.tile([P, S], F32, tag="ln_var", name="ln_var")
    nc.vector.tensor_scalar(var[:], pss[:], 1.0 / E, None, OP.mult)
    nc.vector.tensor_tensor(out=var[:], in0=var[:], in1=msq[:], op=OP.subtract)
    std = tp.tile([P, S], F32, tag="ln_std", name="ln_std")
    nc.scalar.activation(std[:], var[:], AF.Sqrt, bias=eps_t[:])
    rstd = tp.tile([P, S], F32, tag="ln_rstd", name="ln_rstd")
    nc.vector.reciprocal(rstd[:], std[:])
    for e in range(NE):
        t = tp.tile([P, S], F32, tag="ln_t", name="ln_t", bufs=2)
        nc.vector.tensor_tensor(out=t[:], in0=src[e][:], in1=mean[:], op=OP.subtract)
        nc.vector.tensor_tensor(out=t[:], in0=t[:], in1=rstd[:], op=OP.mult)
        nc.scalar.activation(out[e][:], t[:], AF.Identity,
                             bias=b_t[:, e:e + 1], scale=g_t[:, e:e + 1])


def _ln_tiles(nc, tc, pools, src, out, g_t, b_t, ones_f, eps_t):
    """LayerNorm over E (partition axis across the 8 [128,S] tiles of src)."""
    psm, pss = _ln_start(pools)
    for e in range(NE):
        _ln_feed(nc, pools, psm, pss, src[e], e, ones_f)
    _ln_finish(nc, pools, psm, pss, src, out, g_t, b_t, eps_t)


def build_program():
    nc = bacc.Bacc("TRN2", target_bir_lowering=False, debug=False, num_devices=NC)

    # ---- DRAM I/O (all streamed weights pre-swizzled on host so each tile is
    # contiguous per partition) ----
    d_wqr = nc.dram_tensor("wqr", [L, NE, P, NE, P], BF16, kind="ExternalInput")
    d_wkr = nc.dram_tensor("wkr", [L, NE, P, NE, P], BF16, kind="ExternalInput")
    d_wvf = nc.dram_tensor("wvf", [L, NE, P, H * HD], BF16, kind="ExternalInput")
    d_wor = nc.dram_tensor("wor", [L, NE, HD, H, P], BF16, kind="ExternalInput")
    d_w1r = nc.dram_tensor("w1r", [L, NE, P, 4, NE, 2, P], FP8,
                           kind="ExternalInput")
    d_w2r = nc.dram_tensor("w2r", [L, NE, P, NFT, P], BF16, kind="ExternalInput")
    d_wlm = nc.dram_tensor("wlmr", [NVG, P, VGT, NE, 2, P], FP8,
                           kind="ExternalInput")
    d_emb = nc.dram_tensor("emb", [V, E], BF16, kind="ExternalInput")
    d_idx = nc.dram_tensor("idx", [S], I32, kind="ExternalInput")
    d_msk = nc.dram_tensor("maskp", [NSB, P, S], BF16, kind="ExternalInput")
    d_ln1g = nc.dram_tensor("ln1g", [L, P, NE], F32, kind="ExternalInput")
    d_ln1b = nc.dram_tensor("ln1b", [L, P, NE], F32, kind="ExternalInput")
    d_ln2g = nc.dram_tensor("ln2g", [L, P, NE], F32, kind="ExternalInput")
    d_ln2b = nc.dram_tensor("ln2b", [L, P, NE], F32, kind="ExternalInput")
    d_bo = nc.dram_tensor("bo", [L, P, NE], F32, kind="ExternalInput")
    d_b1 = nc.dram_tensor("b1", [L, P, NFT], F32, kind="ExternalInput")
    d_b2 = nc.dram_tensor("b2", [L, P, NE], F32, kind="ExternalInput")
    d_lnfg = nc.dram_tensor("lnfg", [P, NE], F32, kind="ExternalInput")
    d_lnfb = nc.dram_tensor("lnfb", [P, NE], F32, kind="ExternalInput")
    d_blm = nc.dram_tensor("blm", [P, NVT], F32, kind="ExternalInput")
    d_out = nc.dram_tensor("logt", [V, S], BF16, kind="ExternalOutput")

    groups = [[0, 1, 2, 3], [4, 5, 6, 7]]

    with ExitStack() as ctx:
        tc = ctx.enter_context(tile.TileContext(nc, num_cores=NC))
        const = ctx.enter_context(tc.tile_pool(name="const", bufs=1))
        pp_x = ctx.enter_context(tc.tile_pool(name="xres", bufs=1))
        pp_sum = ctx.enter_context(tc.tile_pool(name="xsum", bufs=1))
        pp_msk = ctx.enter_context(tc.tile_pool(name="masks", bufs=1))
        tp = ctx.enter_context(tc.tile_pool(name="tp", bufs=1))
        pacc = ctx.enter_context(tc.tile_pool(name="pacc", bufs=1, space="PSUM"))
        dram = ctx.enter_context(tc.tile_pool(name="ccdram", bufs=2, space="DRAM"))
        pools = {"pacc": pacc, "tp": tp}

        # ---- embedding gather first: critical path at startup ----
        idx_t = const.tile([P, S // P], I32, name="idx_t")
        nc.gpsimd.dma_start(out=idx_t[:], in_=d_idx.ap().rearrange("(g p) -> p g", p=P))

        # queue all four gathers on the Pool engine before anything else
        xgs = []
        for g in range(S // P):
            xg = tp.tile([P, E], BF16, tag="embg", name="embg", bufs=4)
            nc.gpsimd.indirect_dma_start(
                out=xg[:], out_offset=None, in_=d_emb.ap(),
                in_offset=bass.IndirectOffsetOnAxis(ap=idx_t[:, g:g + 1], axis=0))
            xgs.append(xg)

        ident = const.tile([P, P], F32, name="ident")
        make_identity(nc, ident[:])
        identb = const.tile([P, P], BF16, name="identb")
        nc.vector.tensor_copy(identb[:], ident[:])
        ones_f = const.tile([P, P], F32, name="ones_f")
        nc.vector.memset(ones_f[:], 1.0)
        ones_bc = const.tile([P, HD], F32, name="ones_bc")
        nc.vector.memset(ones_bc[:], 1.0)
        eps_t = const.tile([P, 1], F32, name="eps_t")
        nc.vector.memset(eps_t[:], EPS)

        # residual stream xT: 8 tiles [128, S]
        xT = [pp_x.tile([P, S], F32, tag=f"x{e}", name=f"x{e}") for e in range(NE)]
        sum_t = [pp_sum.tile([P, S], F32, tag=f"s{e}", name=f"s{e}") for e in range(NE)]

        for g in range(S // P):
            for e in range(NE):
                pst = pacc.tile([P, S], BF16, tag="ps_a", name="tpose", bufs=3)
                nc.tensor.transpose(pst[:, 0:P], xgs[g][:, e * P:(e + 1) * P],
                                    identb[:])
                nc.vector.tensor_copy(xT[e][:, g * P:(g + 1) * P], pst[:, 0:P])

        # ---- params -> [128, n] tiles (after the gather chain is queued) ----
        def ldvec(dt_ap, n, name):
            t = const.tile([P, n], F32, tag=name, name=name)
            nc.gpsimd.dma_start(out=t[:], in_=dt_ap)
            return t

        t_ln1g = [ldvec(d_ln1g.ap()[l], NE, f"ln1g{l}") for l in range(L)]
        t_ln1b = [ldvec(d_ln1b.ap()[l], NE, f"ln1b{l}") for l in range(L)]
        t_ln2g = [ldvec(d_ln2g.ap()[l], NE, f"ln2g{l}") for l in range(L)]
        t_ln2b = [ldvec(d_ln2b.ap()[l], NE, f"ln2b{l}") for l in range(L)]
        t_bo = [ldvec(d_bo.ap()[l], NE, f"bo{l}") for l in range(L)]
        t_b1 = [ldvec(d_b1.ap()[l], NFT, f"b1{l}") for l in range(L)]
        t_b2 = [ldvec(d_b2.ap()[l], NE, f"b2{l}") for l in range(L)]
        t_lnfg = ldvec(d_lnfg.ap(), NE, "lnfg")
        t_lnfb = ldvec(d_lnfb.ap(), NE, "lnfb")
        t_blm = ldvec(d_blm.ap(), NVT, "blm")

        # causal masks (bf16 multiplicative, per-core data): one DMA
        mask3 = pp_msk.tile([P, NSB * S], BF16, tag="msk", name="msk")
        nc.gpsimd.dma_start(out=mask3[:].rearrange("p (sb s) -> p sb s", sb=NSB),
                    in_=d_msk.ap().rearrange("sb p s -> p sb s"))

        # ---- transformer blocks ----
        for l in range(L):
            with tc.tile_pool(name="qT", bufs=1) as pp_q, \
                 tc.tile_pool(name="oT", bufs=1) as pp_o, \
                 tc.tile_pool(name="attn", bufs=1) as ap_t, \
                 tc.tile_pool(name="vslab", bufs=1) as vp:

                # bf16 activations
                xA = []
                for e in range(NE):
                    xa = ap_t.tile([P, S], BF16, tag=f"xa{e}", name=f"xa{e}")
                    if e % 2 == 0:
                        nc.vector.tensor_copy(xa[:], xT[e][:])
                    else:
                        nc.scalar.copy(xa[:], xT[e][:])
                    xA.append(xa)

                # k/v projections feed pair-grouped transport buffers so the
                # AllGathers launch incrementally and overlap attention.
                # Group 0 ships k and v separately: scores for pairs 0-3 can
                # start as soon as their k rows arrive.
                kvcat = [dram.tile([GKSZ[0]], FP8, tag="kvcat0", name="kvcat0"),
                         dram.tile([GSZ[1]], FP8, tag="kvcat1", name="kvcat1"),
                         dram.tile([GSZ[2]], FP8, tag="kvcat2", name="kvcat2")]
                kvout = [dram.tile([CH * GKSZ[0]], FP8, tag="kvout0", name="kvout0"),
                         dram.tile([CH * GSZ[1]], FP8, tag="kvout1", name="kvout1"),
                         dram.tile([CH * GSZ[2]], FP8, tag="kvout2", name="kvout2")]
                c0v = dram.tile([S * GW[0]], FP8, tag="c0v", name="c0v")
                o0v = dram.tile([CH * S * GW[0]], FP8, tag="o0v", name="o0v")

                kslabs = {0: ap_t.tile([P, 4, S], FP8, tag="kle0", name="kle0"),
                          1: ap_t.tile([P, 3, S], FP8, tag="kle1", name="kle1"),
                          2: ap_t.tile([P, 1, S], FP8, tag="kle2", name="kle2")}

                def kproj(m, g, lm):
                    w = ap_t.tile([P, NE, P], BF16, tag="wb", name="wb", bufs=3)
                    nc.scalar.dma_start(out=w[:], in_=d_wkr.ap()[l, m])
                    ps = pacc.tile([P, S], F32, tag="ps_a", name="acc", bufs=3)
                    for k in range(NE):
                        nc.tensor.matmul(ps[:], lhsT=w[:, k, :], rhs=xA[k][:],
                                         start=(k == 0), stop=(k == NE - 1))
                    nc.scalar.mul(kslabs[g][:, lm, :], ps[:], KVS)

                def kflush(g, n):
                    nc.sync.dma_start(
                        out=kvcat[g][0:n * P * S].rearrange(
                            "(lm p s) -> p lm s", lm=n, p=P),
                        in_=kslabs[g][:, 0:n, :])

                def vproj(n):
                    # psv halves cover tokens tb*128.., head cols n*512..n*512+512
                    ps2 = [pacc.tile([P, 2, 512], F32, tag="ps_a", name="acc", bufs=3)
                           for _ in range(2)]
                    psv = [ps2[0][:, 0, :], ps2[0][:, 1, :],
                           ps2[1][:, 0, :], ps2[1][:, 1, :]]
                    for k in range(NE):
                        wv = ap_t.tile([P, 512], BF16, tag="wv", name="wv", bufs=3)
                        nc.scalar.dma_start(
                            out=wv[:],
                            in_=d_wvf.ap()[l, k, :, n * 512:(n + 1) * 512])
                        for m in range(4):
                            nc.tensor.matmul(psv[m], lhsT=xA[k][:, m * P:(m + 1) * P],
                                             rhs=wv[:], start=(k == 0), stop=(k == NE - 1))
                    for tb in range(4):
                        vv = tp.tile([P, 512], FP8, tag="vlocal", name="vlocal", bufs=2)
                        nc.scalar.mul(vv[:], psv[tb], KVS)
                        if n == 0:  # all 512 cols belong to group 0
                            nc.sync.dma_start(
                                out=c0v[tb * P * GW[0]:(tb + 1) * P * GW[0]]
                                .rearrange("(p f) -> p f", p=P),
                                in_=vv[:])
                        else:  # cols 0-383 -> group 1, cols 384-511 -> group 2
                            nc.sync.dma_start(
                                out=kvcat[1][GKSZ[1] + tb * P * GW[1]:
                                             GKSZ[1] + (tb + 1) * P * GW[1]]
                                .rearrange("(p f) -> p f", p=P),
                                in_=vv[:, 0:GW[1]])
                            nc.sync.dma_start(
                                out=kvcat[2][GKSZ[2] + tb * P * GW[2]:
                                             GKSZ[2] + (tb + 1) * P * GW[2]]
                                .rearrange("(p f) -> p f", p=P),
                                in_=vv[:, GW[1]:512])

                for m in range(4):
                    kproj(m, 0, m)
                kflush(0, 4)
                nc.gpsimd.collective_compute(
                    "AllGather", OP.bypass, replica_groups=groups,
                    ins=[kvcat[0][:]], outs=[kvout[0][:]])
                vproj(0)
                nc.gpsimd.collective_compute(
                    "AllGather", OP.bypass, replica_groups=groups,
                    ins=[c0v[:]], outs=[o0v[:]])
                for m in range(4, 7):
                    kproj(m, 1, m - 4)
                kflush(1, 3)
                kproj(7, 2, 0)
                kflush(2, 1)
                vproj(1)
                nc.gpsimd.collective_compute(
                    "AllGather", OP.bypass, replica_groups=groups,
                    ins=[kvcat[1][:]], outs=[kvout[1][:]])
                nc.gpsimd.collective_compute(
                    "AllGather", OP.bypass, replica_groups=groups,
                    ins=[kvcat[2][:]], outs=[kvout[2][:]])

                qT = []
                for m in range(NE):
                    w = ap_t.tile([P, NE, P], BF16, tag="wb", name="wb", bufs=3)
                    nc.scalar.dma_start(out=w[:], in_=d_wqr.ap()[l, m])
                    ps = pacc.tile([P, S], F32, tag="ps_a", name="acc", bufs=3)
                    for k in range(NE):
                        nc.tensor.matmul(ps[:], lhsT=w[:, k, :], rhs=xA[k][:],
                                         start=(k == 0), stop=(k == NE - 1))
                    q = pp_q.tile([P, S], BF16, tag=f"q{m}", name=f"q{m}")
                    nc.scalar.copy(q[:], ps[:])
                    qT.append(q)

                # attention: scores pair-major, AV per head with the softmax
                # denominator fused as a ones column in the v slab
                oT = []
                for p in range(NHP):
                    g = 0 if p < 4 else (1 if p < 7 else 2)
                    lm = p - PGRP[g][0]
                    pav_e = pacc.tile([65, S], F32, tag="ps_b", name="pave", bufs=2)
                    pav_o = pacc.tile([65, S], F32, tag="ps_b", name="pavo", bufs=2)
                    vsl8 = vp.tile([P, CH, 4, P], FP8, tag="vsl8", name="vsl8", bufs=2)
                    for cc in range(CH):
                        if g == 0:
                            vsrc, vbase = o0v, cc * S * GW[0]
                        else:
                            vsrc, vbase = kvout[g], cc * GSZ[g] + GKSZ[g]
                        nc.sync.dma_start(
                            out=vsl8[:, cc, :, :],
                            in_=vsrc[vbase:vbase + S * GW[g]].rearrange(
                                "(tb q f) -> q tb f", tb=4, q=P)[
                                :, :, lm * P:(lm + 1) * P])
                    # [v_even(64) | 1 | v_odd(64) | 1] per key block
                    vslab = vp.tile([P, CH, 4, 130], BF16, tag="vsl", name="vsl",
                                    bufs=1)
                    nc.vector.tensor_scalar(vslab[:, :, :, 0:HD],
                                            vsl8[:, :, :, 0:HD],
                                            1.0 / KVS, None, OP.mult)
                    nc.vector.tensor_scalar(vslab[:, :, :, 65:65 + HD],
                                            vsl8[:, :, :, HD:P],
                                            1.0 / KVS, None, OP.mult)
                    nc.vector.memset(vslab[:, :, :, HD:HD + 1], 1.0)
                    nc.vector.memset(vslab[:, :, :, 129:130], 1.0)
                    for cc in range(CH):
                        kbase = (cc * GKSZ[0] if g == 0 else cc * GSZ[g]) + lm * P * S
                        ksl8 = ap_t.tile([P, S], FP8, tag="ksl8", name="ksl8", bufs=3)
                        nc.sync.dma_start(
                            out=ksl8[:],
                            in_=kvout[g][kbase:kbase + P * S]
                            .rearrange("(p s) -> p s", p=P))
                        ksl = ap_t.tile([P, S], BF16, tag="kslab", name="kslab", bufs=3)
                        nc.vector.tensor_scalar(ksl[:], ksl8[:], 1.0 / KVS, None, OP.mult)
                        for jh in range(2):
                            sb0 = 4 * cc + 2 * jh
                            sA = pacc.tile([P, 2, S], F32, tag="ps_a", name="sc",
                                           bufs=3)
                            sB = pacc.tile([P, 2, S], F32, tag="ps_a", name="sc",
                                           bufs=3)
                            for d in range(2):
                                j2 = 2 * jh + d
                                nc.tensor.matmul(
                                    sA[:, d, :], lhsT=ksl[0:HD, j2 * P:(j2 + 1) * P],
                                    rhs=qT[p][0:HD, :], tile_position=(0, 0))
                                nc.tensor.matmul(
                                    sB[:, d, :], lhsT=ksl[HD:P, j2 * P:(j2 + 1) * P],
                                    rhs=qT[p][HD:P, :], tile_position=(64, 0))
                            pa = ap_t.tile([P, 2 * S], BF16, tag="pt", name="pt", bufs=4)
                            pb = ap_t.tile([P, 2 * S], BF16, tag="pt", name="pt", bufs=4)
                            nc.scalar.activation(pa[:], sA[:], AF.Exp, scale=HD ** -0.5)
                            nc.scalar.activation(pb[:], sB[:], AF.Exp, scale=HD ** -0.5)
                            nc.vector.tensor_tensor(out=pa[:], in0=pa[:],
                                                    in1=mask3[:, sb0 * S:(sb0 + 2) * S],
                                                    op=OP.mult)
                            nc.vector.tensor_tensor(out=pb[:], in0=pb[:],
                                                    in1=mask3[:, sb0 * S:(sb0 + 2) * S],
                                                    op=OP.mult)
                            for d in range(2):
                                sb = sb0 + d
                                first, last = (sb == 0), (sb == NSB - 1)
                                j2 = 2 * jh + d
                                nc.tensor.matmul(
                                    pav_e[:], lhsT=vslab[:, cc, j2, 0:65],
                                    rhs=pa[:, d * S:(d + 1) * S], start=first,
                                    stop=last, skip_group_check=True)
                                nc.tensor.matmul(
                                    pav_o[:], lhsT=vslab[:, cc, j2, 65:130],
                                    rhs=pb[:, d * S:(d + 1) * S], start=first,
                                    stop=last, skip_group_check=True)
                    for hh, pav in ((0, pav_e), (1, pav_o)):
                        dd = tp.tile([65, S], F32, tag="dd", name="dd", bufs=2)
                        nc.vector.reciprocal(dd[64:65, :], pav[64:65, :])
                        rec_ps = pacc.tile([HD, S], F32, tag="ps_a", name="recp",
                                           bufs=3)
                        nc.tensor.matmul(rec_ps[:], lhsT=ones_bc[64:65, :],
                                         rhs=dd[64:65, :], tile_position=(64, 0))
                        rec_s = tp.tile([HD, S], F32, tag="rec_s", name="rec_s",
                                        bufs=2)
                        nc.vector.tensor_copy(rec_s[:], rec_ps[:])
                        o = pp_o.tile([HD, S], BF16, tag=f"o{2 * p + hh}",
                                      name=f"o{2 * p + hh}")
                        nc.vector.tensor_tensor(out=o[:], in0=pav[0:HD, :],
                                                in1=rec_s[:], op=OP.mult)
                        oT.append(o)

                # output projection + bias + residual (LN stats fed per tile)
                psm1, pss1 = _ln_start(pools)
                for m in range(NE):
                    w = ap_t.tile([HD, H, P], BF16, tag="wo", name="wo", bufs=3)
                    nc.scalar.dma_start(out=w[:], in_=d_wor.ap()[l, m])
                    ps = pacc.tile([P, S], F32, tag="ps_a", name="acc", bufs=3)
                    for h in range(H):
                        nc.tensor.matmul(ps[:], lhsT=w[:, h, :], rhs=oT[h][:],
                                         start=(h == 0), stop=(h == H - 1))
                    yv = tp.tile([P, S], F32, tag="yv", name="yv", bufs=2)
                    nc.scalar.add(yv[:], ps[:], t_bo[l][:, m:m + 1])
                    nc.vector.tensor_tensor(out=sum_t[m][:], in0=yv[:], in1=xT[m][:],
                                            op=OP.add)
                    _ln_feed(nc, pools, psm1, pss1, sum_t[m], m, ones_f)

            _ln_finish(nc, pools, psm1, pss1, sum_t, xT,
                       t_ln1g[l], t_ln1b[l], eps_t)

            # FFN (bf16 matmuls, fp32 psum + residual)
            with tc.tile_pool(name="ht", bufs=1) as pp_h, \
                 tc.tile_pool(name="xbf", bufs=1) as pp_xbf:
                # compensated-DR inputs for FFN1 (x is post-LN1, unit scale)
                xp1 = [pp_xbf.tile([P, 2, S], FP8, tag=f"xf{k}", name=f"xf{k}")
                       for k in range(NE)]
                rp1 = [pp_xbf.tile([P, 2, S], FP8, tag=f"rf{kp}", name=f"rf{kp}")
                       for kp in range(NE // 2)]
                for e in range(NE):
                    nc.vector.tensor_copy(xp1[e][:, 0, :], xT[e][:])
                    x8d = tp.tile([P, S], F32, tag="x8d", name="x8d", bufs=2)
                    nc.scalar.copy(x8d[:], xp1[e][:, 0, :])
                    rt = tp.tile([P, S], F32, tag="rt", name="rt", bufs=2)
                    nc.vector.tensor_tensor(out=rt[:], in0=xT[e][:], in1=x8d[:],
                                            op=OP.subtract)
                    nc.scalar.copy(rp1[e // 2][:, e % 2, :], rt[:])
                    nc.vector.tensor_scalar(xp1[e][:, 1, :], xp1[e][:, 0, :],
                                            1.0 / 16.0, None, OP.mult)
                hT = []
                for fg in range(NE):
                    w = pp_h.tile([P, 4, NE, 2, P], FP8, tag="w4", name="w4", bufs=3)
                    nc.scalar.dma_start(out=w[:], in_=d_w1r.ap()[l, fg])
                    for fi in range(4):
                        f = fg * 4 + fi
                        ps = pacc.tile([P, S], F32, tag="ps_a", name="acc", bufs=3)
                        for k in range(NE):
                            nc.tensor.matmul(
                                ps[:], lhsT=w[:, fi, k, :, :], rhs=xp1[k][:],
                                start=(k == 0), stop=False,
                                perf_mode=mybir.MatmulPerfMode.DoubleRow)
                        for kp in range(NE // 2):
                            nc.tensor.matmul(
                                ps[:], lhsT=w[:, fi, 2 * kp:2 * kp + 2, 0, :],
                                rhs=rp1[kp][:],
                                start=False, stop=(kp == NE // 2 - 1),
                                perf_mode=mybir.MatmulPerfMode.DoubleRow)
                        h = pp_h.tile([P, S], BF16, tag=f"h{f}", name=f"h{f}")
                        nc.scalar.activation(h[:], ps[:], AF.Relu,
                                             bias=t_b1[l][:, f:f + 1],
                                             scale=1.0 / 64.0)
                        hT.append(h)
                psm2, pss2 = _ln_start(pools)
                for m in range(NE):
                    w = pp_h.tile([P, NFT, P], BF16, tag="w32", name="w32", bufs=2)
                    nc.scalar.dma_start(out=w[:], in_=d_w2r.ap()[l, m])
                    ps = pacc.tile([P, S], F32, tag="ps_a", name="acc", bufs=3)
                    for f in range(NFT):
                        nc.tensor.matmul(ps[:], lhsT=w[:, f, :], rhs=hT[f][:],
                                         start=(f == 0), stop=(f == NFT - 1))
                    yv = tp.tile([P, S], F32, tag="yv", name="yv", bufs=2)
                    nc.scalar.add(yv[:], ps[:], t_b2[l][:, m:m + 1])
                    nc.vector.tensor_tensor(out=sum_t[m][:], in0=yv[:], in1=xT[m][:],
                                            op=OP.add)
                    _ln_feed(nc, pools, psm2, pss2, sum_t[m], m, ones_f)
            _ln_finish(nc, pools, psm2, pss2, sum_t, xT,
                       t_ln2g[l], t_ln2b[l], eps_t)

        # ---- final LN + lm_head ----
        # Error-compensated DoubleRow: W is shipped as interleaved fp8 pairs
        # (W8 = fp8(64W), Rw8 = fp8(16*(64W - W8))); x as fp8 x8 plus residual
        # r8 = fp8(x - x8). logits = [x8.(W8 + Rw8/16) + r8.W8]/64 via 12
        # half-rate DR matmuls per vocab tile (vs 8 full-rate bf16).
        _ln_tiles(nc, tc, pools, xT, sum_t, t_lnfg, t_lnfb, ones_f, eps_t)
        with tc.tile_pool(name="lg", bufs=1) as pp_lg, \
             tc.tile_pool(name="xb", bufs=1) as pp_xb:
            xpair = [pp_xb.tile([P, 2, S], FP8, tag=f"xp{k}", name=f"xp{k}")
                     for k in range(NE)]
            rpair = [pp_xb.tile([P, 2, S], FP8, tag=f"rp{kp}", name=f"rp{kp}")
                     for kp in range(NE // 2)]
            for e in range(NE):
                nc.vector.tensor_copy(xpair[e][:, 0, :], sum_t[e][:])
                x8d = tp.tile([P, S], F32, tag="x8d", name="x8d", bufs=2)
                nc.scalar.copy(x8d[:], xpair[e][:, 0, :])
                rt = tp.tile([P, S], F32, tag="rt", name="rt", bufs=2)
                nc.vector.tensor_tensor(out=rt[:], in0=sum_t[e][:], in1=x8d[:],
                                        op=OP.subtract)
                nc.vector.tensor_copy(rpair[e // 2][:, e % 2, :], rt[:])
                nc.vector.tensor_scalar(xpair[e][:, 1, :], xpair[e][:, 0, :],
                                        1.0 / 16.0, None, OP.mult)
            for vg in range(NVG):
                w = pp_lg.tile([P, VGT, NE, 2, P], FP8, tag="wlmt", name="wlmt",
                               bufs=3)
                nc.scalar.dma_start(out=w[:], in_=d_wlm.ap()[vg])
                lgc = pp_lg.tile([P, VGT, S], BF16, tag="lg", name="lg", bufs=3)
                for u in range(VGT):
                    vt = vg * VGT + u
                    ps = pacc.tile([P, S], F32, tag="ps_a", name="acc", bufs=3)
                    for k in range(NE):
                        nc.tensor.matmul(
                            ps[:], lhsT=w[:, u, k, :, :], rhs=xpair[k][:],
                            start=(k == 0), stop=False,
                            perf_mode=mybir.MatmulPerfMode.DoubleRow)
                    for kp in range(NE // 2):
                        nc.tensor.matmul(
                            ps[:], lhsT=w[:, u, 2 * kp:2 * kp + 2, 0, :],
                            rhs=rpair[kp][:],
                            start=False, stop=(kp == NE // 2 - 1),
                            perf_mode=mybir.MatmulPerfMode.DoubleRow)
                    nc.scalar.activation(lgc[:, u, :], ps[:], AF.Identity,
                                         bias=t_blm[:, vt:vt + 1], scale=1.0 / 64.0)
                nc.sync.dma_start(
                    out=d_out.ap()[vg * VGT * P:(vg + 1) * VGT * P, :]
                    .rearrange("(u p) s -> p u s", p=P),
                    in_=lgc[:])

    nc.compile()
    return nc


_CACHED = {}


def _sw(v):
    """[.., n*P] vector -> [.., P, n] partition-major fp32."""
    v = np.asarray(v, np.float32)
    return np.ascontiguousarray(
        v.reshape(*v.shape[:-1], v.shape[-1] // P, P).swapaxes(-1, -2))


def _prep_weights(inputs):
    f32 = np.float32
    bf = ml_dtypes.bfloat16
    Wq, Wk, Wv = inputs["Wq"], inputs["Wk"], inputs["Wv"]
    # [L,H,E,HD] -> [L,E,H*HD]
    wq_flat = np.ascontiguousarray(Wq.transpose(0, 2, 1, 3).reshape(L, E, H * HD))
    wk_flat = np.ascontiguousarray(Wk.transpose(0, 2, 1, 3).reshape(L, E, H * HD))
    wv_flat = np.ascontiguousarray(Wv.transpose(0, 2, 1, 3).reshape(L, E, H * HD))

    def sw_ee(w):  # [L, E_in, E_out] -> [L, m, p, k, f]
        return np.ascontiguousarray(
            w.reshape(L, NE, P, NE, P).transpose(0, 3, 2, 1, 4)).astype(bf)

    wqr = sw_ee(wq_flat)
    wkr = sw_ee(wk_flat)
    # Wo in per-head layout [l, m, d, h, f] so each head's 64 contraction rows
    # sit at partitions 0-63
    wor = np.ascontiguousarray(
        inputs["Wo"].reshape(L, H, HD, NE, P).transpose(0, 3, 2, 1, 4)).astype(bf)
    wvf = np.ascontiguousarray(
        wv_flat.reshape(L, NE, P, H * HD)).astype(bf)
    f8 = ml_dtypes.float8_e4m3
    w1_64 = (64.0 * inputs["W1"]).astype(f32)
    w1_8 = w1_64.astype(f8)
    rw1_8 = (16.0 * (w1_64 - w1_8.astype(f32))).astype(f8)
    w1r = np.ascontiguousarray(
        np.stack([w1_8.reshape(L, NE, P, NE, 4, P),
                  rw1_8.reshape(L, NE, P, NE, 4, P)],
                 axis=5).transpose(0, 3, 2, 4, 1, 5, 6))
    w2r = np.ascontiguousarray(
        inputs["W2"].reshape(L, NFT, P, NE, P).transpose(0, 3, 2, 1, 4)).astype(bf)
    f8 = ml_dtypes.float8_e4m3
    w64 = (64.0 * inputs["Wlm"]).astype(f32)
    w8 = w64.astype(f8)
    rw8 = (16.0 * (w64 - w8.astype(f32))).astype(f8)
    # interleave [k, p, g, u, f] pairs -> [g, p, u, k, pair, f]
    wlmr = np.ascontiguousarray(
        np.stack([w8.reshape(NE, P, NVG, VGT, P), rw8.reshape(NE, P, NVG, VGT, P)],
                 axis=4).transpose(2, 1, 3, 0, 4, 5))
    return {
        "wqr": wqr, "wkr": wkr, "wvf": wvf, "wor": wor,
        "w1r": w1r, "w2r": w2r, "wlmr": wlmr,
        "emb": np.ascontiguousarray(inputs["emb"]).astype(bf),
        "ln1g": _sw(inputs["ln1_g"]), "ln1b": _sw(inputs["ln1_b"]),
        "ln2g": _sw(inputs["ln2_g"]), "ln2b": _sw(inputs["ln2_b"]),
        "bo": _sw(inputs["bo"]), "b1": _sw(inputs["b1"]), "b2": _sw(inputs["b2"]),
        "lnfg": _sw(inputs["lnf_g"]), "lnfb": _sw(inputs["lnf_b"]),
        "blm": _sw(inputs["blm"]),
    }


def kernel(**inputs):
    if "nc" not in _CACHED:
        _CACHED["nc"] = build_program()
    nc = _CACHED["nc"]

    shared = _prep_weights(inputs)
    index = np.asarray(inputs["index"])

    # per-core causal masks and token ids
    jpos = np.arange(S)
    ipos = np.arange(P)
    in_maps = []
    for c in range(NC):
        b, j = c // CH, c % CH
        q0 = j * S
        m = np.zeros((NSB, P, S), np.float32)
        for sb in range(NSB):
            m[sb] = ((sb * P + ipos)[:, None] <= (q0 + jpos)[None, :])
        im = dict(shared)
        im["maskp"] = m.astype(ml_dtypes.bfloat16)
        im["idx"] = np.ascontiguousarray(index[b, q0:q0 + S]).astype(np.int32)
        in_maps.append(im)

    res = bass_utils.run_bass_kernel_spmd(nc, in_maps, core_ids=list(range(NC)))
    out = np.zeros((B, T, V), np.float32)
    for c in range(NC):
        b, j = c // CH, c % CH
        out[b, j * S:(j + 1) * S, :] = res.results[c]["logt"].astype(np.float32).T
    return out
